# revision 45
# baseline (speedup 1.0000x reference)
"""AdaptiveTokenMerger (ToMe block + merger) TRN2 Bass kernel.

Data-parallel over batch: 8 samples -> 8 NeuronCores, one sample per core.
Per-core pipeline (sample x [1024, 1024]):
  A (f32, ranking-critical): LN1 -> qkv -> MHA (transposed-softmax with the
    denominator folded in as an appended ones-column of v) -> Wo -> x_attn
  B: metric scores -> node_max/argmax -> ranks via pairwise comparisons ->
    dst scatter-add expressed as a one-hot matmul
  C (f32r): MLP over rows [x1_even(512); dst_new(512)], fused W1/W2 per
    token-quarter, output accumulated in PSUM across all 32 W1 column tiles
  D: pooling as a rank-dependent one-hot matmul -> Wp -> combined = 3q
  E (f32r): multi-query attention  F (f32r): FFN -> out [504, 1024]

Precision: everything upstream of the rank/argmax decisions is true fp32
(4 cyc/row on PE); post-merge matmuls use float32r (TF32-ish, 1 cyc/row).

PSUM budget (8 banks): BIGA/BIGB [128,1024] (2+2), MM [128,512] x2 (2),
AV [128,4,128] x2 (2).
"""
import numpy as np

import concourse.bass as bass
import concourse.tile as tile
from concourse import bacc, mybir
from concourse.bass import ts
from concourse.bass_utils import run_bass_kernel_spmd
from concourse.masks import make_identity

F32 = mybir.dt.float32
F32R = mybir.dt.float32r
U32 = mybir.dt.uint32

N, C, H = 1024, 1024, 16
R = 16
DH = C // H          # 64
NE = N // 2          # 512
NP = (N - R) // 2    # 504
PP = 126             # pooled tokens per partition tile
KT = C // 128        # 8
AF = mybir.ActivationFunctionType
OP = mybir.AluOpType

DEBUG = False


def build(debug=False):
    nc = bacc.Bacc("TRN2", target_bir_lowering=False, debug=False, num_devices=8)
    x_d = nc.dram_tensor("x", [N, C], F32, kind="ExternalInput").ap()
    wd = {}
    for name, shape in [
        ("g1", [C]), ("be1", [C]), ("Wqkv", [C, 3 * C]), ("bqkv", [3 * C]),
        ("Wo", [C, C]), ("bo", [C]), ("g2", [C]), ("be2", [C]),
        ("W1", [C, 4 * C]), ("bm1", [4 * C]), ("W2", [4 * C, C]), ("bm2", [C]),
        ("Wp", [C, C]), ("bp", [C]), ("Wq", [C, C]), ("bq", [C]),
        ("Wk", [C, DH]), ("bk", [DH]), ("Wv", [C, DH]), ("bv", [DH]),
        ("Wmo", [C, C]), ("bmo", [C]), ("Wf1", [C, 4 * C]), ("bf1", [4 * C]),
        ("Wf2", [4 * C, C]), ("bf2", [C]),
    ]:
        wd[name] = nc.dram_tensor(name, shape, F32, kind="ExternalInput").ap()
    out_d = nc.dram_tensor("out", [NP, C], F32, kind="ExternalOutput").ap()
    dbg = {}
    if debug:
        for name, shape in [
            ("dbg_xattn", [N, C]), ("dbg_nm", [NE]), ("dbg_rank", [NE]),
            ("dbg_nodeidx", [NE]), ("dbg_mlpin", [N, C]), ("dbg_mlpout", [N, C]),
            ("dbg_pooled", [NP, C]),
        ]:
            dbg[name] = nc.dram_tensor(name, shape, F32, kind="ExternalOutput").ap()
    with tile.TileContext(nc) as tc:
        _build_tile(nc, tc, x_d, wd, out_d, dbg)
    nc.compile()
    return nc


def _build_tile(nc, tc, x_d, wd, out_d, dbg):
    # DRAM spill buffers
    qkTd = nc.dram_tensor("qkTd", [2 * C, N], F32).ap()
    aoTd = nc.dram_tensor("aoTd", [C, N], F32).ap()
    x1d = nc.dram_tensor("x1d", [N, C], F32).ap()
    dstnd = nc.dram_tensor("dstnd", [NE, C], F32).ap()
    mqaTd = nc.dram_tensor("mqaTd", [C, NP], F32R).ap()

    pc = tc.alloc_tile_pool(name="const", bufs=1)
    psm = tc.alloc_tile_pool(name="small", bufs=1)
    pw = tc.alloc_tile_pool(name="wstream", bufs=2)
    pt = tc.alloc_tile_pool(name="tmp", bufs=2)
    pp = tc.alloc_tile_pool(name="psum", bufs=1, space="PSUM")

    _ct = {}

    def utile(pool, shape, dtype, tag, bufs=None):
        _ct[tag] = _ct.get(tag, 0) + 1
        kw = {"bufs": bufs} if bufs is not None else {}
        return pool.tile(shape, dtype, tag=tag, name=f"{tag}_{_ct[tag]}", **kw)

    def ps_bigA():
        return utile(pp, [128, 1024], F32, "BIGA")

    def ps_bigB():
        return utile(pp, [128, 1024], F32, "BIGB")

    def ps_mm():
        return utile(pp, [128, 512], F32, "MM", bufs=2)

    def ps_av():
        return utile(pp, [128, 4, 128], F32, "AV", bufs=2)

    # ---------- constants ----------
    ident = pc.tile([128, 128], F32)
    make_identity(nc, ident[:])
    ident_r = pc.tile([128, 128], F32R)
    nc.vector.tensor_copy(ident_r[:], ident[:])
    ones_col = pc.tile([1, 128], F32)
    nc.gpsimd.memset(ones_col[:], 1.0)
    ones_col_r = pc.tile([1, 128], F32R)
    nc.vector.tensor_copy(ones_col_r[:], ones_col[:])
    ones_part = pc.tile([128, 8], F32)
    nc.gpsimd.memset(ones_part[:], 1.0)
    piota = pc.tile([128, 1], F32)
    nc.gpsimd.iota(piota[:], [[0, 1]], channel_multiplier=1,
                   allow_small_or_imprecise_dtypes=True)
    iota512_row = pc.tile([1, 512], F32)
    nc.gpsimd.iota(iota512_row[:], [[1, 512]], channel_multiplier=0,
                   allow_small_or_imprecise_dtypes=True)
    iota504_row = pc.tile([1, 504], F32)
    nc.gpsimd.iota(iota504_row[:], [[1, 504]], channel_multiplier=0,
                   allow_small_or_imprecise_dtypes=True)

    def bcast_row(row_ap, n, tag, pool, scale=1.0):
        t = utile(pool, [128, n], F32, tag)
        for c0 in range(0, n, 512):
            cw = min(512, n - c0)
            p = ps_mm()
            nc.tensor.matmul(p[:, :cw], ones_col[:], row_ap[:, c0:c0 + cw],
                             start=True, stop=True)
            if scale == 1.0:
                nc.vector.tensor_copy(t[:, c0:c0 + cw], p[:, :cw])
            else:
                nc.vector.tensor_scalar_mul(t[:, c0:c0 + cw], p[:, :cw], scale)
        return t

    def load_row(dram_ap, n, tag, pool):
        t = utile(pw, [1, n], F32, "rowstg", bufs=1)
        nc.sync.dma_start(t[:], dram_ap[None, :])
        return t

    def brow(name, pool, scale=1.0):
        n = wd[name].shape[0]
        return bcast_row(load_row(wd[name], n, name + "_r", pool), n,
                         name + "_b", pool, scale)

    def bcol(name, pool, scale=1.0):
        n = wd[name].shape[0]
        t = utile(pool, [128, n // 128], F32, name + "_c")
        nc.sync.dma_start(t[:], wd[name].rearrange("(t p) -> p t", p=128))
        if scale != 1.0:
            nc.vector.tensor_scalar_mul(t[:], t[:], scale)
        return t

    IOTA512B = bcast_row(iota512_row[:], 512, "iota512b", pc)
    IOTA504B = bcast_row(iota504_row[:], 504, "iota504b", pc)

    def transpose_blocks(src_tiles, dst, n_rows, n_cols):
        """dst[c, r] = src[r, c]; dst is tile-list or sink(bj, bi, pf, cw, rw)."""
        for bi in range((n_rows + 127) // 128):
            rw = min(128, n_rows - bi * 128)
            for bj in range((n_cols + 127) // 128):
                cw = min(128, n_cols - bj * 128)
                p = ps_av()
                pf = p.rearrange("p a b -> p (a b)")
                nc.tensor.transpose(pf[:cw, :rw],
                                    src_tiles[bi][:rw, bj * 128:bj * 128 + cw],
                                    ident[:rw, :rw])
                if callable(dst):
                    dst(bj, bi, pf, cw, rw)
                else:
                    nc.vector.tensor_copy(dst[bj][:cw, bi * 128:bi * 128 + rw],
                                          pf[:cw, :rw])

    def refined_rsqrt_recip(vv, tag):
        """returns 1/sqrt(vv) with one Newton step on sqrt (ACT sqrt is loose)."""
        s0 = utile(psm, [128, 1], F32, tag + "_s0")
        nc.scalar.sqrt(s0[:], vv[:])
        r0 = utile(psm, [128, 1], F32, tag + "_r0")
        nc.vector.reciprocal(r0[:], s0[:])
        t = utile(psm, [128, 1], F32, tag + "_t")
        nc.vector.tensor_tensor(t[:], vv[:], r0[:], OP.mult)
        nc.vector.tensor_tensor(t[:], t[:], s0[:], OP.add)
        nc.vector.tensor_scalar_mul(t[:], t[:], 0.5)
        rr = utile(psm, [128, 1], F32, tag + "_rr")
        nc.vector.reciprocal(rr[:], t[:])
        return rr

    def layer_norm(src, dst, gb, bb, lnpool):
        m = utile(psm, [128, 1], F32, "ln_m")
        nc.vector.reduce_sum(m[:], src[:, :C], axis=mybir.AxisListType.X)
        nc.vector.tensor_scalar_mul(m[:], m[:], 1.0 / C)
        xc = utile(lnpool, [128, C], F32, "ln_xc", bufs=2)
        nc.vector.tensor_scalar(xc[:], src[:, :C], m[:], None, OP.subtract)
        ss = utile(psm, [128, 1], F32, "ln_ss")
        nc.scalar.activation(dst[:, :C], xc[:], AF.Square, accum_out=ss[:])
        v = utile(psm, [128, 1], F32, "ln_v")
        nc.vector.tensor_scalar(v[:], ss[:], 1.0 / C, 1e-5, OP.mult, OP.add)
        rstd = refined_rsqrt_recip(v, "ln")
        nc.vector.tensor_scalar(dst[:, :C], xc[:], rstd[:], None, OP.mult)
        nc.vector.tensor_tensor(dst[:, :C], dst[:, :C], gb[:], OP.mult)
        nc.vector.tensor_tensor(dst[:, :C], dst[:, :C], bb[:], OP.add)

    # ================= Stage A: LN1 -> hT =================
    pbA = tc.alloc_tile_pool(name="biasA", bufs=1)
    pHT = tc.alloc_tile_pool(name="pHT", bufs=1)
    pVP = tc.alloc_tile_pool(name="pVP", bufs=1)
    pAttn = tc.alloc_tile_pool(name="pAttn", bufs=1)

    g1b = brow("g1", pbA)
    be1b = brow("be1", pbA)
    hT = [utile(pHT, [128, N], F32, f"hT{k}") for k in range(8)]
    ht = []
    for i in range(8):
        xt = utile(pt, [128, C], F32, "xin")
        nc.sync.dma_start(xt[:], x_d[ts(i, 128), :])
        h = utile(pHT, [128, C], F32, "ht", bufs=4)
        layer_norm(xt, h, g1b, be1b, pbA)
        ht.append(h)
    transpose_blocks(ht, hT, N, C)

    # ===== qk^T -> qkTd (DRAM) ; v_pad (SBUF) =====
    bqkT = bcol("bqkv", pbA)
    for mp in range(8):
        accq = ps_bigA()
        acck = ps_bigB()
        for k in range(KT):
            wq = utile(pw, [128, 128], F32, "wqkb", bufs=6)
            nc.sync.dma_start(wq[:], wd["Wqkv"][ts(k, 128), ts(mp, 128)])
            wk = utile(pw, [128, 128], F32, "wqkb", bufs=6)
            nc.sync.dma_start(wk[:],
                              wd["Wqkv"][ts(k, 128), C + mp * 128:C + (mp + 1) * 128])
            for n2 in range(2):
                nc.tensor.matmul(accq[:, ts(n2, 512)], wq[:], hT[k][:, ts(n2, 512)],
                                 start=(k == 0), stop=(k == KT - 1))
                nc.tensor.matmul(acck[:, ts(n2, 512)], wk[:], hT[k][:, ts(n2, 512)],
                                 start=(k == 0), stop=(k == KT - 1))
        stgq = utile(pAttn, [128, N], F32, "qkstg", bufs=2)
        nc.scalar.activation(stgq[:], accq[:], AF.Identity, bias=bqkT[:, mp:mp + 1])
        nc.sync.dma_start(qkTd[ts(mp, 128), :], stgq[:])
        stgk = utile(pAttn, [128, N], F32, "qkstg", bufs=2)
        nc.scalar.activation(stgk[:], acck[:], AF.Identity,
                             bias=bqkT[:, 8 + mp:9 + mp])
        nc.sync.dma_start(qkTd[C + mp * 128:C + (mp + 1) * 128, :], stgk[:])

    bvqkvb = bcast_row(load_row(wd["bqkv"][2 * C:], C, "bvq_r", pbA), C,
                       "bvq_b", pbA)
    v_pad = [utile(pVP, [128, H, DH + 1], F32, f"vp{j}") for j in range(8)]
    for j in range(8):
        nc.vector.memset(v_pad[j][:, :, DH:DH + 1], 1.0)
    for jg in range(4):
        accs = [ps_bigA(), ps_bigB()]
        for k in range(KT):
            wv = utile(pVP, [128, C], F32, "wv", bufs=3)
            nc.sync.dma_start(wv[:], wd["Wqkv"][ts(k, 128), 2 * C:])
            for jj in range(2):
                j = jg * 2 + jj
                for n2 in range(2):
                    nc.tensor.matmul(accs[jj][:, ts(n2, 512)],
                                     hT[k][:, ts(j, 128)], wv[:, ts(n2, 512)],
                                     start=(k == 0), stop=(k == KT - 1))
        for jj in range(2):
            j = jg * 2 + jj
            for h in range(H):
                nc.vector.tensor_tensor(v_pad[j][:, h, :DH],
                                        accs[jj][:, ts(h, DH)],
                                        bvqkvb[:, ts(h, DH)], OP.add)

    # ===== attention: stream kT/qT per head; out -> aoTd (already c-major) ==
    # out[dh|sum, i] = v_pad[j].T @ expT[j, i], accumulated over j-tiles.
    for h in range(H):
        kth = utile(pAttn, [64, N], F32, "kth", bufs=2)
        nc.sync.dma_start(kth[:], qkTd[C + h * 64:C + h * 64 + 64, :])
        qth = utile(pAttn, [64, N], F32, "qth", bufs=2)
        nc.sync.dma_start(qth[:], qkTd[h * 64:h * 64 + 64, :])
        av = [ps_av().rearrange("p a b -> p (a b)") for _ in range(2)]
        for j in range(8):
            for n2 in range(2):
                sp = ps_mm()
                nc.tensor.matmul(sp[:], kth[:, ts(j, 128)], qth[:, ts(n2, 512)],
                                 start=True, stop=True)
                et = utile(pAttn, [128, 512], F32, "exp", bufs=3)
                nc.scalar.activation(et[:], sp[:], AF.Exp, scale=float(DH ** -0.5))
                nc.tensor.matmul(av[n2][:DH + 1, :512], v_pad[j][:, h, :], et[:],
                                 start=(j == 0), stop=(j == 7))
        for n2 in range(2):
            rrow = utile(pAttn, [1, 512], F32, "rrow", bufs=2)
            nc.vector.reciprocal(rrow[:], av[n2][DH:DH + 1, :512])
            rb = ps_mm()
            nc.tensor.matmul(rb[:DH, :512], ones_col[:, :DH], rrow[:],
                             start=True, stop=True)
            rbs = utile(pAttn, [64, 512], F32, "rbs", bufs=2)
            nc.vector.tensor_copy(rbs[:], rb[:DH, :512])
            stg = utile(pAttn, [64, 512], F32, "aot_stg", bufs=2)
            nc.vector.tensor_tensor(stg[:], av[n2][:DH, :512], rbs[:],
                                    OP.mult)
            nc.sync.dma_start(aoTd[h * 64:h * 64 + 64, ts(n2, 512)], stg[:])
    pAttn.release()
    pVP.release()
    pHT.release()
    pbA.release()

    # ================= Wo -> x_attn, x1 (-> DRAM), metric =================
    pB2 = tc.alloc_tile_pool(name="pB2", bufs=1)
    bob = brow("bo", pB2)
    xa = [utile(pB2, [128, C], F32, f"xa{m}") for m in range(8)]
    woR = [utile(pB2, [128, C], F32, f"woR{k}") for k in range(8)]
    for k in range(KT):
        nc.sync.dma_start(woR[k][:], wd["Wo"][ts(k, 128), :])
    rn = psm.tile([128, 8], F32)
    for m in range(8):
        acc = ps_bigA()
        for k in range(KT):
            ao = utile(pw, [128, 128], F32, "wqkb", bufs=6)
            nc.sync.dma_start(ao[:], aoTd[ts(k, 128), ts(m, 128)])
            for n2 in range(2):
                nc.tensor.matmul(acc[:, ts(n2, 512)], ao[:],
                                 woR[k][:, ts(n2, 512)],
                                 start=(k == 0), stop=(k == KT - 1))
        nc.vector.tensor_tensor(xa[m][:], acc[:], bob[:], OP.add)
        xt = utile(pt, [128, C], F32, "xin")
        nc.sync.dma_start(xt[:], x_d[ts(m, 128), :])
        x1stg = utile(pw, [128, C], F32, "x1stg", bufs=2)
        nc.vector.tensor_tensor(x1stg[:], xa[m][:], xt[:], OP.add)
        nc.sync.dma_start(x1d[ts(m, 128), :], x1stg[:])
        ss = utile(psm, [128, 1], F32, "nrm_ss")
        sq = utile(pt, [128, C], F32, "ln_xc")
        nc.scalar.activation(sq[:], xa[m][:], AF.Square, accum_out=ss[:])
        rr = refined_rsqrt_recip(ss, "nrm")
        nc.vector.tensor_copy(rn[:, m:m + 1], rr[:])
        nc.vector.tensor_scalar(xa[m][:], xa[m][:], rn[:, m:m + 1], None, OP.mult)
        if dbg:
            nc.sync.dma_start(dbg["dbg_xattn"][ts(m, 128), :], xa[m][:])

    # ===== de-interleave metric -> maT/mbT; scores; node stats; ranks =====
    pB3 = tc.alloc_tile_pool(name="pB3", bufs=1)
    xae = [utile(pB3, [128, C], F32, f"xae{m}") for m in range(4)]
    xao = [utile(pB3, [128, C], F32, f"xao{m}") for m in range(4)]
    for m in range(4):
        nc.sync.dma_start(xae[m][:64, :], xa[2 * m][0:128:2, :])
        nc.sync.dma_start(xae[m][64:, :], xa[2 * m + 1][0:128:2, :])
        nc.sync.dma_start(xao[m][:64, :], xa[2 * m][1:128:2, :])
        nc.sync.dma_start(xao[m][64:, :], xa[2 * m + 1][1:128:2, :])
    pB4 = tc.alloc_tile_pool(name="pB4", bufs=1)
    maT = [utile(pB4, [128, NE], F32, f"maT{k}") for k in range(8)]
    mbT = [utile(pB4, [128, NE], F32, f"mbT{k}") for k in range(8)]
    transpose_blocks(xae, maT, NE, C)
    transpose_blocks(xao, mbT, NE, C)

    nm_t = psm.tile([128, 4], F32)
    ni_t = psm.tile([128, 4], F32)
    for m in range(4):
        acc = ps_bigA()
        for k in range(KT):
            nc.tensor.matmul(acc[:, :512], maT[k][:, ts(m, 128)], mbT[k][:],
                             start=(k == 0), stop=(k == KT - 1))
        mx8 = utile(psm, [128, 8], F32, "mx8")
        ix8 = utile(psm, [128, 8], U32, "ix8")
        nc.vector.max_with_indices(mx8[:], ix8[:], acc[:, :512])
        nc.vector.tensor_copy(nm_t[:, m:m + 1], mx8[:, 0:1])
        nc.vector.tensor_copy(ni_t[:, m:m + 1], ix8[:, 0:1])

    nm_row = utile(pB4, [1, 512], F32, "nm_row")
    for m in range(4):
        p = ps_av()
        pf = p.rearrange("p a b -> p (a b)")
        nc.tensor.transpose(pf[:1, :128], nm_t[:, m:m + 1], ident[:])
        nc.vector.tensor_copy(nm_row[:, ts(m, 128)], pf[:1, :128])
    NMB = bcast_row(nm_row[:], 512, "nmb", pB4)

    rank_t = psm.tile([128, 4], F32)
    for m in range(4):
        gt = utile(pB4, [128, 512], F32, "rk_gt", bufs=1)
        nc.vector.tensor_scalar(gt[:], NMB[:], nm_t[:, m:m + 1], None, OP.is_gt)
        eq = utile(pB4, [128, 512], F32, "rk_eq", bufs=1)
        nc.vector.tensor_scalar(eq[:], NMB[:], nm_t[:, m:m + 1], None, OP.is_equal)
        flt = utile(pB4, [128, 512], F32, "rk_flt", bufs=1)
        pio = utile(psm, [128, 1], F32, "rk_pio")
        nc.vector.tensor_scalar_add(pio[:], piota[:], float(128 * m))
        nc.vector.tensor_scalar(flt[:], IOTA512B[:], pio[:], None, OP.is_lt)
        nc.vector.tensor_tensor(eq[:], eq[:], flt[:], OP.mult)
        nc.vector.tensor_tensor(gt[:], gt[:], eq[:], OP.add)
        nc.vector.reduce_sum(rank_t[:, m:m + 1], gt[:], axis=mybir.AxisListType.X)
    if dbg:
        for (tt, nme) in [(nm_t, "dbg_nm"), (rank_t, "dbg_rank"),
                          (ni_t, "dbg_nodeidx")]:
            nc.sync.dma_start(dbg[nme].rearrange("(m p) -> p m", p=128), tt[:])
    pB4.release()
    pB3.release()
    pB2.release()

    # ================= dst merge (x1 from DRAM; dstn -> DRAM) =============
    pM = tc.alloc_tile_pool(name="pM", bufs=1)
    x1e = [utile(pM, [128, C + 8], F32R, f"x1e{m}") for m in range(4)]
    x1o = [utile(pM, [128, C], F32, f"x1o{m}") for m in range(4)]
    for m in range(4):
        nc.vector.tensor_copy(x1e[m][:, C:C + 8], ones_part[:])
        nc.sync.dma_start(x1e[m][:, :C],
                          x1d[256 * m:256 * m + 256:2, :].bitcast(F32R))
        nc.sync.dma_start(x1o[m][:], x1d[256 * m + 1:256 * m + 256:2, :])
    st = [utile(pM, [128, 512], F32R, f"st{m}") for m in range(4)]
    for m in range(4):
        msk = utile(psm, [128, 1], F32, "st_m")
        nc.vector.tensor_scalar(msk[:], rank_t[:, m:m + 1], float(R) - 0.5, None,
                                OP.is_lt)
        nc.vector.tensor_scalar(st[m][:], IOTA512B[:], ni_t[:, m:m + 1], None,
                                OP.is_equal)
        nc.vector.tensor_scalar(st[m][:], st[m][:], msk[:], None, OP.mult)
    for m in range(4):
        acc = ps_bigA()
        cacc = ps_av()
        for k in range(4):
            for n2 in range(2):
                nc.tensor.matmul(acc[:, ts(n2, 512)], st[k][:, ts(m, 128)],
                                 x1e[k][:, n2 * 512:n2 * 512 + 512],
                                 start=(k == 0), stop=(k == 3))
            nc.tensor.matmul(cacc[:, 0, :8], st[k][:, ts(m, 128)],
                             x1e[k][:, C:C + 8], start=(k == 0), stop=(k == 3))
        cnt = utile(psm, [128, 1], F32, "cnt")
        nc.vector.tensor_scalar_add(cnt[:], cacc[:, 0, 0:1], 1.0)
        rec = utile(psm, [128, 1], F32, "cntr")
        nc.vector.reciprocal(rec[:], cnt[:])
        dst_stg = utile(pM, [128, C], F32, "dst_stg", bufs=2)
        nc.vector.tensor_tensor(dst_stg[:], acc[:], x1o[m][:], OP.add)
        nc.vector.tensor_scalar(dst_stg[:], dst_stg[:], rec[:], None, OP.mult)
        nc.sync.dma_start(dstnd[ts(m, 128), :], dst_stg[:])

    # ========== MLP (f32r): W1/W2 streamed once; SBUF out accumulation ======
    def row_src_ap(i):
        if i < 4:
            return x1d[256 * i:256 * i + 256:2, :]
        return dstnd[ts(i - 4, 128), :]

    pM.release()
    pZ = tc.alloc_tile_pool(name="pZ", bufs=1)
    pC4 = tc.alloc_tile_pool(name="pC4", bufs=1)
    g2b = brow("g2", pC4)
    be2b = brow("be2", pC4)
    h2 = []
    for i in range(8):
        rsrc = utile(pt, [128, C], F32, "xin")
        nc.sync.dma_start(rsrc[:], row_src_ap(i))
        h = utile(pC4, [128, C], F32, "ht", bufs=3)
        layer_norm(rsrc, h, g2b, be2b, pC4)
        h2.append(h)
        if dbg:
            nc.sync.dma_start(dbg["dbg_mlpin"][ts(i, 128), :], rsrc[:])
    h2T = [utile(pC4, [128, N], F32R, f"h2T{k}") for k in range(8)]
    transpose_blocks(h2, h2T, N, C)

    bm1T = bcol("bm1", pC4)
    bm2b = brow("bm2", pC4)
    # ---- MLP c-major: Y^T groups of 8 hidden tiles, Z^T accum in SBUF ----
    zacc = [utile(pZ, [128, N], F32R, f"zacc{c}") for c in range(8)]
    for g in range(4):
        ytiles = []
        w1h = None
        for mi in range(8):
            mt = g * 8 + mi
            if mi % 4 == 0:
                w1h = []
                for k in range(KT):
                    w = utile(pC4, [128, 512], F32R, "w1h", bufs=8)
                    nc.sync.dma_start(
                        w[:], wd["W1"][ts(k, 128), ts(2 * g + mi // 4, 512)]
                        .bitcast(F32R))
                    w1h.append(w)
            yp = ps_bigA() if mi % 2 == 0 else ps_bigB()
            for k in range(KT):
                for n2 in range(2):
                    nc.tensor.matmul(yp[:, ts(n2, 512)],
                                     w1h[k][:, ts(mi % 4, 128)],
                                     h2T[k][:, ts(n2, 512)],
                                     start=(k == 0), stop=(k == KT - 1))
            yt = utile(pC4, [128, N], F32R, "yt", bufs=8)
            nc.scalar.activation(yt[:], yp[:], AF.Gelu_apprx_tanh,
                                 bias=bm1T[:, mt:mt + 1])
            ytiles.append(yt)
        for ch in range(2):
            w2h = []
            for mi in range(8):
                mt = g * 8 + mi
                w = utile(pC4, [128, 512], F32R, "w2h", bufs=8)
                nc.sync.dma_start(w[:],
                                  wd["W2"][ts(mt, 128), ts(ch, 512)].bitcast(F32R))
                w2h.append(w)
            for cc in range(4):
                c = ch * 4 + cc
                for n2 in range(2):
                    zp = ps_mm() if n2 == 0 else                         ps_av().rearrange("p a b -> p (a b)")
                    for mi in range(8):
                        nc.tensor.matmul(zp[:, :512],
                                         w2h[mi][:, ts(cc, 128)],
                                         ytiles[mi][:, ts(n2, 512)],
                                         start=(mi == 0), stop=(mi == 7))
                    with nc.allow_low_precision(reason="mlp psums f32r"):
                        if g == 0:
                            nc.vector.tensor_copy(zacc[c][:, ts(n2, 512)],
                                                  zp[:, :512])
                        else:
                            nc.vector.tensor_tensor(zacc[c][:, ts(n2, 512)],
                                                    zacc[c][:, ts(n2, 512)],
                                                    zp[:, :512], OP.add)

    # ---- residual + in-place transpose: zacc becomes row-major mod ----
    def mod_block(dst_i, dst_c, pf, rb):
        tmp = utile(pC4, [128, 128], F32, "rtmp", bufs=4)
        nc.vector.tensor_tensor(tmp[:], pf[:128, :128], rb, OP.add)
        with nc.allow_low_precision(reason="mod rows stored f32r"):
            nc.vector.tensor_tensor(zacc[dst_i][:, ts(dst_c, 128)], tmp[:],
                                    bm2b[:, ts(dst_c, 128)], OP.add)

    for i in range(8):
        resf = utile(pt, [128, C], F32, "xin")
        nc.sync.dma_start(resf[:], row_src_ap(i))
        for c in range(i, 8):
            p = ps_av()
            pf = p.rearrange("p a b -> p (a b)")
            nc.tensor.transpose(pf[:128, :128].bitcast(F32R),
                                zacc[c][:, ts(i, 128)], ident_r[:, :])
            if c != i:
                p2 = ps_mm()
                nc.tensor.transpose(p2[:128, :128].bitcast(F32R),
                                    zacc[i][:, ts(c, 128)], ident_r[:, :])
            mod_block(i, c, pf, resf[:, ts(c, 128)])
            if c != i:
                rb2 = utile(pC4, [128, 128], F32, "resb", bufs=2)
                nc.sync.dma_start(rb2[:], row_src_ap(c)[:, ts(i, 128)])
                mod_block(c, i, p2, rb2[:])
    mod_sb = zacc
    if dbg:
        for i in range(8):
            mff = utile(pC4, [128, C], F32, "mof", bufs=2)
            nc.vector.tensor_copy(mff[:], mod_sb[i][:])
            nc.sync.dma_start(dbg["dbg_mlpout"][ts(i, 128), :], mff[:])
    pC4.release()

    # ================= Stage D: pooling + Wp -> combined^T =================
    pD = tc.alloc_tile_pool(name="pD", bufs=1)
    # ApT[p, f] = 0.5 iff source row p pools into output f:
    #   even block: base = rank[p]-16, match iff (2f - base) in {-1, 0}
    #   dst  block: base = d,          match iff (2(f-248) - base) in {-1, 0}
    iota2e = utile(pD, [128, 504], F32, "iota2e")
    nc.vector.tensor_scalar_mul(iota2e[:], IOTA504B[:], 2.0)
    apT = [utile(pD, [128, 504], F32R, f"apT{m}") for m in range(8)]
    for m in range(8):
        base = utile(psm, [128, 1], F32, "ap_r")
        if m < 4:
            nc.vector.tensor_scalar_add(base[:], rank_t[:, m:m + 1], -float(R))
        else:
            nc.vector.tensor_scalar_add(base[:], piota[:],
                                        float(128 * (m - 4) + NE - R))
        d1 = utile(pD, [128, 504], F32, "ap_d1")
        nc.vector.tensor_scalar(d1[:], iota2e[:], base[:], None, OP.subtract)
        a1 = utile(pD, [128, 504], F32, "ap_a1")
        nc.vector.tensor_scalar(a1[:], d1[:], -1.5, None, OP.is_ge)
        b1 = utile(pD, [128, 504], F32, "ap_b1")
        nc.vector.tensor_scalar(b1[:], d1[:], 0.5, None, OP.is_le)
        nc.vector.scalar_tensor_tensor(apT[m][:], a1[:], 0.5, b1[:],
                                       OP.mult, OP.mult)
    # pooledT[c][128c, 504] = sum_i mod_sb[i][:, c-tile]^T @ apT[i]
    pooledT = [utile(pD, [128, NP], F32R, f"pooledT{k}") for k in range(8)]
    for c in range(8):
        zp = ps_mm() if c % 2 == 0 else         ps_av().rearrange("p a b -> p (a b)")
        for i in range(8):
            nc.tensor.matmul(zp[:, :NP], mod_sb[i][:, ts(c, 128)], apT[i][:],
                             start=(i == 0), stop=(i == 7))
        nc.vector.tensor_copy(pooledT[c][:], zp[:, :NP])
    if dbg:
        for m in range(4):
            pst = utile(pD, [128, C], F32, "pstg", bufs=2)
            for bj in range(8):
                p = ps_av()
                pf = p.rearrange("p a b -> p (a b)")
                nc.tensor.transpose(pf[:PP, :128].bitcast(F32R),
                                    pooledT[bj][:, m * PP:(m + 1) * PP],
                                    ident_r[:, :])
                nc.vector.tensor_copy(pst[:PP, ts(bj, 128)], pf[:PP, :128])
            nc.sync.dma_start(dbg["dbg_pooled"][ts(m, PP), :], pst[:PP, :])

    pE = tc.alloc_tile_pool(name="pE", bufs=1)
    bp3T = bcol("bp", pD, scale=3.0)
    cmbTr = [utile(pD, [128, NP], F32R, f"cmbTr{m}") for m in range(8)]
    for mg in range(2):
        wcs = []
        for k in range(KT):
            wcr = utile(pD, [128, 512], F32R, f"wpc{k}", bufs=1)
            nc.sync.dma_start(wcr[:], wd["Wp"][ts(k, 128), ts(mg, 512)].bitcast(F32R))
            wcs.append(wcr)
        for mi in range(4):
            m = mg * 4 + mi
            acc = ps_mm()
            for k in range(KT):
                nc.tensor.matmul(acc[:, :NP], wcs[k][:, ts(mi, 128)],
                                 pooledT[k][:], start=(k == 0), stop=(k == KT - 1))
            nc.scalar.activation(cmbTr[m][:], acc[:, :NP], AF.Identity,
                                 bias=bp3T[:, m:m + 1], scale=3.0)

    # ================= Stage E: MQA =================
    bqT = bcol("bq", pE)

    def make_mqT(m):
        acc = ps_mm()
        for k in range(KT):
            wr = utile(pw, [128, 128], F32R, "w1r", bufs=4)
            nc.sync.dma_start(wr[:], wd["Wq"][ts(k, 128), ts(m, 128)].bitcast(F32R))
            nc.tensor.matmul(acc[:, :NP], wr[:], cmbTr[k][:],
                             start=(k == 0), stop=(k == KT - 1))
        t = utile(pE, [128, NP], F32R, "mqT", bufs=2)
        nc.scalar.activation(t[:], acc[:, :NP], AF.Identity, bias=bqT[:, m:m + 1])
        return t

    wkvr = utile(pE, [128, KT, 2 * DH], F32R, "wkvr")
    for k in range(KT):
        nc.sync.dma_start(wkvr[:, k, :DH], wd["Wk"][ts(k, 128), :].bitcast(F32R))
        nc.sync.dma_start(wkvr[:, k, DH:], wd["Wv"][ts(k, 128), :].bitcast(F32R))
    bkT = utile(pE, [64, 1], F32, "bkT")
    nc.sync.dma_start(bkT[:], wd["bk"][:, None])
    mkT = utile(pE, [128, NP], F32R, "mkT")
    macc = ps_mm()
    for k in range(KT):
        nc.tensor.matmul(macc[:64, :NP], wkvr[:, k, :DH], cmbTr[k][:],
                         start=(k == 0), stop=(k == KT - 1))
    mkf = utile(pE, [64, NP], F32, "mkf")
    nc.scalar.activation(mkf[:], macc[:64, :NP], AF.Identity, bias=bkT[:])
    nc.vector.tensor_copy(mkT[:64, :], mkf[:])
    nc.sync.dma_start(mkT[64:, :], mkT[:64, :])

    bvb = bcast_row(load_row(wd["bv"], DH, "bv_r", pE), DH, "bv_b", pE)
    mv_pad = [utile(pE, [128, DH + 1], F32R, f"mvp{m}") for m in range(4)]
    for m in range(4):
        acc = ps_av()
        for k in range(KT):
            nc.tensor.matmul(acc[:PP, 0, :DH], cmbTr[k][:, m * PP:(m + 1) * PP],
                             wkvr[:, k, DH:], start=(k == 0), stop=(k == KT - 1))
        nc.vector.tensor_copy(mv_pad[m][:, DH:], ones_part[:, :1])
        nc.vector.tensor_tensor(mv_pad[m][:PP, :DH], acc[:PP, 0, :DH], bvb[:PP, :],
                                OP.add)

    mqT_cur = None
    for h in range(H):
        po = (h % 2) * 64
        if h % 2 == 0:
            mqT_cur = make_mqT(h // 2)
        mqT_h = mqT_cur[po:po + 64, :]
        spA = ps_bigA()
        spB = ps_bigB()
        ep = []
        for mm in range(4):
            spbig = spA if mm < 2 else spB
            sp = spbig[:, 512 * (mm % 2):512 * (mm % 2) + NP]
            nc.tensor.matmul(sp[:PP, :NP], mkT[po:po + 64, mm * PP:(mm + 1) * PP],
                             mqT_h[:], start=True, stop=True)
            et = utile(pE, [128, NP], F32R, "e2", bufs=8)
            nc.scalar.activation(et[:PP, :], sp[:PP, :NP], AF.Exp,
                                 scale=float(DH ** -0.5))
            ep.append(et)
        av2 = ps_av().rearrange("p a b -> p (a b)")
        for mm in range(4):
            nc.tensor.matmul(av2[:DH + 1, :NP], mv_pad[mm][:PP, :],
                             ep[mm][:PP, :], start=(mm == 0), stop=(mm == 3))
        rrow = utile(pE, [1, NP], F32R, "rrow2", bufs=4)
        with nc.allow_low_precision(reason="softmax denom; 11-bit ok downstream"):
            nc.vector.reciprocal(rrow[:], av2[DH:DH + 1, :NP])
        rb = ps_mm()
        nc.tensor.matmul(rb[:DH, :NP], ones_col_r[:, :DH], rrow[:],
                         start=True, stop=True)
        rbs = utile(pE, [64, NP], F32, "rbs2", bufs=4)
        nc.vector.tensor_copy(rbs[:], rb[:DH, :NP])
        stg = utile(pE, [64, NP], F32R, "mqstg", bufs=4)
        nc.vector.tensor_tensor(stg[:], av2[:DH, :NP], rbs[:], OP.mult)
        nc.sync.dma_start(mqaTd[h * 64:h * 64 + 64, :], stg[:])
    pE.release()
    pD.release()
    pZ.release()

    # ================= Stage F: Wmo + FFN =================
    pF = tc.alloc_tile_pool(name="pF", bufs=1)
    pF1 = tc.alloc_tile_pool(name="pF1", bufs=1)
    mqaT = [utile(pF1, [128, NP], F32R, f"mqaT{k}") for k in range(8)]
    for k in range(8):
        nc.sync.dma_start(mqaT[k][:, :NP], mqaTd[ts(k, 128), :])
    bmoT = bcol("bmo", pF1)
    omoT = [utile(pF, [128, NP], F32R, f"omoT{m}") for m in range(8)]
    for mg in range(2):
        wcs = []
        for k in range(KT):
            wcr = utile(pF1, [128, 512], F32R, f"wmc{k}", bufs=1)
            nc.sync.dma_start(wcr[:],
                              wd["Wmo"][ts(k, 128), ts(mg, 512)].bitcast(F32R))
            wcs.append(wcr)
        for mi in range(4):
            m = mg * 4 + mi
            acc = ps_mm()
            for k in range(KT):
                nc.tensor.matmul(acc[:, :NP], wcs[k][:, ts(mi, 128)],
                                 mqaT[k][:], start=(k == 0), stop=(k == KT - 1))
            nc.scalar.activation(omoT[m][:], acc[:, :NP], AF.Identity,
                                 bias=bmoT[:, m:m + 1])
    pF1.release()

    # ---- FFN c-major: Y halves of 16 hidden tiles, Z accum in SBUF ----
    bf1T = bcol("bf1", pF)
    bf2b = brow("bf2", pF)
    zf = [utile(pF, [128, NP], F32R, f"zf{c}") for c in range(8)]
    for hf in range(2):
        yfs = []
        wf1h = None
        for kki in range(16):
            kk = hf * 16 + kki
            if kki % 4 == 0:
                wf1h = []
                for k in range(KT):
                    w = utile(pF, [128, 512], F32R, "wf1h", bufs=9)
                    nc.sync.dma_start(
                        w[:], wd["Wf1"][ts(k, 128), ts(kk // 4, 512)]
                        .bitcast(F32R))
                    wf1h.append(w)
            yp = ps_mm() if kki % 2 == 0 else             ps_av().rearrange("p a b -> p (a b)")
            for k in range(KT):
                nc.tensor.matmul(yp[:, :NP], wf1h[k][:, ts(kki % 4, 128)],
                                 omoT[k][:], start=(k == 0), stop=(k == KT - 1))
            yf = utile(pF, [128, NP], F32R, "yf", bufs=17)
            nc.scalar.activation(yf[:], yp[:, :NP], AF.Silu,
                                 bias=bf1T[:, kk:kk + 1])
            yfs.append(yf)
        for ch in range(2):
            wf2h = []
            for kki in range(16):
                kk = hf * 16 + kki
                w = utile(pF, [128, 512], F32R, "wf2h", bufs=17)
                nc.sync.dma_start(w[:],
                                  wd["Wf2"][ts(kk, 128), ts(ch, 512)].bitcast(F32R))
                wf2h.append(w)
            for cc in range(4):
                c = ch * 4 + cc
                zp = ps_bigA() if cc % 2 == 0 else ps_bigB()
                for kki in range(16):
                    nc.tensor.matmul(zp[:, :NP], wf2h[kki][:, ts(cc, 128)],
                                     yfs[kki][:], start=(kki == 0),
                                     stop=(kki == 15))
                with nc.allow_low_precision(reason="ffn out partials f32r"):
                    if hf == 0:
                        nc.vector.tensor_copy(zf[c][:], zp[:, :NP])
                    else:
                        nc.vector.tensor_tensor(zf[c][:], zf[c][:], zp[:, :NP],
                                                OP.add)
    # ---- transpose back to row-major + bias -> out ----
    for tl in range(4):
        of = utile(pF, [128, C], F32, "of", bufs=2)
        for c in range(8):
            p = ps_av()
            pf = p.rearrange("p a b -> p (a b)")
            nc.tensor.transpose(pf[:PP, :128].bitcast(F32R),
                                zf[c][:, tl * PP:(tl + 1) * PP], ident_r[:, :])
            nc.vector.tensor_tensor(of[:PP, ts(c, 128)], pf[:PP, :128],
                                    bf2b[:PP, ts(c, 128)], OP.add)
        nc.sync.dma_start(out_d[tl * PP:(tl + 1) * PP, :], of[:PP, :])
    pF.release()
    for pool in (pt, pw, psm, pc, pp):
        pool.release()


_BUILT = None


def kernel(**inputs):
    global _BUILT
    if _BUILT is None:
        _BUILT = build(debug=DEBUG)
    nc = _BUILT
    x = np.ascontiguousarray(inputs["x"], dtype=np.float32)
    base = {k: np.ascontiguousarray(v, dtype=np.float32) for k, v in inputs.items()
            if k != "x"}
    in_maps = []
    for i in range(8):
        m = dict(base)
        m["x"] = x[i]
        in_maps.append(m)
    res = run_bass_kernel_spmd(nc, in_maps, core_ids=list(range(8)))
    out = np.stack([res.results[i]["out"] for i in range(8)], axis=0)
    return out.astype(np.float32)



# revision 52
# speedup vs baseline: 1.0126x; 1.0126x over previous
"""AdaptiveTokenMerger (ToMe block + merger) TRN2 Bass kernel.

Data-parallel over batch: 8 samples -> 8 NeuronCores, one sample per core.
Per-core pipeline (sample x [1024, 1024]):
  A (f32, ranking-critical): LN1 -> qkv -> MHA (transposed-softmax with the
    denominator folded in as an appended ones-column of v) -> Wo -> x_attn
  B: metric scores -> node_max/argmax -> ranks via pairwise comparisons ->
    dst scatter-add expressed as a one-hot matmul
  C (f32r): MLP over rows [x1_even(512); dst_new(512)], fused W1/W2 per
    token-quarter, output accumulated in PSUM across all 32 W1 column tiles
  D: pooling as a rank-dependent one-hot matmul -> Wp -> combined = 3q
  E (f32r): multi-query attention  F (f32r): FFN -> out [504, 1024]

Precision: everything upstream of the rank/argmax decisions is true fp32
(4 cyc/row on PE); post-merge matmuls use float32r (TF32-ish, 1 cyc/row).

PSUM budget (8 banks): BIGA/BIGB [128,1024] (2+2), MM [128,512] x2 (2),
AV [128,4,128] x2 (2).
"""
import numpy as np

import concourse.bass as bass
import concourse.tile as tile
from concourse import bacc, mybir
from concourse.bass import ts
from concourse.bass_utils import run_bass_kernel_spmd
from concourse.masks import make_identity

F32 = mybir.dt.float32
F32R = mybir.dt.float32r
BF16 = mybir.dt.bfloat16
U32 = mybir.dt.uint32

N, C, H = 1024, 1024, 16
R = 16
DH = C // H          # 64
NE = N // 2          # 512
NP = (N - R) // 2    # 504
PP = 126             # pooled tokens per partition tile
KT = C // 128        # 8
AF = mybir.ActivationFunctionType
OP = mybir.AluOpType

DEBUG = False


def build(debug=False):
    nc = bacc.Bacc("TRN2", target_bir_lowering=False, debug=False, num_devices=8)
    x_d = nc.dram_tensor("x", [N, C], F32, kind="ExternalInput").ap()
    wd = {}
    for name, shape in [
        ("g1", [C]), ("be1", [C]), ("Wqkv", [C, 3 * C]), ("bqkv", [3 * C]),
        ("Wo", [C, C]), ("bo", [C]), ("g2", [C]), ("be2", [C]),
        ("W1", [C, 4 * C]), ("bm1", [4 * C]), ("W2", [4 * C, C]), ("bm2", [C]),
        ("Wp", [C, C]), ("bp", [C]), ("Wq", [C, C]), ("bq", [C]),
        ("Wk", [C, DH]), ("bk", [DH]), ("Wv", [C, DH]), ("bv", [DH]),
        ("Wmo", [C, C]), ("bmo", [C]), ("Wf1", [C, 4 * C]), ("bf1", [4 * C]),
        ("Wf2", [4 * C, C]), ("bf2", [C]),
    ]:
        wd[name] = nc.dram_tensor(name, shape, F32, kind="ExternalInput").ap()
    out_d = nc.dram_tensor("out", [NP, C], F32, kind="ExternalOutput").ap()
    dbg = {}
    if debug:
        for name, shape in [
            ("dbg_xattn", [N, C]), ("dbg_nm", [NE]), ("dbg_rank", [NE]),
            ("dbg_nodeidx", [NE]), ("dbg_mlpin", [N, C]), ("dbg_mlpout", [N, C]),
            ("dbg_pooled", [NP, C]),
        ]:
            dbg[name] = nc.dram_tensor(name, shape, F32, kind="ExternalOutput").ap()
    with tile.TileContext(nc) as tc:
        _build_tile(nc, tc, x_d, wd, out_d, dbg)
    nc.compile()
    return nc


def _build_tile(nc, tc, x_d, wd, out_d, dbg):
    # DRAM spill buffers
    qkTd = nc.dram_tensor("qkTd", [2 * C, N], F32).ap()
    aoTd = nc.dram_tensor("aoTd", [C, N], F32).ap()
    x1d = nc.dram_tensor("x1d", [N, C], F32).ap()
    dstnd = nc.dram_tensor("dstnd", [NE, C], F32).ap()
    mqaTd = nc.dram_tensor("mqaTd", [C, NP], F32R).ap()

    pc = tc.alloc_tile_pool(name="const", bufs=1)
    psm = tc.alloc_tile_pool(name="small", bufs=1)
    pw = tc.alloc_tile_pool(name="wstream", bufs=2)
    pt = tc.alloc_tile_pool(name="tmp", bufs=2)
    pp = tc.alloc_tile_pool(name="psum", bufs=1, space="PSUM")

    _ct = {}

    def utile(pool, shape, dtype, tag, bufs=None):
        _ct[tag] = _ct.get(tag, 0) + 1
        kw = {"bufs": bufs} if bufs is not None else {}
        return pool.tile(shape, dtype, tag=tag, name=f"{tag}_{_ct[tag]}", **kw)

    def ps_bigA():
        return utile(pp, [128, 1024], F32, "BIGA")

    def ps_bigB():
        return utile(pp, [128, 1024], F32, "BIGB")

    def ps_mm():
        return utile(pp, [128, 512], F32, "MM", bufs=2)

    def ps_av():
        return utile(pp, [128, 4, 128], F32, "AV", bufs=2)

    # ---------- constants ----------
    ident = pc.tile([128, 128], F32)
    make_identity(nc, ident[:])
    ident_r = pc.tile([128, 128], F32R)
    nc.vector.tensor_copy(ident_r[:], ident[:])
    ones_col = pc.tile([1, 128], F32)
    nc.gpsimd.memset(ones_col[:], 1.0)
    ones_col_r = pc.tile([1, 128], F32R)
    nc.vector.tensor_copy(ones_col_r[:], ones_col[:])
    ones_part = pc.tile([128, 8], F32)
    nc.gpsimd.memset(ones_part[:], 1.0)
    piota = pc.tile([128, 1], F32)
    nc.gpsimd.iota(piota[:], [[0, 1]], channel_multiplier=1,
                   allow_small_or_imprecise_dtypes=True)
    iota512_row = pc.tile([1, 512], F32)
    nc.gpsimd.iota(iota512_row[:], [[1, 512]], channel_multiplier=0,
                   allow_small_or_imprecise_dtypes=True)
    iota504_row = pc.tile([1, 504], F32)
    nc.gpsimd.iota(iota504_row[:], [[1, 504]], channel_multiplier=0,
                   allow_small_or_imprecise_dtypes=True)

    def bcast_row(row_ap, n, tag, pool, scale=1.0):
        t = utile(pool, [128, n], F32, tag)
        for c0 in range(0, n, 512):
            cw = min(512, n - c0)
            p = ps_mm()
            nc.tensor.matmul(p[:, :cw], ones_col[:], row_ap[:, c0:c0 + cw],
                             start=True, stop=True)
            if scale == 1.0:
                nc.vector.tensor_copy(t[:, c0:c0 + cw], p[:, :cw])
            else:
                nc.vector.tensor_scalar_mul(t[:, c0:c0 + cw], p[:, :cw], scale)
        return t

    def load_row(dram_ap, n, tag, pool):
        t = utile(pw, [1, n], F32, "rowstg", bufs=1)
        nc.sync.dma_start(t[:], dram_ap[None, :])
        return t

    def brow(name, pool, scale=1.0):
        n = wd[name].shape[0]
        return bcast_row(load_row(wd[name], n, name + "_r", pool), n,
                         name + "_b", pool, scale)

    def bcol(name, pool, scale=1.0):
        n = wd[name].shape[0]
        t = utile(pool, [128, n // 128], F32, name + "_c")
        nc.sync.dma_start(t[:], wd[name].rearrange("(t p) -> p t", p=128))
        if scale != 1.0:
            nc.vector.tensor_scalar_mul(t[:], t[:], scale)
        return t

    IOTA512B = bcast_row(iota512_row[:], 512, "iota512b", pc)
    IOTA504B = bcast_row(iota504_row[:], 504, "iota504b", pc)

    def transpose_blocks(src_tiles, dst, n_rows, n_cols):
        """dst[c, r] = src[r, c]; dst is tile-list or sink(bj, bi, pf, cw, rw)."""
        for bi in range((n_rows + 127) // 128):
            rw = min(128, n_rows - bi * 128)
            for bj in range((n_cols + 127) // 128):
                cw = min(128, n_cols - bj * 128)
                p = ps_av()
                pf = p.rearrange("p a b -> p (a b)")
                nc.tensor.transpose(pf[:cw, :rw],
                                    src_tiles[bi][:rw, bj * 128:bj * 128 + cw],
                                    ident[:rw, :rw])
                if callable(dst):
                    dst(bj, bi, pf, cw, rw)
                else:
                    nc.vector.tensor_copy(dst[bj][:cw, bi * 128:bi * 128 + rw],
                                          pf[:cw, :rw])

    def refined_rsqrt_recip(vv, tag):
        """returns 1/sqrt(vv) with one Newton step on sqrt (ACT sqrt is loose)."""
        s0 = utile(psm, [128, 1], F32, tag + "_s0")
        nc.scalar.sqrt(s0[:], vv[:])
        r0 = utile(psm, [128, 1], F32, tag + "_r0")
        nc.vector.reciprocal(r0[:], s0[:])
        t = utile(psm, [128, 1], F32, tag + "_t")
        nc.vector.tensor_tensor(t[:], vv[:], r0[:], OP.mult)
        nc.vector.tensor_tensor(t[:], t[:], s0[:], OP.add)
        nc.vector.tensor_scalar_mul(t[:], t[:], 0.5)
        rr = utile(psm, [128, 1], F32, tag + "_rr")
        nc.vector.reciprocal(rr[:], t[:])
        return rr

    def layer_norm(src, dst, gb, bb, lnpool):
        m = utile(psm, [128, 1], F32, "ln_m")
        nc.vector.reduce_sum(m[:], src[:, :C], axis=mybir.AxisListType.X)
        nc.vector.tensor_scalar_mul(m[:], m[:], 1.0 / C)
        xc = utile(lnpool, [128, C], F32, "ln_xc", bufs=1)
        nc.vector.tensor_scalar(xc[:], src[:, :C], m[:], None, OP.subtract)
        ss = utile(psm, [128, 1], F32, "ln_ss")
        nc.scalar.activation(dst[:, :C], xc[:], AF.Square, accum_out=ss[:])
        v = utile(psm, [128, 1], F32, "ln_v")
        nc.vector.tensor_scalar(v[:], ss[:], 1.0 / C, 1e-5, OP.mult, OP.add)
        rstd = refined_rsqrt_recip(v, "ln")
        nc.vector.tensor_scalar(dst[:, :C], xc[:], rstd[:], None, OP.mult)
        nc.vector.tensor_tensor(dst[:, :C], dst[:, :C], gb[:], OP.mult)
        nc.vector.tensor_tensor(dst[:, :C], dst[:, :C], bb[:], OP.add)

    # ================= Stage A: LN1 -> hT =================
    pbA = tc.alloc_tile_pool(name="biasA", bufs=1)
    pHT = tc.alloc_tile_pool(name="pHT", bufs=1)
    pVP = tc.alloc_tile_pool(name="pVP", bufs=1)
    pAttn = tc.alloc_tile_pool(name="pAttn", bufs=1)

    g1b = brow("g1", pbA)
    be1b = brow("be1", pbA)
    hT = [utile(pHT, [128, N], F32, f"hT{k}") for k in range(8)]
    ht = []
    for i in range(8):
        xt = utile(pt, [128, C], F32, "xin")
        nc.sync.dma_start(xt[:], x_d[ts(i, 128), :])
        h = utile(pHT, [128, C], F32, "ht", bufs=4)
        layer_norm(xt, h, g1b, be1b, pbA)
        ht.append(h)
    transpose_blocks(ht, hT, N, C)

    # ===== qk^T -> qkTd (DRAM) ; v_pad (SBUF) =====
    bqkT = bcol("bqkv", pbA)
    for mp in range(8):
        accq = ps_bigA()
        acck = ps_bigB()
        for k in range(KT):
            wq = utile(pw, [128, 128], F32, "wqkb", bufs=4)
            nc.sync.dma_start(wq[:], wd["Wqkv"][ts(k, 128), ts(mp, 128)])
            wk = utile(pw, [128, 128], F32, "wqkb", bufs=4)
            nc.sync.dma_start(wk[:],
                              wd["Wqkv"][ts(k, 128), C + mp * 128:C + (mp + 1) * 128])
            for n2 in range(2):
                nc.tensor.matmul(accq[:, ts(n2, 512)], wq[:], hT[k][:, ts(n2, 512)],
                                 start=(k == 0), stop=(k == KT - 1))
                nc.tensor.matmul(acck[:, ts(n2, 512)], wk[:], hT[k][:, ts(n2, 512)],
                                 start=(k == 0), stop=(k == KT - 1))
        stgq = utile(pAttn, [128, N], F32, "qkstg", bufs=2)
        nc.scalar.activation(stgq[:], accq[:], AF.Identity, bias=bqkT[:, mp:mp + 1])
        nc.sync.dma_start(qkTd[ts(mp, 128), :], stgq[:])
        stgk = utile(pAttn, [128, N], F32, "qkstg", bufs=2)
        nc.scalar.activation(stgk[:], acck[:], AF.Identity,
                             bias=bqkT[:, 8 + mp:9 + mp])
        nc.sync.dma_start(qkTd[C + mp * 128:C + (mp + 1) * 128, :], stgk[:])

    bvqkvb = bcast_row(load_row(wd["bqkv"][2 * C:], C, "bvq_r", pbA), C,
                       "bvq_b", pbA)
    v_pad = [utile(pVP, [128, H, DH + 1], F32, f"vp{j}") for j in range(8)]
    for j in range(8):
        nc.vector.memset(v_pad[j][:, :, DH:DH + 1], 1.0)
    for jg in range(4):
        accs = [ps_bigA(), ps_bigB()]
        for k in range(KT):
            wv = utile(pVP, [128, C], F32, "wv", bufs=3)
            nc.sync.dma_start(wv[:], wd["Wqkv"][ts(k, 128), 2 * C:])
            for jj in range(2):
                j = jg * 2 + jj
                for n2 in range(2):
                    nc.tensor.matmul(accs[jj][:, ts(n2, 512)],
                                     hT[k][:, ts(j, 128)], wv[:, ts(n2, 512)],
                                     start=(k == 0), stop=(k == KT - 1))
        for jj in range(2):
            j = jg * 2 + jj
            for h in range(H):
                nc.vector.tensor_tensor(v_pad[j][:, h, :DH],
                                        accs[jj][:, ts(h, DH)],
                                        bvqkvb[:, ts(h, DH)], OP.add)

    # ===== attention: stream kT/qT per head; out -> aoTd (already c-major) ==
    # out[dh|sum, i] = v_pad[j].T @ expT[j, i], accumulated over j-tiles.
    for h in range(H):
        kth = utile(pAttn, [64, N], F32, "kth", bufs=2)
        nc.sync.dma_start(kth[:], qkTd[C + h * 64:C + h * 64 + 64, :])
        qth = utile(pAttn, [64, N], F32, "qth", bufs=2)
        nc.sync.dma_start(qth[:], qkTd[h * 64:h * 64 + 64, :])
        av = [ps_av().rearrange("p a b -> p (a b)") for _ in range(2)]
        for j in range(8):
            for n2 in range(2):
                sp = ps_mm()
                nc.tensor.matmul(sp[:], kth[:, ts(j, 128)], qth[:, ts(n2, 512)],
                                 start=True, stop=True)
                et = utile(pAttn, [128, 512], F32, "exp", bufs=3)
                nc.scalar.activation(et[:], sp[:], AF.Exp, scale=float(DH ** -0.5))
                nc.tensor.matmul(av[n2][:DH + 1, :512], v_pad[j][:, h, :], et[:],
                                 start=(j == 0), stop=(j == 7))
        for n2 in range(2):
            rrow = utile(pAttn, [1, 512], F32, "rrow", bufs=2)
            nc.vector.reciprocal(rrow[:], av[n2][DH:DH + 1, :512])
            rb = ps_mm()
            nc.tensor.matmul(rb[:DH, :512], ones_col[:, :DH], rrow[:],
                             start=True, stop=True)
            rbs = utile(pAttn, [64, 512], F32, "rbs", bufs=2)
            nc.vector.tensor_copy(rbs[:], rb[:DH, :512])
            stg = utile(pAttn, [64, 512], F32, "aot_stg", bufs=2)
            nc.vector.tensor_tensor(stg[:], av[n2][:DH, :512], rbs[:],
                                    OP.mult)
            nc.sync.dma_start(aoTd[h * 64:h * 64 + 64, ts(n2, 512)], stg[:])
    pAttn.release()
    pVP.release()
    pHT.release()
    pbA.release()

    # ================= Wo -> x_attn, x1 (-> DRAM), metric =================
    pB2 = tc.alloc_tile_pool(name="pB2", bufs=1)
    bob = brow("bo", pB2)
    xa = [utile(pB2, [128, C], F32, f"xa{m}") for m in range(8)]
    woR = [utile(pB2, [128, C], F32, f"woR{k}") for k in range(8)]
    for k in range(KT):
        nc.sync.dma_start(woR[k][:], wd["Wo"][ts(k, 128), :])
    rn = psm.tile([128, 8], F32)
    for m in range(8):
        acc = ps_bigA()
        for k in range(KT):
            ao = utile(pw, [128, 128], F32, "wqkb", bufs=4)
            nc.sync.dma_start(ao[:], aoTd[ts(k, 128), ts(m, 128)])
            for n2 in range(2):
                nc.tensor.matmul(acc[:, ts(n2, 512)], ao[:],
                                 woR[k][:, ts(n2, 512)],
                                 start=(k == 0), stop=(k == KT - 1))
        nc.vector.tensor_tensor(xa[m][:], acc[:], bob[:], OP.add)
        xt = utile(pt, [128, C], F32, "xin")
        nc.sync.dma_start(xt[:], x_d[ts(m, 128), :])
        x1stg = utile(pw, [128, C], F32, "x1stg", bufs=2)
        nc.vector.tensor_tensor(x1stg[:], xa[m][:], xt[:], OP.add)
        nc.sync.dma_start(x1d[ts(m, 128), :], x1stg[:])
        ss = utile(psm, [128, 1], F32, "nrm_ss")
        sq = utile(pt, [128, C], F32, "ln_xc")
        nc.scalar.activation(sq[:], xa[m][:], AF.Square, accum_out=ss[:])
        rr = refined_rsqrt_recip(ss, "nrm")
        nc.vector.tensor_copy(rn[:, m:m + 1], rr[:])
        nc.vector.tensor_scalar(xa[m][:], xa[m][:], rn[:, m:m + 1], None, OP.mult)
        if dbg:
            nc.sync.dma_start(dbg["dbg_xattn"][ts(m, 128), :], xa[m][:])

    # ===== de-interleave metric -> maT/mbT; scores; node stats; ranks =====
    pB3 = tc.alloc_tile_pool(name="pB3", bufs=1)
    xae = [utile(pB3, [128, C], F32, f"xae{m}") for m in range(4)]
    xao = [utile(pB3, [128, C], F32, f"xao{m}") for m in range(4)]
    for m in range(4):
        nc.sync.dma_start(xae[m][:64, :], xa[2 * m][0:128:2, :])
        nc.sync.dma_start(xae[m][64:, :], xa[2 * m + 1][0:128:2, :])
        nc.sync.dma_start(xao[m][:64, :], xa[2 * m][1:128:2, :])
        nc.sync.dma_start(xao[m][64:, :], xa[2 * m + 1][1:128:2, :])
    pB4 = tc.alloc_tile_pool(name="pB4", bufs=1)
    maT = [utile(pB4, [128, NE], F32, f"maT{k}") for k in range(8)]
    mbT = [utile(pB4, [128, NE], F32, f"mbT{k}") for k in range(8)]
    transpose_blocks(xae, maT, NE, C)
    transpose_blocks(xao, mbT, NE, C)

    nm_t = psm.tile([128, 4], F32)
    ni_t = psm.tile([128, 4], F32)
    for m in range(4):
        acc = ps_bigA()
        for k in range(KT):
            nc.tensor.matmul(acc[:, :512], maT[k][:, ts(m, 128)], mbT[k][:],
                             start=(k == 0), stop=(k == KT - 1))
        mx8 = utile(psm, [128, 8], F32, "mx8")
        ix8 = utile(psm, [128, 8], U32, "ix8")
        nc.vector.max_with_indices(mx8[:], ix8[:], acc[:, :512])
        nc.vector.tensor_copy(nm_t[:, m:m + 1], mx8[:, 0:1])
        nc.vector.tensor_copy(ni_t[:, m:m + 1], ix8[:, 0:1])

    nm_row = utile(pB4, [1, 512], F32, "nm_row")
    for m in range(4):
        p = ps_av()
        pf = p.rearrange("p a b -> p (a b)")
        nc.tensor.transpose(pf[:1, :128], nm_t[:, m:m + 1], ident[:])
        nc.vector.tensor_copy(nm_row[:, ts(m, 128)], pf[:1, :128])
    NMB = bcast_row(nm_row[:], 512, "nmb", pB4)

    rank_t = psm.tile([128, 4], F32)
    for m in range(4):
        gt = utile(pB4, [128, 512], F32, "rk_gt", bufs=1)
        nc.vector.tensor_scalar(gt[:], NMB[:], nm_t[:, m:m + 1], None, OP.is_gt)
        eq = utile(pB4, [128, 512], F32, "rk_eq", bufs=1)
        nc.vector.tensor_scalar(eq[:], NMB[:], nm_t[:, m:m + 1], None, OP.is_equal)
        flt = utile(pB4, [128, 512], F32, "rk_flt", bufs=1)
        pio = utile(psm, [128, 1], F32, "rk_pio")
        nc.vector.tensor_scalar_add(pio[:], piota[:], float(128 * m))
        nc.vector.tensor_scalar(flt[:], IOTA512B[:], pio[:], None, OP.is_lt)
        nc.vector.tensor_tensor(eq[:], eq[:], flt[:], OP.mult)
        nc.vector.tensor_tensor(gt[:], gt[:], eq[:], OP.add)
        nc.vector.reduce_sum(rank_t[:, m:m + 1], gt[:], axis=mybir.AxisListType.X)
    if dbg:
        for (tt, nme) in [(nm_t, "dbg_nm"), (rank_t, "dbg_rank"),
                          (ni_t, "dbg_nodeidx")]:
            nc.sync.dma_start(dbg[nme].rearrange("(m p) -> p m", p=128), tt[:])
    pB4.release()
    pB3.release()
    pB2.release()

    # ================= dst merge (x1 from DRAM; dstn -> DRAM) =============
    pM = tc.alloc_tile_pool(name="pM", bufs=1)
    x1e = [utile(pM, [128, C + 8], F32R, f"x1e{m}") for m in range(4)]
    x1o = [utile(pM, [128, C], F32, f"x1o{m}") for m in range(4)]
    for m in range(4):
        nc.vector.tensor_copy(x1e[m][:, C:C + 8], ones_part[:])
        nc.sync.dma_start(x1e[m][:, :C],
                          x1d[256 * m:256 * m + 256:2, :].bitcast(F32R))
        nc.sync.dma_start(x1o[m][:], x1d[256 * m + 1:256 * m + 256:2, :])
    st = [utile(pM, [128, 512], F32R, f"st{m}") for m in range(4)]
    for m in range(4):
        msk = utile(psm, [128, 1], F32, "st_m")
        nc.vector.tensor_scalar(msk[:], rank_t[:, m:m + 1], float(R) - 0.5, None,
                                OP.is_lt)
        nc.vector.tensor_scalar(st[m][:], IOTA512B[:], ni_t[:, m:m + 1], None,
                                OP.is_equal)
        nc.vector.tensor_scalar(st[m][:], st[m][:], msk[:], None, OP.mult)
    for m in range(4):
        acc = ps_bigA()
        cacc = ps_av()
        for k in range(4):
            for n2 in range(2):
                nc.tensor.matmul(acc[:, ts(n2, 512)], st[k][:, ts(m, 128)],
                                 x1e[k][:, n2 * 512:n2 * 512 + 512],
                                 start=(k == 0), stop=(k == 3))
            nc.tensor.matmul(cacc[:, 0, :8], st[k][:, ts(m, 128)],
                             x1e[k][:, C:C + 8], start=(k == 0), stop=(k == 3))
        cnt = utile(psm, [128, 1], F32, "cnt")
        nc.vector.tensor_scalar_add(cnt[:], cacc[:, 0, 0:1], 1.0)
        rec = utile(psm, [128, 1], F32, "cntr")
        nc.vector.reciprocal(rec[:], cnt[:])
        dst_stg = utile(pM, [128, C], F32, "dst_stg", bufs=2)
        nc.vector.tensor_tensor(dst_stg[:], acc[:], x1o[m][:], OP.add)
        nc.vector.tensor_scalar(dst_stg[:], dst_stg[:], rec[:], None, OP.mult)
        nc.sync.dma_start(dstnd[ts(m, 128), :], dst_stg[:])

    # ========== MLP (f32r): W1/W2 streamed once; SBUF out accumulation ======
    def row_src_ap(i):
        if i < 4:
            return x1d[256 * i:256 * i + 256:2, :]
        return dstnd[ts(i - 4, 128), :]

    pM.release()
    pZ = tc.alloc_tile_pool(name="pZ", bufs=1)
    pC4 = tc.alloc_tile_pool(name="pC4", bufs=1)
    g2b = brow("g2", pC4)
    be2b = brow("be2", pC4)
    h2 = []
    for i in range(8):
        rsrc = utile(pt, [128, C], F32, "xin")
        nc.sync.dma_start(rsrc[:], row_src_ap(i))
        h = utile(pC4, [128, C], F32, "ht", bufs=2)
        layer_norm(rsrc, h, g2b, be2b, pC4)
        h2.append(h)
        if dbg:
            nc.sync.dma_start(dbg["dbg_mlpin"][ts(i, 128), :], rsrc[:])
    h2T = [utile(pC4, [128, N], F32R, f"h2T{k}") for k in range(8)]
    transpose_blocks(h2, h2T, N, C)

    bm1T = bcol("bm1", pC4)
    bm2T = bcol("bm2", psm)
    # ---- MLP c-major: Y^T groups of 8 hidden tiles, Z^T accum in SBUF ----
    zacc = [utile(pZ, [128, N], F32R, f"zacc{c}") for c in range(8)]
    for g in range(4):
        ytiles = []
        w1h = None
        for mi in range(8):
            mt = g * 8 + mi
            if mi % 4 == 0:
                w1h = []
                for k in range(KT):
                    w = utile(pC4, [128, 512], F32R, "w1h", bufs=8)
                    nc.sync.dma_start(
                        w[:], wd["W1"][ts(k, 128), ts(2 * g + mi // 4, 512)]
                        .bitcast(F32R))
                    w1h.append(w)
            yp = ps_bigA() if mi % 2 == 0 else ps_bigB()
            for k in range(KT):
                for n2 in range(2):
                    nc.tensor.matmul(yp[:, ts(n2, 512)],
                                     w1h[k][:, ts(mi % 4, 128)],
                                     h2T[k][:, ts(n2, 512)],
                                     start=(k == 0), stop=(k == KT - 1))
            yt = utile(pC4, [128, N], F32R, "yt", bufs=8)
            nc.scalar.activation(yt[:], yp[:], AF.Gelu_apprx_tanh,
                                 bias=bm1T[:, mt:mt + 1])
            ytiles.append(yt)
        for ch in range(2):
            w2h = []
            for mi in range(8):
                mt = g * 8 + mi
                w = utile(pC4, [128, 512], F32R, "w2h", bufs=8)
                nc.sync.dma_start(w[:],
                                  wd["W2"][ts(mt, 128), ts(ch, 512)].bitcast(F32R))
                w2h.append(w)
            for cc in range(4):
                c = ch * 4 + cc
                for n2 in range(2):
                    zp = ps_mm() if n2 == 0 else                         ps_av().rearrange("p a b -> p (a b)")
                    for mi in range(8):
                        nc.tensor.matmul(zp[:, :512],
                                         w2h[mi][:, ts(cc, 128)],
                                         ytiles[mi][:, ts(n2, 512)],
                                         start=(mi == 0), stop=(mi == 7))
                    with nc.allow_low_precision(reason="mlp psums f32r"):
                        if g == 0:
                            nc.vector.tensor_scalar(zacc[c][:, ts(n2, 512)],
                                                    zp[:, :512],
                                                    bm2T[:, c:c + 1], None,
                                                    OP.add)
                        else:
                            nc.vector.tensor_tensor(zacc[c][:, ts(n2, 512)],
                                                    zacc[c][:, ts(n2, 512)],
                                                    zp[:, :512], OP.add)

    # ---- residual + transpose -> row-major mod (bf16, pZ-resident) ----
    mod_sb = [utile(pZ, [128, N], BF16, f"modr{i}") for i in range(8)]
    for i in range(8):
        resf = utile(pt, [128, C], F32, "xin")
        nc.sync.dma_start(resf[:], row_src_ap(i))
        zt = ps_bigA() if i % 2 == 0 else ps_bigB()
        for c in range(8):
            nc.tensor.transpose(zt[:, ts(c, 128)].bitcast(F32R),
                                zacc[c][:, ts(i, 128)], ident_r[:, :])
        with nc.allow_low_precision(reason="mod rows stored bf16"):
            nc.vector.tensor_tensor(mod_sb[i][:], zt[:], resf[:], OP.add)
    if dbg:
        for i in range(8):
            mff = utile(pC4, [128, C], F32, "mof", bufs=2)
            nc.vector.tensor_copy(mff[:], mod_sb[i][:])
            nc.sync.dma_start(dbg["dbg_mlpout"][ts(i, 128), :], mff[:])
    pC4.release()

    # ================= Stage D: pooling + Wp -> combined^T =================
    pD = tc.alloc_tile_pool(name="pD", bufs=1)
    # ApT[p, f] = 0.5 iff source row p pools into output f:
    #   even block: base = rank[p]-16, match iff (2f - base) in {-1, 0}
    #   dst  block: base = d,          match iff (2(f-248) - base) in {-1, 0}
    iota2e = utile(pD, [128, 504], F32, "iota2e")
    nc.vector.tensor_scalar_mul(iota2e[:], IOTA504B[:], 2.0)
    apT = [utile(pD, [128, 504], BF16, f"apT{m}") for m in range(8)]
    for m in range(8):
        base = utile(psm, [128, 1], F32, "ap_r")
        if m < 4:
            nc.vector.tensor_scalar_add(base[:], rank_t[:, m:m + 1], -float(R))
        else:
            nc.vector.tensor_scalar_add(base[:], piota[:],
                                        float(128 * (m - 4) + NE - R))
        d1 = utile(pD, [128, 504], F32, "ap_d1")
        nc.vector.tensor_scalar(d1[:], iota2e[:], base[:], None, OP.subtract)
        a1 = utile(pD, [128, 504], F32, "ap_a1")
        nc.vector.tensor_scalar(a1[:], d1[:], -1.5, None, OP.is_ge)
        b1 = utile(pD, [128, 504], F32, "ap_b1")
        nc.vector.tensor_scalar(b1[:], d1[:], 0.5, None, OP.is_le)
        nc.vector.scalar_tensor_tensor(apT[m][:], a1[:], 0.5, b1[:],
                                       OP.mult, OP.mult)
    # pooledT[c][128c, 504] = sum_i mod_sb[i][:, c-tile]^T @ apT[i]
    pooledT = [utile(pD, [128, NP], F32R, f"pooledT{k}") for k in range(8)]
    for c in range(8):
        zp = ps_mm() if c % 2 == 0 else         ps_av().rearrange("p a b -> p (a b)")
        for i in range(8):
            nc.tensor.matmul(zp[:, :NP], mod_sb[i][:, ts(c, 128)], apT[i][:],
                             start=(i == 0), stop=(i == 7))
        nc.vector.tensor_copy(pooledT[c][:], zp[:, :NP])
    if dbg:
        for m in range(4):
            pst = utile(pD, [128, C], F32, "pstg", bufs=2)
            for bj in range(8):
                p = ps_av()
                pf = p.rearrange("p a b -> p (a b)")
                nc.tensor.transpose(pf[:PP, :128].bitcast(F32R),
                                    pooledT[bj][:, m * PP:(m + 1) * PP],
                                    ident_r[:, :])
                nc.vector.tensor_copy(pst[:PP, ts(bj, 128)], pf[:PP, :128])
            nc.sync.dma_start(dbg["dbg_pooled"][ts(m, PP), :], pst[:PP, :])

    pE = tc.alloc_tile_pool(name="pE", bufs=1)
    bp3T = bcol("bp", pD, scale=3.0)
    cmbTr = [utile(pD, [128, NP], F32R, f"cmbTr{m}") for m in range(8)]
    for mg in range(2):
        wcs = []
        for k in range(KT):
            wcr = utile(pD, [128, 512], F32R, f"wpc{k}", bufs=1)
            nc.sync.dma_start(wcr[:], wd["Wp"][ts(k, 128), ts(mg, 512)].bitcast(F32R))
            wcs.append(wcr)
        for mi in range(4):
            m = mg * 4 + mi
            acc = ps_mm()
            for k in range(KT):
                nc.tensor.matmul(acc[:, :NP], wcs[k][:, ts(mi, 128)],
                                 pooledT[k][:], start=(k == 0), stop=(k == KT - 1))
            nc.scalar.activation(cmbTr[m][:], acc[:, :NP], AF.Identity,
                                 bias=bp3T[:, m:m + 1], scale=3.0)

    # ================= Stage E: MQA =================
    bqT = bcol("bq", pE)

    def make_mqT(m):
        acc = ps_mm()
        for k in range(KT):
            wr = utile(pw, [128, 128], F32R, "w1r", bufs=4)
            nc.sync.dma_start(wr[:], wd["Wq"][ts(k, 128), ts(m, 128)].bitcast(F32R))
            nc.tensor.matmul(acc[:, :NP], wr[:], cmbTr[k][:],
                             start=(k == 0), stop=(k == KT - 1))
        t = utile(pE, [128, NP], F32R, "mqT", bufs=2)
        nc.scalar.activation(t[:], acc[:, :NP], AF.Identity, bias=bqT[:, m:m + 1])
        return t

    wkvr = utile(pE, [128, KT, 2 * DH], F32R, "wkvr")
    for k in range(KT):
        nc.sync.dma_start(wkvr[:, k, :DH], wd["Wk"][ts(k, 128), :].bitcast(F32R))
        nc.sync.dma_start(wkvr[:, k, DH:], wd["Wv"][ts(k, 128), :].bitcast(F32R))
    bkT = utile(pE, [64, 1], F32, "bkT")
    nc.sync.dma_start(bkT[:], wd["bk"][:, None])
    mkT = utile(pE, [128, NP], F32R, "mkT")
    macc = ps_mm()
    for k in range(KT):
        nc.tensor.matmul(macc[:64, :NP], wkvr[:, k, :DH], cmbTr[k][:],
                         start=(k == 0), stop=(k == KT - 1))
    mkf = utile(pE, [64, NP], F32, "mkf")
    nc.scalar.activation(mkf[:], macc[:64, :NP], AF.Identity, bias=bkT[:])
    nc.vector.tensor_copy(mkT[:64, :], mkf[:])
    nc.sync.dma_start(mkT[64:, :], mkT[:64, :])

    bvb = bcast_row(load_row(wd["bv"], DH, "bv_r", pE), DH, "bv_b", pE)
    mv_pad = [utile(pE, [128, DH + 1], F32R, f"mvp{m}") for m in range(4)]
    for m in range(4):
        acc = ps_av()
        for k in range(KT):
            nc.tensor.matmul(acc[:PP, 0, :DH], cmbTr[k][:, m * PP:(m + 1) * PP],
                             wkvr[:, k, DH:], start=(k == 0), stop=(k == KT - 1))
        nc.vector.tensor_copy(mv_pad[m][:, DH:], ones_part[:, :1])
        nc.vector.tensor_tensor(mv_pad[m][:PP, :DH], acc[:PP, 0, :DH], bvb[:PP, :],
                                OP.add)

    mqT_cur = None
    for h in range(H):
        po = (h % 2) * 64
        if h % 2 == 0:
            mqT_cur = make_mqT(h // 2)
        mqT_h = mqT_cur[po:po + 64, :]
        spA = ps_bigA()
        spB = ps_bigB()
        ep = []
        for mm in range(4):
            spbig = spA if mm < 2 else spB
            sp = spbig[:, 512 * (mm % 2):512 * (mm % 2) + NP]
            nc.tensor.matmul(sp[:PP, :NP], mkT[po:po + 64, mm * PP:(mm + 1) * PP],
                             mqT_h[:], start=True, stop=True)
            et = utile(pE, [128, NP], F32R, "e2", bufs=8)
            nc.scalar.activation(et[:PP, :], sp[:PP, :NP], AF.Exp,
                                 scale=float(DH ** -0.5))
            ep.append(et)
        av2 = ps_av().rearrange("p a b -> p (a b)")
        for mm in range(4):
            nc.tensor.matmul(av2[:DH + 1, :NP], mv_pad[mm][:PP, :],
                             ep[mm][:PP, :], start=(mm == 0), stop=(mm == 3))
        rrow = utile(pE, [1, NP], F32R, "rrow2", bufs=4)
        with nc.allow_low_precision(reason="softmax denom; 11-bit ok downstream"):
            nc.vector.reciprocal(rrow[:], av2[DH:DH + 1, :NP])
        rb = ps_mm()
        nc.tensor.matmul(rb[:DH, :NP], ones_col_r[:, :DH], rrow[:],
                         start=True, stop=True)
        rbs = utile(pE, [64, NP], F32, "rbs2", bufs=4)
        nc.vector.tensor_copy(rbs[:], rb[:DH, :NP])
        stg = utile(pE, [64, NP], F32R, "mqstg", bufs=4)
        nc.vector.tensor_tensor(stg[:], av2[:DH, :NP], rbs[:], OP.mult)
        nc.sync.dma_start(mqaTd[h * 64:h * 64 + 64, :], stg[:])
    pE.release()
    pD.release()
    pZ.release()

    # ================= Stage F: Wmo + FFN =================
    pF = tc.alloc_tile_pool(name="pF", bufs=1)
    pF1 = tc.alloc_tile_pool(name="pF1", bufs=1)
    mqaT = [utile(pF1, [128, NP], F32R, f"mqaT{k}") for k in range(8)]
    for k in range(8):
        nc.sync.dma_start(mqaT[k][:, :NP], mqaTd[ts(k, 128), :])
    bmoT = bcol("bmo", pF1)
    omoT = [utile(pF, [128, NP], F32R, f"omoT{m}") for m in range(8)]
    for mg in range(2):
        wcs = []
        for k in range(KT):
            wcr = utile(pF1, [128, 512], F32R, f"wmc{k}", bufs=1)
            nc.sync.dma_start(wcr[:],
                              wd["Wmo"][ts(k, 128), ts(mg, 512)].bitcast(F32R))
            wcs.append(wcr)
        for mi in range(4):
            m = mg * 4 + mi
            acc = ps_mm()
            for k in range(KT):
                nc.tensor.matmul(acc[:, :NP], wcs[k][:, ts(mi, 128)],
                                 mqaT[k][:], start=(k == 0), stop=(k == KT - 1))
            nc.scalar.activation(omoT[m][:], acc[:, :NP], AF.Identity,
                                 bias=bmoT[:, m:m + 1])
    pF1.release()

    # ---- FFN c-major: Y halves of 16 hidden tiles, Z accum in SBUF ----
    bf1T = bcol("bf1", pF)
    bf2b = brow("bf2", pF)
    zf = [utile(pF, [128, NP], F32R, f"zf{c}") for c in range(8)]
    for hf in range(2):
        yfs = []
        wf1h = None
        for kki in range(16):
            kk = hf * 16 + kki
            if kki % 4 == 0:
                wf1h = []
                for k in range(KT):
                    w = utile(pF, [128, 512], F32R, "wf1h", bufs=9)
                    nc.sync.dma_start(
                        w[:], wd["Wf1"][ts(k, 128), ts(kk // 4, 512)]
                        .bitcast(F32R))
                    wf1h.append(w)
            yp = ps_mm() if kki % 2 == 0 else             ps_av().rearrange("p a b -> p (a b)")
            for k in range(KT):
                nc.tensor.matmul(yp[:, :NP], wf1h[k][:, ts(kki % 4, 128)],
                                 omoT[k][:], start=(k == 0), stop=(k == KT - 1))
            yf = utile(pF, [128, NP], F32R, "yf", bufs=17)
            nc.scalar.activation(yf[:], yp[:, :NP], AF.Silu,
                                 bias=bf1T[:, kk:kk + 1])
            yfs.append(yf)
        for ch in range(2):
            wf2h = []
            for kki in range(16):
                kk = hf * 16 + kki
                w = utile(pF, [128, 512], F32R, "wf2h", bufs=17)
                nc.sync.dma_start(w[:],
                                  wd["Wf2"][ts(kk, 128), ts(ch, 512)].bitcast(F32R))
                wf2h.append(w)
            for cc in range(4):
                c = ch * 4 + cc
                zp = ps_bigA() if cc % 2 == 0 else ps_bigB()
                for kki in range(16):
                    nc.tensor.matmul(zp[:, :NP], wf2h[kki][:, ts(cc, 128)],
                                     yfs[kki][:], start=(kki == 0),
                                     stop=(kki == 15))
                with nc.allow_low_precision(reason="ffn out partials f32r"):
                    if hf == 0:
                        nc.vector.tensor_copy(zf[c][:], zp[:, :NP])
                    else:
                        nc.vector.tensor_tensor(zf[c][:], zf[c][:], zp[:, :NP],
                                                OP.add)
    # ---- transpose back to row-major + bias -> out ----
    for tl in range(4):
        of = utile(pF, [128, C], F32, "of", bufs=2)
        for c in range(8):
            p = ps_av()
            pf = p.rearrange("p a b -> p (a b)")
            nc.tensor.transpose(pf[:PP, :128].bitcast(F32R),
                                zf[c][:, tl * PP:(tl + 1) * PP], ident_r[:, :])
            nc.vector.tensor_tensor(of[:PP, ts(c, 128)], pf[:PP, :128],
                                    bf2b[:PP, ts(c, 128)], OP.add)
        nc.sync.dma_start(out_d[tl * PP:(tl + 1) * PP, :], of[:PP, :])
    pF.release()
    for pool in (pt, pw, psm, pc, pp):
        pool.release()


_BUILT = None


def kernel(**inputs):
    global _BUILT
    if _BUILT is None:
        _BUILT = build(debug=DEBUG)
    nc = _BUILT
    x = np.ascontiguousarray(inputs["x"], dtype=np.float32)
    base = {k: np.ascontiguousarray(v, dtype=np.float32) for k, v in inputs.items()
            if k != "x"}
    in_maps = []
    for i in range(8):
        m = dict(base)
        m["x"] = x[i]
        in_maps.append(m)
    res = run_bass_kernel_spmd(nc, in_maps, core_ids=list(range(8)))
    out = np.stack([res.results[i]["out"] for i in range(8)], axis=0)
    return out.astype(np.float32)



# revision 54
# speedup vs baseline: 1.0321x; 1.0193x over previous
"""AdaptiveTokenMerger (ToMe block + merger) TRN2 Bass kernel.

Data-parallel over batch: 8 samples -> 8 NeuronCores, one sample per core.
Per-core pipeline (sample x [1024, 1024]):
  A (f32, ranking-critical): LN1 -> qkv -> MHA (transposed-softmax with the
    denominator folded in as an appended ones-column of v) -> Wo -> x_attn
  B: metric scores -> node_max/argmax -> ranks via pairwise comparisons ->
    dst scatter-add expressed as a one-hot matmul
  C (f32r): MLP over rows [x1_even(512); dst_new(512)], fused W1/W2 per
    token-quarter, output accumulated in PSUM across all 32 W1 column tiles
  D: pooling as a rank-dependent one-hot matmul -> Wp -> combined = 3q
  E (f32r): multi-query attention  F (f32r): FFN -> out [504, 1024]

Precision: everything upstream of the rank/argmax decisions is true fp32
(4 cyc/row on PE); post-merge matmuls use float32r (TF32-ish, 1 cyc/row).

PSUM budget (8 banks): BIGA/BIGB [128,1024] (2+2), MM [128,512] x2 (2),
AV [128,4,128] x2 (2).
"""
import numpy as np

import concourse.bass as bass
import concourse.tile as tile
from concourse import bacc, mybir
from concourse.bass import ts
from concourse.bass_utils import run_bass_kernel_spmd
from concourse.masks import make_identity

F32 = mybir.dt.float32
F32R = mybir.dt.float32r
BF16 = mybir.dt.bfloat16
U32 = mybir.dt.uint32

N, C, H = 1024, 1024, 16
R = 16
DH = C // H          # 64
NE = N // 2          # 512
NP = (N - R) // 2    # 504
PP = 126             # pooled tokens per partition tile
KT = C // 128        # 8
AF = mybir.ActivationFunctionType
OP = mybir.AluOpType

DEBUG = False


def build(debug=False):
    nc = bacc.Bacc("TRN2", target_bir_lowering=False, debug=False, num_devices=8)
    x_d = nc.dram_tensor("x", [N, C], F32, kind="ExternalInput").ap()
    wd = {}
    for name, shape in [
        ("g1", [C]), ("be1", [C]), ("Wqkv", [C, 3 * C]), ("bqkv", [3 * C]),
        ("Wo", [C, C]), ("bo", [C]), ("g2", [C]), ("be2", [C]),
        ("W1", [C, 4 * C]), ("bm1", [4 * C]), ("W2", [4 * C, C]), ("bm2", [C]),
        ("Wp", [C, C]), ("bp", [C]), ("Wq", [C, C]), ("bq", [C]),
        ("Wk", [C, DH]), ("bk", [DH]), ("Wv", [C, DH]), ("bv", [DH]),
        ("Wmo", [C, C]), ("bmo", [C]), ("Wf1", [C, 4 * C]), ("bf1", [4 * C]),
        ("Wf2", [4 * C, C]), ("bf2", [C]),
    ]:
        wd[name] = nc.dram_tensor(name, shape, F32, kind="ExternalInput").ap()
    out_d = nc.dram_tensor("out", [NP, C], F32, kind="ExternalOutput").ap()
    dbg = {}
    if debug:
        for name, shape in [
            ("dbg_xattn", [N, C]), ("dbg_nm", [NE]), ("dbg_rank", [NE]),
            ("dbg_nodeidx", [NE]), ("dbg_mlpin", [N, C]), ("dbg_mlpout", [N, C]),
            ("dbg_pooled", [NP, C]),
        ]:
            dbg[name] = nc.dram_tensor(name, shape, F32, kind="ExternalOutput").ap()
    with tile.TileContext(nc) as tc:
        _build_tile(nc, tc, x_d, wd, out_d, dbg)
    nc.compile()
    return nc


def _build_tile(nc, tc, x_d, wd, out_d, dbg):
    # DRAM spill buffers
    qkTd = nc.dram_tensor("qkTd", [2 * C, N], F32).ap()
    aoTd = nc.dram_tensor("aoTd", [C, N], F32).ap()
    x1d = nc.dram_tensor("x1d", [N, C], F32).ap()
    dstnd = nc.dram_tensor("dstnd", [NE, C], F32).ap()
    mqaTd = nc.dram_tensor("mqaTd", [C, NP], F32R).ap()

    pc = tc.alloc_tile_pool(name="const", bufs=1)
    psm = tc.alloc_tile_pool(name="small", bufs=1)
    pw = tc.alloc_tile_pool(name="wstream", bufs=2)
    pt = tc.alloc_tile_pool(name="tmp", bufs=2)
    pp = tc.alloc_tile_pool(name="psum", bufs=1, space="PSUM")

    _ct = {}

    def utile(pool, shape, dtype, tag, bufs=None):
        _ct[tag] = _ct.get(tag, 0) + 1
        kw = {"bufs": bufs} if bufs is not None else {}
        return pool.tile(shape, dtype, tag=tag, name=f"{tag}_{_ct[tag]}", **kw)

    def ps_bigA():
        return utile(pp, [128, 1024], F32, "BIGA")

    def ps_bigB():
        return utile(pp, [128, 1024], F32, "BIGB")

    def ps_mm():
        return utile(pp, [128, 512], F32, "MM", bufs=2)

    def ps_av():
        return utile(pp, [128, 4, 128], F32, "AV", bufs=2)

    # ---------- constants ----------
    ident = pc.tile([128, 128], F32)
    make_identity(nc, ident[:])
    ident_r = pc.tile([128, 128], F32R)
    nc.vector.tensor_copy(ident_r[:], ident[:])
    ones_col = pc.tile([1, 128], F32)
    nc.gpsimd.memset(ones_col[:], 1.0)
    ones_col_r = pc.tile([1, 128], F32R)
    nc.vector.tensor_copy(ones_col_r[:], ones_col[:])
    ones_part = pc.tile([128, 8], F32)
    nc.gpsimd.memset(ones_part[:], 1.0)
    piota = pc.tile([128, 1], F32)
    nc.gpsimd.iota(piota[:], [[0, 1]], channel_multiplier=1,
                   allow_small_or_imprecise_dtypes=True)
    iota512_row = pc.tile([1, 512], F32)
    nc.gpsimd.iota(iota512_row[:], [[1, 512]], channel_multiplier=0,
                   allow_small_or_imprecise_dtypes=True)
    iota504_row = pc.tile([1, 504], F32)
    nc.gpsimd.iota(iota504_row[:], [[1, 504]], channel_multiplier=0,
                   allow_small_or_imprecise_dtypes=True)

    def bcast_row(row_ap, n, tag, pool, scale=1.0):
        t = utile(pool, [128, n], F32, tag)
        for c0 in range(0, n, 512):
            cw = min(512, n - c0)
            p = ps_mm()
            nc.tensor.matmul(p[:, :cw], ones_col[:], row_ap[:, c0:c0 + cw],
                             start=True, stop=True)
            if scale == 1.0:
                nc.vector.tensor_copy(t[:, c0:c0 + cw], p[:, :cw])
            else:
                nc.vector.tensor_scalar_mul(t[:, c0:c0 + cw], p[:, :cw], scale)
        return t

    def load_row(dram_ap, n, tag, pool):
        t = utile(pw, [1, n], F32, "rowstg", bufs=1)
        nc.sync.dma_start(t[:], dram_ap[None, :])
        return t

    def brow(name, pool, scale=1.0):
        n = wd[name].shape[0]
        return bcast_row(load_row(wd[name], n, name + "_r", pool), n,
                         name + "_b", pool, scale)

    def bcol(name, pool, scale=1.0):
        n = wd[name].shape[0]
        t = utile(pool, [128, n // 128], F32, name + "_c")
        nc.sync.dma_start(t[:], wd[name].rearrange("(t p) -> p t", p=128))
        if scale != 1.0:
            nc.vector.tensor_scalar_mul(t[:], t[:], scale)
        return t

    IOTA512B = bcast_row(iota512_row[:], 512, "iota512b", pc)
    IOTA504B = bcast_row(iota504_row[:], 504, "iota504b", pc)

    def transpose_blocks(src_tiles, dst, n_rows, n_cols):
        """dst[c, r] = src[r, c]; dst is tile-list or sink(bj, bi, pf, cw, rw)."""
        for bi in range((n_rows + 127) // 128):
            rw = min(128, n_rows - bi * 128)
            for bj in range((n_cols + 127) // 128):
                cw = min(128, n_cols - bj * 128)
                p = ps_av()
                pf = p.rearrange("p a b -> p (a b)")
                nc.tensor.transpose(pf[:cw, :rw],
                                    src_tiles[bi][:rw, bj * 128:bj * 128 + cw],
                                    ident[:rw, :rw])
                if callable(dst):
                    dst(bj, bi, pf, cw, rw)
                else:
                    nc.vector.tensor_copy(dst[bj][:cw, bi * 128:bi * 128 + rw],
                                          pf[:cw, :rw])

    def refined_rsqrt_recip(vv, tag):
        """returns 1/sqrt(vv) with one Newton step on sqrt (ACT sqrt is loose)."""
        s0 = utile(psm, [128, 1], F32, tag + "_s0")
        nc.scalar.sqrt(s0[:], vv[:])
        r0 = utile(psm, [128, 1], F32, tag + "_r0")
        nc.vector.reciprocal(r0[:], s0[:])
        t = utile(psm, [128, 1], F32, tag + "_t")
        nc.vector.tensor_tensor(t[:], vv[:], r0[:], OP.mult)
        nc.vector.tensor_tensor(t[:], t[:], s0[:], OP.add)
        nc.vector.tensor_scalar_mul(t[:], t[:], 0.5)
        rr = utile(psm, [128, 1], F32, tag + "_rr")
        nc.vector.reciprocal(rr[:], t[:])
        return rr

    def layer_norm(src, dst, gb, bb, lnpool):
        m = utile(psm, [128, 1], F32, "ln_m")
        nc.vector.reduce_sum(m[:], src[:, :C], axis=mybir.AxisListType.X)
        nc.vector.tensor_scalar_mul(m[:], m[:], 1.0 / C)
        xc = utile(lnpool, [128, C], F32, "ln_xc", bufs=1)
        nc.vector.tensor_scalar(xc[:], src[:, :C], m[:], None, OP.subtract)
        ss = utile(psm, [128, 1], F32, "ln_ss")
        nc.scalar.activation(dst[:, :C], xc[:], AF.Square, accum_out=ss[:])
        v = utile(psm, [128, 1], F32, "ln_v")
        nc.vector.tensor_scalar(v[:], ss[:], 1.0 / C, 1e-5, OP.mult, OP.add)
        rstd = refined_rsqrt_recip(v, "ln")
        nc.vector.tensor_scalar(dst[:, :C], xc[:], rstd[:], None, OP.mult)
        nc.vector.tensor_tensor(dst[:, :C], dst[:, :C], gb[:], OP.mult)
        nc.vector.tensor_tensor(dst[:, :C], dst[:, :C], bb[:], OP.add)

    # ================= Stage A: LN1 -> hT =================
    pbA = tc.alloc_tile_pool(name="biasA", bufs=1)
    pHT = tc.alloc_tile_pool(name="pHT", bufs=1)
    pVP = tc.alloc_tile_pool(name="pVP", bufs=1)
    pAttn = tc.alloc_tile_pool(name="pAttn", bufs=1)

    g1b = brow("g1", pbA)
    be1b = brow("be1", pbA)
    hT = [utile(pHT, [128, N], F32, f"hT{k}") for k in range(8)]
    ht = []
    for i in range(8):
        xt = utile(pt, [128, C], F32, "xin")
        nc.sync.dma_start(xt[:], x_d[ts(i, 128), :])
        h = utile(pHT, [128, C], F32, "ht", bufs=4)
        layer_norm(xt, h, g1b, be1b, pbA)
        ht.append(h)
    transpose_blocks(ht, hT, N, C)

    # ===== qk^T -> qkTd (DRAM) ; v_pad (SBUF) =====
    bqkT = bcol("bqkv", pbA)
    for mp in range(8):
        accq = ps_bigA()
        acck = ps_bigB()
        for k in range(KT):
            wq = utile(pw, [128, 128], F32, "wqkb", bufs=4)
            nc.sync.dma_start(wq[:], wd["Wqkv"][ts(k, 128), ts(mp, 128)])
            wk = utile(pw, [128, 128], F32, "wqkb", bufs=4)
            nc.sync.dma_start(wk[:],
                              wd["Wqkv"][ts(k, 128), C + mp * 128:C + (mp + 1) * 128])
            for n2 in range(2):
                nc.tensor.matmul(accq[:, ts(n2, 512)], wq[:], hT[k][:, ts(n2, 512)],
                                 start=(k == 0), stop=(k == KT - 1))
                nc.tensor.matmul(acck[:, ts(n2, 512)], wk[:], hT[k][:, ts(n2, 512)],
                                 start=(k == 0), stop=(k == KT - 1))
        stgq = utile(pAttn, [128, N], F32, "qkstg", bufs=2)
        nc.scalar.activation(stgq[:], accq[:], AF.Identity, bias=bqkT[:, mp:mp + 1])
        nc.sync.dma_start(qkTd[ts(mp, 128), :], stgq[:])
        stgk = utile(pAttn, [128, N], F32, "qkstg", bufs=2)
        nc.scalar.activation(stgk[:], acck[:], AF.Identity,
                             bias=bqkT[:, 8 + mp:9 + mp])
        nc.sync.dma_start(qkTd[C + mp * 128:C + (mp + 1) * 128, :], stgk[:])

    bvqkvb = bcast_row(load_row(wd["bqkv"][2 * C:], C, "bvq_r", pbA), C,
                       "bvq_b", pbA)
    v_pad = [utile(pVP, [128, H, DH + 1], F32, f"vp{j}") for j in range(8)]
    for j in range(8):
        nc.vector.memset(v_pad[j][:, :, DH:DH + 1], 1.0)
    for jg in range(4):
        accs = [ps_bigA(), ps_bigB()]
        for k in range(KT):
            wv = utile(pVP, [128, C], F32, "wv", bufs=3)
            nc.sync.dma_start(wv[:], wd["Wqkv"][ts(k, 128), 2 * C:])
            for jj in range(2):
                j = jg * 2 + jj
                for n2 in range(2):
                    nc.tensor.matmul(accs[jj][:, ts(n2, 512)],
                                     hT[k][:, ts(j, 128)], wv[:, ts(n2, 512)],
                                     start=(k == 0), stop=(k == KT - 1))
        for jj in range(2):
            j = jg * 2 + jj
            for h in range(H):
                nc.vector.tensor_tensor(v_pad[j][:, h, :DH],
                                        accs[jj][:, ts(h, DH)],
                                        bvqkvb[:, ts(h, DH)], OP.add)

    # ===== attention: stream kT/qT per head; out -> aoTd (already c-major) ==
    # out[dh|sum, i] = v_pad[j].T @ expT[j, i], accumulated over j-tiles.
    for h in range(H):
        kth = utile(pAttn, [64, N], F32, "kth", bufs=2)
        nc.sync.dma_start(kth[:], qkTd[C + h * 64:C + h * 64 + 64, :])
        qth = utile(pAttn, [64, N], F32, "qth", bufs=2)
        nc.sync.dma_start(qth[:], qkTd[h * 64:h * 64 + 64, :])
        av = [ps_av().rearrange("p a b -> p (a b)") for _ in range(2)]
        for j in range(8):
            for n2 in range(2):
                sp = ps_mm()
                nc.tensor.matmul(sp[:], kth[:, ts(j, 128)], qth[:, ts(n2, 512)],
                                 start=True, stop=True)
                et = utile(pAttn, [128, 512], F32, "exp", bufs=3)
                nc.scalar.activation(et[:], sp[:], AF.Exp, scale=float(DH ** -0.5))
                nc.tensor.matmul(av[n2][:DH + 1, :512], v_pad[j][:, h, :], et[:],
                                 start=(j == 0), stop=(j == 7))
        for n2 in range(2):
            rrow = utile(pAttn, [1, 512], F32, "rrow", bufs=2)
            nc.vector.reciprocal(rrow[:], av[n2][DH:DH + 1, :512])
            rb = ps_mm()
            nc.tensor.matmul(rb[:DH, :512], ones_col[:, :DH], rrow[:],
                             start=True, stop=True)
            rbs = utile(pAttn, [64, 512], F32, "rbs", bufs=2)
            nc.vector.tensor_copy(rbs[:], rb[:DH, :512])
            stg = utile(pAttn, [64, 512], F32, "aot_stg", bufs=2)
            nc.vector.tensor_tensor(stg[:], av[n2][:DH, :512], rbs[:],
                                    OP.mult)
            nc.sync.dma_start(aoTd[h * 64:h * 64 + 64, ts(n2, 512)], stg[:])
    pAttn.release()
    pVP.release()
    pHT.release()
    pbA.release()

    # ================= Wo -> x_attn, x1 (-> DRAM), metric =================
    pB2 = tc.alloc_tile_pool(name="pB2", bufs=1)
    bob = brow("bo", pB2)
    xa = [utile(pB2, [128, C], F32, f"xa{m}") for m in range(8)]
    woR = [utile(pB2, [128, C], F32, f"woR{k}") for k in range(8)]
    for k in range(KT):
        nc.sync.dma_start(woR[k][:], wd["Wo"][ts(k, 128), :])
    rn = psm.tile([128, 8], F32)
    for m in range(8):
        acc = ps_bigA()
        for k in range(KT):
            ao = utile(pw, [128, 128], F32, "wqkb", bufs=4)
            nc.sync.dma_start(ao[:], aoTd[ts(k, 128), ts(m, 128)])
            for n2 in range(2):
                nc.tensor.matmul(acc[:, ts(n2, 512)], ao[:],
                                 woR[k][:, ts(n2, 512)],
                                 start=(k == 0), stop=(k == KT - 1))
        nc.vector.tensor_tensor(xa[m][:], acc[:], bob[:], OP.add)
        xt = utile(pt, [128, C], F32, "xin")
        nc.sync.dma_start(xt[:], x_d[ts(m, 128), :])
        x1stg = utile(pw, [128, C], F32, "x1stg", bufs=2)
        nc.vector.tensor_tensor(x1stg[:], xa[m][:], xt[:], OP.add)
        nc.sync.dma_start(x1d[ts(m, 128), :], x1stg[:])
        ss = utile(psm, [128, 1], F32, "nrm_ss")
        sq = utile(pt, [128, C], F32, "ln_xc")
        nc.scalar.activation(sq[:], xa[m][:], AF.Square, accum_out=ss[:])
        rr = refined_rsqrt_recip(ss, "nrm")
        nc.vector.tensor_copy(rn[:, m:m + 1], rr[:])
        nc.vector.tensor_scalar(xa[m][:], xa[m][:], rn[:, m:m + 1], None, OP.mult)
        if dbg:
            nc.sync.dma_start(dbg["dbg_xattn"][ts(m, 128), :], xa[m][:])

    # ===== de-interleave metric -> maT/mbT; scores; node stats; ranks =====
    pB3 = tc.alloc_tile_pool(name="pB3", bufs=1)
    xae = [utile(pB3, [128, C], F32, f"xae{m}") for m in range(4)]
    xao = [utile(pB3, [128, C], F32, f"xao{m}") for m in range(4)]
    for m in range(4):
        nc.sync.dma_start(xae[m][:64, :], xa[2 * m][0:128:2, :])
        nc.sync.dma_start(xae[m][64:, :], xa[2 * m + 1][0:128:2, :])
        nc.sync.dma_start(xao[m][:64, :], xa[2 * m][1:128:2, :])
        nc.sync.dma_start(xao[m][64:, :], xa[2 * m + 1][1:128:2, :])
    pB4 = tc.alloc_tile_pool(name="pB4", bufs=1)
    maT = [utile(pB4, [128, NE], F32, f"maT{k}") for k in range(8)]
    mbT = [utile(pB4, [128, NE], F32, f"mbT{k}") for k in range(8)]
    transpose_blocks(xae, maT, NE, C)
    transpose_blocks(xao, mbT, NE, C)

    nm_t = psm.tile([128, 4], F32)
    ni_t = psm.tile([128, 4], F32)
    for m in range(4):
        acc = ps_bigA() if m % 2 == 0 else ps_bigB()
        for k in range(KT):
            nc.tensor.matmul(acc[:, :512], maT[k][:, ts(m, 128)], mbT[k][:],
                             start=(k == 0), stop=(k == KT - 1))
        mx8 = utile(psm, [128, 8], F32, "mx8")
        ix8 = utile(psm, [128, 8], U32, "ix8")
        nc.vector.max_with_indices(mx8[:], ix8[:], acc[:, :512])
        nc.vector.tensor_copy(nm_t[:, m:m + 1], mx8[:, 0:1])
        nc.vector.tensor_copy(ni_t[:, m:m + 1], ix8[:, 0:1])

    nm_row = utile(pB4, [1, 512], F32, "nm_row")
    for m in range(4):
        p = ps_av()
        pf = p.rearrange("p a b -> p (a b)")
        nc.tensor.transpose(pf[:1, :128], nm_t[:, m:m + 1], ident[:])
        nc.vector.tensor_copy(nm_row[:, ts(m, 128)], pf[:1, :128])
    NMB = bcast_row(nm_row[:], 512, "nmb", pB4)

    rank_t = psm.tile([128, 4], F32)
    for m in range(4):
        gt = utile(pB4, [128, 512], F32, "rk_gt", bufs=1)
        nc.vector.tensor_scalar(gt[:], NMB[:], nm_t[:, m:m + 1], None, OP.is_gt)
        eq = utile(pB4, [128, 512], F32, "rk_eq", bufs=1)
        nc.vector.tensor_scalar(eq[:], NMB[:], nm_t[:, m:m + 1], None, OP.is_equal)
        flt = utile(pB4, [128, 512], F32, "rk_flt", bufs=1)
        pio = utile(psm, [128, 1], F32, "rk_pio")
        nc.vector.tensor_scalar_add(pio[:], piota[:], float(128 * m))
        nc.vector.tensor_scalar(flt[:], IOTA512B[:], pio[:], None, OP.is_lt)
        nc.vector.tensor_tensor(eq[:], eq[:], flt[:], OP.mult)
        nc.vector.tensor_tensor(gt[:], gt[:], eq[:], OP.add)
        nc.vector.reduce_sum(rank_t[:, m:m + 1], gt[:], axis=mybir.AxisListType.X)
    if dbg:
        for (tt, nme) in [(nm_t, "dbg_nm"), (rank_t, "dbg_rank"),
                          (ni_t, "dbg_nodeidx")]:
            nc.sync.dma_start(dbg[nme].rearrange("(m p) -> p m", p=128), tt[:])
    pB4.release()
    pB3.release()
    pB2.release()

    # ================= dst merge (x1 from DRAM; dstn -> DRAM) =============
    pM = tc.alloc_tile_pool(name="pM", bufs=1)
    x1e = [utile(pM, [128, C + 8], F32R, f"x1e{m}") for m in range(4)]
    x1o = [utile(pM, [128, C], F32, f"x1o{m}") for m in range(4)]
    for m in range(4):
        nc.vector.tensor_copy(x1e[m][:, C:C + 8], ones_part[:])
        nc.sync.dma_start(x1e[m][:, :C],
                          x1d[256 * m:256 * m + 256:2, :].bitcast(F32R))
        nc.sync.dma_start(x1o[m][:], x1d[256 * m + 1:256 * m + 256:2, :])
    st = [utile(pM, [128, 512], F32R, f"st{m}") for m in range(4)]
    for m in range(4):
        msk = utile(psm, [128, 1], F32, "st_m")
        nc.vector.tensor_scalar(msk[:], rank_t[:, m:m + 1], float(R) - 0.5, None,
                                OP.is_lt)
        nc.vector.tensor_scalar(st[m][:], IOTA512B[:], ni_t[:, m:m + 1], None,
                                OP.is_equal)
        nc.vector.tensor_scalar(st[m][:], st[m][:], msk[:], None, OP.mult)
    for m in range(4):
        acc = ps_bigA()
        cacc = ps_av()
        for k in range(4):
            for n2 in range(2):
                nc.tensor.matmul(acc[:, ts(n2, 512)], st[k][:, ts(m, 128)],
                                 x1e[k][:, n2 * 512:n2 * 512 + 512],
                                 start=(k == 0), stop=(k == 3))
            nc.tensor.matmul(cacc[:, 0, :8], st[k][:, ts(m, 128)],
                             x1e[k][:, C:C + 8], start=(k == 0), stop=(k == 3))
        cnt = utile(psm, [128, 1], F32, "cnt")
        nc.vector.tensor_scalar_add(cnt[:], cacc[:, 0, 0:1], 1.0)
        rec = utile(psm, [128, 1], F32, "cntr")
        nc.vector.reciprocal(rec[:], cnt[:])
        dst_stg = utile(pM, [128, C], F32, "dst_stg", bufs=2)
        nc.vector.tensor_tensor(dst_stg[:], acc[:], x1o[m][:], OP.add)
        nc.vector.tensor_scalar(dst_stg[:], dst_stg[:], rec[:], None, OP.mult)
        nc.sync.dma_start(dstnd[ts(m, 128), :], dst_stg[:])

    # ========== MLP (f32r): W1/W2 streamed once; SBUF out accumulation ======
    def row_src_ap(i):
        if i < 4:
            return x1d[256 * i:256 * i + 256:2, :]
        return dstnd[ts(i - 4, 128), :]

    pM.release()
    pZ = tc.alloc_tile_pool(name="pZ", bufs=1)
    pC4 = tc.alloc_tile_pool(name="pC4", bufs=1)
    g2b = brow("g2", pC4)
    be2b = brow("be2", pC4)
    h2 = []
    for i in range(8):
        rsrc = utile(pt, [128, C], F32, "xin")
        nc.sync.dma_start(rsrc[:], row_src_ap(i))
        h = utile(pC4, [128, C], F32, "ht", bufs=2)
        layer_norm(rsrc, h, g2b, be2b, pC4)
        h2.append(h)
        if dbg:
            nc.sync.dma_start(dbg["dbg_mlpin"][ts(i, 128), :], rsrc[:])
    h2T = [utile(pC4, [128, N], F32R, f"h2T{k}") for k in range(8)]
    transpose_blocks(h2, h2T, N, C)

    bm1T = bcol("bm1", pC4)
    bm2T = bcol("bm2", psm)
    # ---- MLP c-major: Y^T groups of 8 hidden tiles, Z^T accum in SBUF ----
    zacc = [utile(pZ, [128, N], F32R, f"zacc{c}") for c in range(8)]
    for g in range(4):
        ytiles = []
        w1h = None
        for mi in range(8):
            mt = g * 8 + mi
            if mi % 4 == 0:
                w1h = []
                for k in range(KT):
                    w = utile(pC4, [128, 512], F32R, "w1h", bufs=8)
                    nc.sync.dma_start(
                        w[:], wd["W1"][ts(k, 128), ts(2 * g + mi // 4, 512)]
                        .bitcast(F32R))
                    w1h.append(w)
            yp = ps_bigA() if mi % 2 == 0 else ps_bigB()
            for k in range(KT):
                for n2 in range(2):
                    nc.tensor.matmul(yp[:, ts(n2, 512)],
                                     w1h[k][:, ts(mi % 4, 128)],
                                     h2T[k][:, ts(n2, 512)],
                                     start=(k == 0), stop=(k == KT - 1))
            yt = utile(pC4, [128, N], F32R, "yt", bufs=8)
            nc.scalar.activation(yt[:], yp[:], AF.Gelu_apprx_tanh,
                                 bias=bm1T[:, mt:mt + 1])
            ytiles.append(yt)
        for ch in range(2):
            w2h = []
            for mi in range(8):
                mt = g * 8 + mi
                w = utile(pC4, [128, 512], F32R, "w2h", bufs=8)
                nc.sync.dma_start(w[:],
                                  wd["W2"][ts(mt, 128), ts(ch, 512)].bitcast(F32R))
                w2h.append(w)
            for cc in range(4):
                c = ch * 4 + cc
                for n2 in range(2):
                    zp = ps_mm() if n2 == 0 else                         ps_av().rearrange("p a b -> p (a b)")
                    for mi in range(8):
                        nc.tensor.matmul(zp[:, :512],
                                         w2h[mi][:, ts(cc, 128)],
                                         ytiles[mi][:, ts(n2, 512)],
                                         start=(mi == 0), stop=(mi == 7))
                    with nc.allow_low_precision(reason="mlp psums f32r"):
                        if g == 0:
                            nc.vector.tensor_scalar(zacc[c][:, ts(n2, 512)],
                                                    zp[:, :512],
                                                    bm2T[:, c:c + 1], None,
                                                    OP.add)
                        else:
                            nc.vector.tensor_tensor(zacc[c][:, ts(n2, 512)],
                                                    zacc[c][:, ts(n2, 512)],
                                                    zp[:, :512], OP.add)

    # ---- residual + transpose -> row-major mod (bf16, pZ-resident) ----
    mod_sb = [utile(pZ, [128, N], BF16, f"modr{i}") for i in range(8)]
    for i in range(8):
        resf = utile(pt, [128, C], F32, "xin")
        nc.sync.dma_start(resf[:], row_src_ap(i))
        zt = ps_bigA() if i % 2 == 0 else ps_bigB()
        for c in range(8):
            nc.tensor.transpose(zt[:, ts(c, 128)].bitcast(F32R),
                                zacc[c][:, ts(i, 128)], ident_r[:, :])
        with nc.allow_low_precision(reason="mod rows stored bf16"):
            nc.vector.tensor_tensor(mod_sb[i][:], zt[:], resf[:], OP.add)
    if dbg:
        for i in range(8):
            mff = utile(pC4, [128, C], F32, "mof", bufs=2)
            nc.vector.tensor_copy(mff[:], mod_sb[i][:])
            nc.sync.dma_start(dbg["dbg_mlpout"][ts(i, 128), :], mff[:])
    pC4.release()

    # ================= Stage D: pooling + Wp -> combined^T =================
    pD = tc.alloc_tile_pool(name="pD", bufs=1)
    # ApT[p, f] = 0.5 iff source row p pools into output f:
    #   even block: base = rank[p]-16, match iff (2f - base) in {-1, 0}
    #   dst  block: base = d,          match iff (2(f-248) - base) in {-1, 0}
    iota2e = utile(pD, [128, 504], F32, "iota2e")
    nc.vector.tensor_scalar_mul(iota2e[:], IOTA504B[:], 2.0)
    apT = [utile(pD, [128, 504], BF16, f"apT{m}") for m in range(8)]
    for m in range(8):
        base = utile(psm, [128, 1], F32, "ap_r")
        if m < 4:
            nc.vector.tensor_scalar_add(base[:], rank_t[:, m:m + 1], -float(R))
        else:
            nc.vector.tensor_scalar_add(base[:], piota[:],
                                        float(128 * (m - 4) + NE - R))
        d1 = utile(pD, [128, 504], F32, "ap_d1")
        nc.vector.tensor_scalar(d1[:], iota2e[:], base[:], None, OP.subtract)
        a1 = utile(pD, [128, 504], F32, "ap_a1")
        nc.vector.tensor_scalar(a1[:], d1[:], -1.5, None, OP.is_ge)
        b1 = utile(pD, [128, 504], F32, "ap_b1")
        nc.vector.tensor_scalar(b1[:], d1[:], 0.5, None, OP.is_le)
        nc.vector.scalar_tensor_tensor(apT[m][:], a1[:], 0.5, b1[:],
                                       OP.mult, OP.mult)
    # pooledT[c][128c, 504] = sum_i mod_sb[i][:, c-tile]^T @ apT[i]
    pooledT = [utile(pD, [128, NP], F32R, f"pooledT{k}") for k in range(8)]
    for c in range(8):
        zp = ps_mm() if c % 2 == 0 else         ps_av().rearrange("p a b -> p (a b)")
        for i in range(8):
            nc.tensor.matmul(zp[:, :NP], mod_sb[i][:, ts(c, 128)], apT[i][:],
                             start=(i == 0), stop=(i == 7))
        nc.vector.tensor_copy(pooledT[c][:], zp[:, :NP])
    if dbg:
        for m in range(4):
            pst = utile(pD, [128, C], F32, "pstg", bufs=2)
            for bj in range(8):
                p = ps_av()
                pf = p.rearrange("p a b -> p (a b)")
                nc.tensor.transpose(pf[:PP, :128].bitcast(F32R),
                                    pooledT[bj][:, m * PP:(m + 1) * PP],
                                    ident_r[:, :])
                nc.vector.tensor_copy(pst[:PP, ts(bj, 128)], pf[:PP, :128])
            nc.sync.dma_start(dbg["dbg_pooled"][ts(m, PP), :], pst[:PP, :])

    pE = tc.alloc_tile_pool(name="pE", bufs=1)
    bp3T = bcol("bp", pD, scale=3.0)
    cmbTr = [utile(pD, [128, NP], F32R, f"cmbTr{m}") for m in range(8)]
    for mg in range(2):
        wcs = []
        for k in range(KT):
            wcr = utile(pD, [128, 512], F32R, f"wpc{k}", bufs=1)
            nc.sync.dma_start(wcr[:], wd["Wp"][ts(k, 128), ts(mg, 512)].bitcast(F32R))
            wcs.append(wcr)
        for mi in range(4):
            m = mg * 4 + mi
            acc = ps_mm()
            for k in range(KT):
                nc.tensor.matmul(acc[:, :NP], wcs[k][:, ts(mi, 128)],
                                 pooledT[k][:], start=(k == 0), stop=(k == KT - 1))
            nc.scalar.activation(cmbTr[m][:], acc[:, :NP], AF.Identity,
                                 bias=bp3T[:, m:m + 1], scale=3.0)

    # ================= Stage E: MQA =================
    bqT = bcol("bq", pE)

    _wq512 = {}

    def make_mqT(m):
        mg = m // 4
        if mg not in _wq512:
            ws = []
            for k in range(KT):
                w = utile(pE, [128, 512], F32R, "wq512", bufs=8)
                nc.sync.dma_start(
                    w[:], wd["Wq"][ts(k, 128), ts(mg, 512)].bitcast(F32R))
                ws.append(w)
            _wq512.clear()
            _wq512[mg] = ws
        ws = _wq512[mg]
        acc = ps_mm()
        for k in range(KT):
            nc.tensor.matmul(acc[:, :NP], ws[k][:, ts(m % 4, 128)], cmbTr[k][:],
                             start=(k == 0), stop=(k == KT - 1))
        t = utile(pE, [128, NP], F32R, "mqT", bufs=2)
        with nc.allow_low_precision(reason="bias add, f32r storage"):
            nc.vector.tensor_scalar(t[:], acc[:, :NP], bqT[:, m:m + 1], None,
                                    OP.add)
        return t

    wkvr = utile(pE, [128, KT, 2 * DH], F32R, "wkvr")
    for k in range(KT):
        nc.sync.dma_start(wkvr[:, k, :DH], wd["Wk"][ts(k, 128), :].bitcast(F32R))
        nc.sync.dma_start(wkvr[:, k, DH:], wd["Wv"][ts(k, 128), :].bitcast(F32R))
    bkT = utile(pE, [64, 1], F32, "bkT")
    nc.sync.dma_start(bkT[:], wd["bk"][:, None])
    mkT = utile(pE, [128, NP], F32R, "mkT")
    macc = ps_mm()
    for k in range(KT):
        nc.tensor.matmul(macc[:64, :NP], wkvr[:, k, :DH], cmbTr[k][:],
                         start=(k == 0), stop=(k == KT - 1))
    mkf = utile(pE, [64, NP], F32, "mkf")
    nc.scalar.activation(mkf[:], macc[:64, :NP], AF.Identity, bias=bkT[:])
    nc.vector.tensor_copy(mkT[:64, :], mkf[:])
    nc.sync.dma_start(mkT[64:, :], mkT[:64, :])

    bvb = bcast_row(load_row(wd["bv"], DH, "bv_r", pE), DH, "bv_b", pE)
    mv_pad = [utile(pE, [128, DH + 1], F32R, f"mvp{m}") for m in range(4)]
    for m in range(4):
        acc = ps_av()
        for k in range(KT):
            nc.tensor.matmul(acc[:PP, 0, :DH], cmbTr[k][:, m * PP:(m + 1) * PP],
                             wkvr[:, k, DH:], start=(k == 0), stop=(k == KT - 1))
        nc.vector.tensor_copy(mv_pad[m][:, DH:], ones_part[:, :1])
        nc.vector.tensor_tensor(mv_pad[m][:PP, :DH], acc[:PP, 0, :DH], bvb[:PP, :],
                                OP.add)

    mqT_cur = None
    for h in range(H):
        po = (h % 2) * 64
        if h % 2 == 0:
            mqT_cur = make_mqT(h // 2)
        mqT_h = mqT_cur[po:po + 64, :]
        spA = ps_bigA()
        spB = ps_bigB()
        ep = []
        for mm in range(4):
            spbig = spA if mm < 2 else spB
            sp = spbig[:, 512 * (mm % 2):512 * (mm % 2) + NP]
            nc.tensor.matmul(sp[:PP, :NP], mkT[po:po + 64, mm * PP:(mm + 1) * PP],
                             mqT_h[:], start=True, stop=True)
            et = utile(pE, [128, NP], F32R, "e2", bufs=6)
            nc.scalar.activation(et[:PP, :], sp[:PP, :NP], AF.Exp,
                                 scale=float(DH ** -0.5))
            ep.append(et)
        av2 = ps_av().rearrange("p a b -> p (a b)")
        for mm in range(4):
            nc.tensor.matmul(av2[:DH + 1, :NP], mv_pad[mm][:PP, :],
                             ep[mm][:PP, :], start=(mm == 0), stop=(mm == 3))
        rrow = utile(pE, [1, NP], F32R, "rrow2", bufs=2)
        with nc.allow_low_precision(reason="softmax denom; 11-bit ok downstream"):
            nc.vector.reciprocal(rrow[:], av2[DH:DH + 1, :NP])
        rb = ps_mm()
        nc.tensor.matmul(rb[:DH, :NP], ones_col_r[:, :DH], rrow[:],
                         start=True, stop=True)
        rbs = utile(pE, [64, NP], F32, "rbs2", bufs=2)
        nc.vector.tensor_copy(rbs[:], rb[:DH, :NP])
        stg = utile(pE, [64, NP], F32R, "mqstg", bufs=2)
        nc.vector.tensor_tensor(stg[:], av2[:DH, :NP], rbs[:], OP.mult)
        nc.sync.dma_start(mqaTd[h * 64:h * 64 + 64, :], stg[:])
    pE.release()
    pD.release()
    pZ.release()

    # ================= Stage F: Wmo + FFN =================
    pF = tc.alloc_tile_pool(name="pF", bufs=1)
    pF1 = tc.alloc_tile_pool(name="pF1", bufs=1)
    mqaT = [utile(pF1, [128, NP], F32R, f"mqaT{k}") for k in range(8)]
    for k in range(8):
        nc.sync.dma_start(mqaT[k][:, :NP], mqaTd[ts(k, 128), :])
    bmoT = bcol("bmo", pF1)
    omoT = [utile(pF, [128, NP], F32R, f"omoT{m}") for m in range(8)]
    for mg in range(2):
        wcs = []
        for k in range(KT):
            wcr = utile(pF1, [128, 512], F32R, f"wmc{k}", bufs=1)
            nc.sync.dma_start(wcr[:],
                              wd["Wmo"][ts(k, 128), ts(mg, 512)].bitcast(F32R))
            wcs.append(wcr)
        for mi in range(4):
            m = mg * 4 + mi
            acc = ps_mm()
            for k in range(KT):
                nc.tensor.matmul(acc[:, :NP], wcs[k][:, ts(mi, 128)],
                                 mqaT[k][:], start=(k == 0), stop=(k == KT - 1))
            nc.scalar.activation(omoT[m][:], acc[:, :NP], AF.Identity,
                                 bias=bmoT[:, m:m + 1])
    pF1.release()

    # ---- FFN c-major: Y halves of 16 hidden tiles, Z accum in SBUF ----
    bf1T = bcol("bf1", pF)
    bf2b = brow("bf2", pF)
    zf = [utile(pF, [128, NP], F32R, f"zf{c}") for c in range(8)]
    for hf in range(2):
        yfs = []
        wf1h = None
        for kki in range(16):
            kk = hf * 16 + kki
            if kki % 4 == 0:
                wf1h = []
                for k in range(KT):
                    w = utile(pF, [128, 512], F32R, "wf1h", bufs=9)
                    nc.sync.dma_start(
                        w[:], wd["Wf1"][ts(k, 128), ts(kk // 4, 512)]
                        .bitcast(F32R))
                    wf1h.append(w)
            yp = ps_mm() if kki % 2 == 0 else             ps_av().rearrange("p a b -> p (a b)")
            for k in range(KT):
                nc.tensor.matmul(yp[:, :NP], wf1h[k][:, ts(kki % 4, 128)],
                                 omoT[k][:], start=(k == 0), stop=(k == KT - 1))
            yf = utile(pF, [128, NP], F32R, "yf", bufs=17)
            nc.scalar.activation(yf[:], yp[:, :NP], AF.Silu,
                                 bias=bf1T[:, kk:kk + 1])
            yfs.append(yf)
        for ch in range(2):
            wf2h = []
            for kki in range(16):
                kk = hf * 16 + kki
                w = utile(pF, [128, 512], F32R, "wf2h", bufs=17)
                nc.sync.dma_start(w[:],
                                  wd["Wf2"][ts(kk, 128), ts(ch, 512)].bitcast(F32R))
                wf2h.append(w)
            for cc in range(4):
                c = ch * 4 + cc
                zp = ps_bigA() if cc % 2 == 0 else ps_bigB()
                for kki in range(16):
                    nc.tensor.matmul(zp[:, :NP], wf2h[kki][:, ts(cc, 128)],
                                     yfs[kki][:], start=(kki == 0),
                                     stop=(kki == 15))
                with nc.allow_low_precision(reason="ffn out partials f32r"):
                    if hf == 0:
                        nc.vector.tensor_copy(zf[c][:], zp[:, :NP])
                    else:
                        nc.vector.tensor_tensor(zf[c][:], zf[c][:], zp[:, :NP],
                                                OP.add)
    # ---- transpose back to row-major + bias -> out ----
    for tl in range(4):
        of = utile(pF, [128, C], F32, "of", bufs=2)
        for c in range(8):
            p = ps_av()
            pf = p.rearrange("p a b -> p (a b)")
            nc.tensor.transpose(pf[:PP, :128].bitcast(F32R),
                                zf[c][:, tl * PP:(tl + 1) * PP], ident_r[:, :])
            nc.vector.tensor_tensor(of[:PP, ts(c, 128)], pf[:PP, :128],
                                    bf2b[:PP, ts(c, 128)], OP.add)
        nc.sync.dma_start(out_d[tl * PP:(tl + 1) * PP, :], of[:PP, :])
    pF.release()
    for pool in (pt, pw, psm, pc, pp):
        pool.release()


_BUILT = None


def kernel(**inputs):
    global _BUILT
    if _BUILT is None:
        _BUILT = build(debug=DEBUG)
    nc = _BUILT
    x = np.ascontiguousarray(inputs["x"], dtype=np.float32)
    base = {k: np.ascontiguousarray(v, dtype=np.float32) for k, v in inputs.items()
            if k != "x"}
    in_maps = []
    for i in range(8):
        m = dict(base)
        m["x"] = x[i]
        in_maps.append(m)
    res = run_bass_kernel_spmd(nc, in_maps, core_ids=list(range(8)))
    out = np.stack([res.results[i]["out"] for i in range(8)], axis=0)
    return out.astype(np.float32)



# revision 65
# speedup vs baseline: 1.0384x; 1.0060x over previous
"""AdaptiveTokenMerger (ToMe block + merger) TRN2 Bass kernel.

Data-parallel over batch: 8 samples -> 8 NeuronCores, one sample per core.
Per-core pipeline (sample x [1024, 1024]):
  A (f32, ranking-critical): LN1 -> qkv -> MHA (transposed-softmax with the
    denominator folded in as an appended ones-column of v) -> Wo -> x_attn
  B: metric scores -> node_max/argmax -> ranks via pairwise comparisons ->
    dst scatter-add expressed as a one-hot matmul
  C (f32r): MLP over rows [x1_even(512); dst_new(512)], fused W1/W2 per
    token-quarter, output accumulated in PSUM across all 32 W1 column tiles
  D: pooling as a rank-dependent one-hot matmul -> Wp -> combined = 3q
  E (f32r): multi-query attention  F (f32r): FFN -> out [504, 1024]

Precision: everything upstream of the rank/argmax decisions is true fp32
(4 cyc/row on PE); post-merge matmuls use float32r (TF32-ish, 1 cyc/row).

PSUM budget (8 banks): BIGA/BIGB [128,1024] (2+2), MM [128,512] x2 (2),
AV [128,4,128] x2 (2).
"""
import numpy as np

import concourse.bass as bass
import concourse.tile as tile
from concourse import bacc, mybir
from concourse.bass import ts
from concourse.bass_utils import run_bass_kernel_spmd
from concourse.masks import make_identity

F32 = mybir.dt.float32
F32R = mybir.dt.float32r
FP16 = mybir.dt.float16
BF16 = mybir.dt.bfloat16
U32 = mybir.dt.uint32

N, C, H = 1024, 1024, 16
R = 16
DH = C // H          # 64
NE = N // 2          # 512
NP = (N - R) // 2    # 504
PP = 126             # pooled tokens per partition tile
KT = C // 128        # 8
AF = mybir.ActivationFunctionType
OP = mybir.AluOpType

DEBUG = False


def build(debug=False):
    nc = bacc.Bacc("TRN2", target_bir_lowering=False, debug=False, num_devices=8)
    x_d = nc.dram_tensor("x", [N, C], F32, kind="ExternalInput").ap()
    wd = {}
    for name, shape in [
        ("g1", [C]), ("be1", [C]), ("Wqkv", [C, 3 * C]), ("bqkv", [3 * C]),
        ("Wo", [C, C]), ("bo", [C]), ("g2", [C]), ("be2", [C]),
        ("W1", [C, 4 * C]), ("bm1", [4 * C]), ("W2", [4 * C, C]), ("bm2", [C]),
        ("Wp", [C, C]), ("bp", [C]), ("Wq", [C, C]), ("bq", [C]),
        ("Wk", [C, DH]), ("bk", [DH]), ("Wv", [C, DH]), ("bv", [DH]),
        ("Wmo", [C, C]), ("bmo", [C]), ("Wf1", [C, 4 * C]), ("bf1", [4 * C]),
        ("Wf2", [4 * C, C]), ("bf2", [C]),
    ]:
        wd[name] = nc.dram_tensor(name, shape, F32, kind="ExternalInput").ap()
    out_d = nc.dram_tensor("out", [NP, C], F32, kind="ExternalOutput").ap()
    dbg = {}
    if debug:
        for name, shape in [
            ("dbg_xattn", [N, C]), ("dbg_nm", [NE]), ("dbg_rank", [NE]),
            ("dbg_nodeidx", [NE]), ("dbg_mlpin", [N, C]), ("dbg_mlpout", [N, C]),
            ("dbg_pooled", [NP, C]),
        ]:
            dbg[name] = nc.dram_tensor(name, shape, F32, kind="ExternalOutput").ap()
    with tile.TileContext(nc) as tc:
        _build_tile(nc, tc, x_d, wd, out_d, dbg)
    nc.compile()
    return nc


def _build_tile(nc, tc, x_d, wd, out_d, dbg):
    # DRAM spill buffers
    qkTd = nc.dram_tensor("qkTd", [2 * C, N], F32).ap()
    aoTd = nc.dram_tensor("aoTd", [C, N], F32).ap()
    x1d = nc.dram_tensor("x1d", [N, C], F32).ap()
    dstnd = nc.dram_tensor("dstnd", [NE, C], F32).ap()
    mqaTd = nc.dram_tensor("mqaTd", [C, NP], F32R).ap()

    pc = tc.alloc_tile_pool(name="const", bufs=1)
    psm = tc.alloc_tile_pool(name="small", bufs=1)
    pw = tc.alloc_tile_pool(name="wstream", bufs=2)
    pt = tc.alloc_tile_pool(name="tmp", bufs=2)
    pp = tc.alloc_tile_pool(name="psum", bufs=1, space="PSUM")

    _ct = {}

    def utile(pool, shape, dtype, tag, bufs=None):
        _ct[tag] = _ct.get(tag, 0) + 1
        kw = {"bufs": bufs} if bufs is not None else {}
        return pool.tile(shape, dtype, tag=tag, name=f"{tag}_{_ct[tag]}", **kw)

    def ps_bigA():
        return utile(pp, [128, 1024], F32, "BIGA")

    def ps_bigB():
        return utile(pp, [128, 1024], F32, "BIGB")

    def ps_mm():
        return utile(pp, [128, 512], F32, "MM", bufs=2)

    def ps_av():
        return utile(pp, [128, 4, 128], F32, "AV", bufs=2)

    # ---------- constants ----------
    ident = pc.tile([128, 128], F32)
    make_identity(nc, ident[:])
    ident_r = pc.tile([128, 128], F32R)
    nc.vector.tensor_copy(ident_r[:], ident[:])
    ones_col = pc.tile([1, 128], F32)
    nc.gpsimd.memset(ones_col[:], 1.0)
    ones_col_r = pc.tile([1, 128], F32R)
    nc.vector.tensor_copy(ones_col_r[:], ones_col[:])
    ones_part = pc.tile([128, 8], F32)
    nc.gpsimd.memset(ones_part[:], 1.0)
    piota = pc.tile([128, 1], F32)
    nc.gpsimd.iota(piota[:], [[0, 1]], channel_multiplier=1,
                   allow_small_or_imprecise_dtypes=True)
    iota512_row = pc.tile([1, 512], F32)
    nc.gpsimd.iota(iota512_row[:], [[1, 512]], channel_multiplier=0,
                   allow_small_or_imprecise_dtypes=True)
    iota504_row = pc.tile([1, 504], F32)
    nc.gpsimd.iota(iota504_row[:], [[1, 504]], channel_multiplier=0,
                   allow_small_or_imprecise_dtypes=True)

    def bcast_row(row_ap, n, tag, pool, scale=1.0):
        t = utile(pool, [128, n], F32, tag)
        for c0 in range(0, n, 512):
            cw = min(512, n - c0)
            p = ps_mm()
            nc.tensor.matmul(p[:, :cw], ones_col[:], row_ap[:, c0:c0 + cw],
                             start=True, stop=True)
            if scale == 1.0:
                nc.vector.tensor_copy(t[:, c0:c0 + cw], p[:, :cw])
            else:
                nc.vector.tensor_scalar_mul(t[:, c0:c0 + cw], p[:, :cw], scale)
        return t

    def load_row(dram_ap, n, tag, pool):
        t = utile(pw, [1, n], F32, "rowstg", bufs=1)
        nc.sync.dma_start(t[:], dram_ap[None, :])
        return t

    def brow(name, pool, scale=1.0):
        n = wd[name].shape[0]
        return bcast_row(load_row(wd[name], n, name + "_r", pool), n,
                         name + "_b", pool, scale)

    def bcol(name, pool, scale=1.0):
        n = wd[name].shape[0]
        t = utile(pool, [128, n // 128], F32, name + "_c")
        nc.sync.dma_start(t[:], wd[name].rearrange("(t p) -> p t", p=128))
        if scale != 1.0:
            nc.vector.tensor_scalar_mul(t[:], t[:], scale)
        return t

    IOTA512B = bcast_row(iota512_row[:], 512, "iota512b", pc)
    IOTA504B = bcast_row(iota504_row[:], 504, "iota504b", pc)

    def transpose_blocks(src_tiles, dst, n_rows, n_cols):
        """dst[c, r] = src[r, c]; dst is tile-list or sink(bj, bi, pf, cw, rw)."""
        for bi in range((n_rows + 127) // 128):
            rw = min(128, n_rows - bi * 128)
            for bj in range((n_cols + 127) // 128):
                cw = min(128, n_cols - bj * 128)
                p = ps_av()
                pf = p.rearrange("p a b -> p (a b)")
                nc.tensor.transpose(pf[:cw, :rw],
                                    src_tiles[bi][:rw, bj * 128:bj * 128 + cw],
                                    ident[:rw, :rw])
                if callable(dst):
                    dst(bj, bi, pf, cw, rw)
                else:
                    nc.vector.tensor_copy(dst[bj][:cw, bi * 128:bi * 128 + rw],
                                          pf[:cw, :rw])

    def refined_rsqrt_recip(vv, tag):
        """returns 1/sqrt(vv) with one Newton step on sqrt (ACT sqrt is loose)."""
        s0 = utile(psm, [128, 1], F32, tag + "_s0")
        nc.scalar.sqrt(s0[:], vv[:])
        r0 = utile(psm, [128, 1], F32, tag + "_r0")
        nc.vector.reciprocal(r0[:], s0[:])
        t = utile(psm, [128, 1], F32, tag + "_t")
        nc.vector.tensor_tensor(t[:], vv[:], r0[:], OP.mult)
        nc.vector.tensor_tensor(t[:], t[:], s0[:], OP.add)
        nc.vector.tensor_scalar_mul(t[:], t[:], 0.5)
        rr = utile(psm, [128, 1], F32, tag + "_rr")
        nc.vector.reciprocal(rr[:], t[:])
        return rr

    def layer_norm(src, dst, gb, bb, lnpool):
        m = utile(psm, [128, 1], F32, "ln_m")
        nc.vector.reduce_sum(m[:], src[:, :C], axis=mybir.AxisListType.X)
        nc.vector.tensor_scalar_mul(m[:], m[:], 1.0 / C)
        xc = utile(lnpool, [128, C], F32, "ln_xc", bufs=1)
        nc.vector.tensor_scalar(xc[:], src[:, :C], m[:], None, OP.subtract)
        ss = utile(psm, [128, 1], F32, "ln_ss")
        nc.scalar.activation(dst[:, :C], xc[:], AF.Square, accum_out=ss[:])
        v = utile(psm, [128, 1], F32, "ln_v")
        nc.vector.tensor_scalar(v[:], ss[:], 1.0 / C, 1e-5, OP.mult, OP.add)
        rstd = refined_rsqrt_recip(v, "ln")
        nc.vector.tensor_scalar(dst[:, :C], xc[:], rstd[:], None, OP.mult)
        nc.vector.tensor_tensor(dst[:, :C], dst[:, :C], gb[:], OP.mult)
        nc.vector.tensor_tensor(dst[:, :C], dst[:, :C], bb[:], OP.add)

    # ================= Stage A: LN1 -> hT =================
    pbA = tc.alloc_tile_pool(name="biasA", bufs=1)
    pHT = tc.alloc_tile_pool(name="pHT", bufs=1)

    g1b = brow("g1", pbA)
    be1b = brow("be1", pbA)
    hT = [utile(pHT, [128, N], F32, f"hT{k}") for k in range(8)]
    ht = []
    for i in range(8):
        xt = utile(pt, [128, C], F32, "xin")
        nc.sync.dma_start(xt[:], x_d[ts(i, 128), :])
        h = utile(pHT, [128, C], F32, "ht", bufs=2)
        layer_norm(xt, h, g1b, be1b, pbA)
        ht.append(h)
    transpose_blocks(ht, hT, N, C)

    def split16(dst_hi, dst_lo, src_ap):
        nc.scalar.activation(dst_hi, src_ap, AF.Identity)
        with nc.allow_low_precision(reason="fp16 split residual"):
            nc.vector.tensor_tensor(dst_lo, src_ap, dst_hi, OP.subtract)

    def split16s(dst_hi, dst_lo, src_ap):
        # scale by 64 so the lo half stays in fp16 normal range (w ~ 0.02)
        nc.scalar.activation(dst_hi, src_ap, AF.Identity, scale=64.0)
        with nc.allow_low_precision(reason="fp16 split residual"):
            nc.vector.scalar_tensor_tensor(dst_lo, src_ap, 64.0, dst_hi,
                                           OP.mult, OP.subtract)

    # ===== qk^T -> qkTd (DRAM) ; v_pad (SBUF) =====
    pQK = tc.alloc_tile_pool(name="pQK", bufs=1)
    bqkT = bcol("bqkv", pbA)
    hThi = [utile(pQK, [128, N], FP16, f"hThi{k}") for k in range(8)]
    hTlo = [utile(pQK, [128, N], FP16, f"hTlo{k}") for k in range(8)]
    for k in range(8):
        split16(hThi[k][:], hTlo[k][:], hT[k][:])
    for mpg in range(2):
        wqh, wql, wkh, wkl = [], [], [], []
        for k in range(KT):
            wqf = utile(pw, [128, 512], F32, "wqk512", bufs=3)
            nc.sync.dma_start(wqf[:], wd["Wqkv"][ts(k, 128), ts(mpg, 512)])
            hi = utile(pQK, [128, 512], FP16, "wqh", bufs=8)
            lo = utile(pQK, [128, 512], FP16, "wql", bufs=8)
            split16(hi[:], lo[:], wqf[:])
            wqh.append(hi)
            wql.append(lo)
            wkf = utile(pw, [128, 512], F32, "wqk512", bufs=3)
            nc.sync.dma_start(wkf[:],
                              wd["Wqkv"][ts(k, 128), C + mpg * 512:C + mpg * 512 + 512])
            hi = utile(pQK, [128, 512], FP16, "wkh", bufs=8)
            lo = utile(pQK, [128, 512], FP16, "wkl", bufs=8)
            split16(hi[:], lo[:], wkf[:])
            wkh.append(hi)
            wkl.append(lo)
        for mi in range(4):
            mp = mpg * 4 + mi
            accq = ps_bigA()
            acck = ps_bigB()
            for k in range(KT):
                for n2 in range(2):
                    for wgt, mov in ((wqh[k], hThi[k]), (wqh[k], hTlo[k]),
                                     (wql[k], hThi[k])):
                        nc.tensor.matmul(accq[:, ts(n2, 512)],
                                         wgt[:, ts(mi, 128)],
                                         mov[:, ts(n2, 512)],
                                         start=(k == 0 and wgt is wqh[k]
                                                and mov is hThi[k]),
                                         stop=(k == KT - 1 and wgt is wql[k]))
                    for wgt, mov in ((wkh[k], hThi[k]), (wkh[k], hTlo[k]),
                                     (wkl[k], hThi[k])):
                        nc.tensor.matmul(acck[:, ts(n2, 512)],
                                         wgt[:, ts(mi, 128)],
                                         mov[:, ts(n2, 512)],
                                         start=(k == 0 and wgt is wkh[k]
                                                and mov is hThi[k]),
                                         stop=(k == KT - 1 and wgt is wkl[k]))
            stgq = utile(pQK, [128, N], F32, "qkstg", bufs=2)
            nc.scalar.activation(stgq[:], accq[:], AF.Identity,
                                 bias=bqkT[:, mp:mp + 1])
            nc.sync.dma_start(qkTd[ts(mp, 128), :], stgq[:])
            stgk = utile(pQK, [128, N], F32, "qkstg", bufs=2)
            nc.scalar.activation(stgk[:], acck[:], AF.Identity,
                                 bias=bqkT[:, 8 + mp:9 + mp])
            nc.sync.dma_start(qkTd[C + mp * 128:C + (mp + 1) * 128, :], stgk[:])

    pQK.release()
    pVP = tc.alloc_tile_pool(name="pVP", bufs=1)
    bvqkvb = bcast_row(load_row(wd["bqkv"][2 * C:], C, "bvq_r", pbA), C,
                       "bvq_b", pbA)
    v_pad = [utile(pVP, [128, H, DH + 1], F32, f"vp{j}") for j in range(8)]
    for j in range(8):
        nc.vector.memset(v_pad[j][:, :, DH:DH + 1], 1.0)
    for jg in range(4):
        accs = [ps_bigA(), ps_bigB()]
        for k in range(KT):
            wv = utile(pVP, [128, C], F32, "wv", bufs=2)
            nc.sync.dma_start(wv[:], wd["Wqkv"][ts(k, 128), 2 * C:])
            for jj in range(2):
                j = jg * 2 + jj
                for n2 in range(2):
                    nc.tensor.matmul(accs[jj][:, ts(n2, 512)],
                                     hT[k][:, ts(j, 128)], wv[:, ts(n2, 512)],
                                     start=(k == 0), stop=(k == KT - 1))
        for jj in range(2):
            j = jg * 2 + jj
            for h in range(H):
                nc.vector.tensor_tensor(v_pad[j][:, h, :DH],
                                        accs[jj][:, ts(h, DH)],
                                        bvqkvb[:, ts(h, DH)], OP.add)

    # ===== attention: stream kT/qT per head; out -> aoTd (already c-major) ==
    # out[dh|sum, i] = v_pad[j].T @ expT[j, i], accumulated over j-tiles.
    pAttn = tc.alloc_tile_pool(name="pAttn", bufs=1)
    for h in range(H):
        kth = utile(pAttn, [64, N], F32, "kth", bufs=2)
        nc.sync.dma_start(kth[:], qkTd[C + h * 64:C + h * 64 + 64, :])
        qth = utile(pAttn, [64, N], F32, "qth", bufs=2)
        nc.sync.dma_start(qth[:], qkTd[h * 64:h * 64 + 64, :])
        av = [ps_av().rearrange("p a b -> p (a b)") for _ in range(2)]
        for j in range(8):
            for n2 in range(2):
                sp = ps_mm()
                nc.tensor.matmul(sp[:], kth[:, ts(j, 128)], qth[:, ts(n2, 512)],
                                 start=True, stop=True)
                et = utile(pAttn, [128, 512], F32, "exp", bufs=3)
                nc.scalar.activation(et[:], sp[:], AF.Exp, scale=float(DH ** -0.5))
                nc.tensor.matmul(av[n2][:DH + 1, :512], v_pad[j][:, h, :], et[:],
                                 start=(j == 0), stop=(j == 7))
        for n2 in range(2):
            rrow = utile(pAttn, [1, 512], F32, "rrow", bufs=2)
            nc.vector.reciprocal(rrow[:], av[n2][DH:DH + 1, :512])
            rb = ps_mm()
            nc.tensor.matmul(rb[:DH, :512], ones_col[:, :DH], rrow[:],
                             start=True, stop=True)
            rbs = utile(pAttn, [64, 512], F32, "rbs", bufs=2)
            nc.vector.tensor_copy(rbs[:], rb[:DH, :512])
            stg = utile(pAttn, [64, 512], F32, "aot_stg", bufs=2)
            nc.vector.tensor_tensor(stg[:], av[n2][:DH, :512], rbs[:],
                                    OP.mult)
            nc.sync.dma_start(aoTd[h * 64:h * 64 + 64, ts(n2, 512)], stg[:])
    pAttn.release()
    pVP.release()
    pHT.release()
    pbA.release()

    # ================= Wo -> x_attn, x1 (-> DRAM), metric =================
    pB2 = tc.alloc_tile_pool(name="pB2", bufs=1)
    bob = brow("bo", pB2)
    xa = [utile(pB2, [128, C], F32, f"xa{m}") for m in range(8)]
    woR = [utile(pB2, [128, C], F32, f"woR{k}") for k in range(8)]
    for k in range(KT):
        nc.sync.dma_start(woR[k][:], wd["Wo"][ts(k, 128), :])
    rn = psm.tile([128, 8], F32)
    for m in range(8):
        acc = ps_bigA()
        for k in range(KT):
            ao = utile(pw, [128, 128], F32, "wqkb", bufs=4)
            nc.sync.dma_start(ao[:], aoTd[ts(k, 128), ts(m, 128)])
            for n2 in range(2):
                nc.tensor.matmul(acc[:, ts(n2, 512)], ao[:],
                                 woR[k][:, ts(n2, 512)],
                                 start=(k == 0), stop=(k == KT - 1))
        nc.vector.tensor_tensor(xa[m][:], acc[:], bob[:], OP.add)
        xt = utile(pt, [128, C], F32, "xin")
        nc.sync.dma_start(xt[:], x_d[ts(m, 128), :])
        x1stg = utile(pw, [128, C], F32, "x1stg", bufs=1)
        nc.vector.tensor_tensor(x1stg[:], xa[m][:], xt[:], OP.add)
        nc.sync.dma_start(x1d[ts(m, 128), :], x1stg[:])
        ss = utile(psm, [128, 1], F32, "nrm_ss")
        sq = utile(pt, [128, C], F32, "ln_xc")
        nc.scalar.activation(sq[:], xa[m][:], AF.Square, accum_out=ss[:])
        rr = refined_rsqrt_recip(ss, "nrm")
        nc.vector.tensor_copy(rn[:, m:m + 1], rr[:])
        nc.vector.tensor_scalar(xa[m][:], xa[m][:], rn[:, m:m + 1], None, OP.mult)
        if dbg:
            nc.sync.dma_start(dbg["dbg_xattn"][ts(m, 128), :], xa[m][:])

    # ===== de-interleave metric -> maT/mbT; scores; node stats; ranks =====
    pB3 = tc.alloc_tile_pool(name="pB3", bufs=1)
    xae = [utile(pB3, [128, C], F32, f"xae{m}") for m in range(4)]
    xao = [utile(pB3, [128, C], F32, f"xao{m}") for m in range(4)]
    for m in range(4):
        nc.sync.dma_start(xae[m][:64, :], xa[2 * m][0:128:2, :])
        nc.sync.dma_start(xae[m][64:, :], xa[2 * m + 1][0:128:2, :])
        nc.sync.dma_start(xao[m][:64, :], xa[2 * m][1:128:2, :])
        nc.sync.dma_start(xao[m][64:, :], xa[2 * m + 1][1:128:2, :])
    pB4 = tc.alloc_tile_pool(name="pB4", bufs=1)
    maT = [utile(pB4, [128, NE], F32, f"maT{k}") for k in range(8)]
    mbT = [utile(pB4, [128, NE], F32, f"mbT{k}") for k in range(8)]
    transpose_blocks(xae, maT, NE, C)
    transpose_blocks(xao, mbT, NE, C)

    nm_t = psm.tile([128, 4], F32)
    ni_t = psm.tile([128, 4], F32)
    for m in range(4):
        acc = ps_bigA() if m % 2 == 0 else ps_bigB()
        for k in range(KT):
            nc.tensor.matmul(acc[:, :512], maT[k][:, ts(m, 128)], mbT[k][:],
                             start=(k == 0), stop=(k == KT - 1))
        mx8 = utile(psm, [128, 8], F32, "mx8")
        ix8 = utile(psm, [128, 8], U32, "ix8")
        nc.vector.max_with_indices(mx8[:], ix8[:], acc[:, :512])
        nc.vector.tensor_copy(nm_t[:, m:m + 1], mx8[:, 0:1])
        nc.vector.tensor_copy(ni_t[:, m:m + 1], ix8[:, 0:1])

    nm_row = utile(pB4, [1, 512], F32, "nm_row")
    for m in range(4):
        p = ps_av()
        pf = p.rearrange("p a b -> p (a b)")
        nc.tensor.transpose(pf[:1, :128], nm_t[:, m:m + 1], ident[:])
        nc.vector.tensor_copy(nm_row[:, ts(m, 128)], pf[:1, :128])
    NMB = bcast_row(nm_row[:], 512, "nmb", pB4)

    rank_t = psm.tile([128, 4], F32)
    for m in range(4):
        gt = utile(pB4, [128, 512], F32, "rk_gt", bufs=1)
        nc.vector.tensor_scalar(gt[:], NMB[:], nm_t[:, m:m + 1], None, OP.is_gt)
        eq = utile(pB4, [128, 512], F32, "rk_eq", bufs=1)
        nc.vector.tensor_scalar(eq[:], NMB[:], nm_t[:, m:m + 1], None, OP.is_equal)
        flt = utile(pB4, [128, 512], F32, "rk_flt", bufs=1)
        pio = utile(psm, [128, 1], F32, "rk_pio")
        nc.vector.tensor_scalar_add(pio[:], piota[:], float(128 * m))
        nc.vector.tensor_scalar(flt[:], IOTA512B[:], pio[:], None, OP.is_lt)
        nc.vector.tensor_tensor(eq[:], eq[:], flt[:], OP.mult)
        nc.vector.tensor_tensor(gt[:], gt[:], eq[:], OP.add)
        nc.vector.reduce_sum(rank_t[:, m:m + 1], gt[:], axis=mybir.AxisListType.X)
    if dbg:
        for (tt, nme) in [(nm_t, "dbg_nm"), (rank_t, "dbg_rank"),
                          (ni_t, "dbg_nodeidx")]:
            nc.sync.dma_start(dbg[nme].rearrange("(m p) -> p m", p=128), tt[:])
    pB4.release()
    pB3.release()
    pB2.release()

    # ================= dst merge (x1 from DRAM; dstn -> DRAM) =============
    pM = tc.alloc_tile_pool(name="pM", bufs=1)
    x1e = [utile(pM, [128, C + 8], F32R, f"x1e{m}") for m in range(4)]
    x1o = [utile(pM, [128, C], F32, f"x1o{m}") for m in range(4)]
    for m in range(4):
        nc.vector.tensor_copy(x1e[m][:, C:C + 8], ones_part[:])
        nc.sync.dma_start(x1e[m][:, :C],
                          x1d[256 * m:256 * m + 256:2, :].bitcast(F32R))
        nc.sync.dma_start(x1o[m][:], x1d[256 * m + 1:256 * m + 256:2, :])
    st = [utile(pM, [128, 512], F32R, f"st{m}") for m in range(4)]
    for m in range(4):
        msk = utile(psm, [128, 1], F32, "st_m")
        nc.vector.tensor_scalar(msk[:], rank_t[:, m:m + 1], float(R) - 0.5, None,
                                OP.is_lt)
        nc.vector.tensor_scalar(st[m][:], IOTA512B[:], ni_t[:, m:m + 1], None,
                                OP.is_equal)
        nc.vector.tensor_scalar(st[m][:], st[m][:], msk[:], None, OP.mult)
    for m in range(4):
        acc = ps_bigA()
        cacc = ps_av()
        for k in range(4):
            for n2 in range(2):
                nc.tensor.matmul(acc[:, ts(n2, 512)], st[k][:, ts(m, 128)],
                                 x1e[k][:, n2 * 512:n2 * 512 + 512],
                                 start=(k == 0), stop=(k == 3))
            nc.tensor.matmul(cacc[:, 0, :8], st[k][:, ts(m, 128)],
                             x1e[k][:, C:C + 8], start=(k == 0), stop=(k == 3))
        cnt = utile(psm, [128, 1], F32, "cnt")
        nc.vector.tensor_scalar_add(cnt[:], cacc[:, 0, 0:1], 1.0)
        rec = utile(psm, [128, 1], F32, "cntr")
        nc.vector.reciprocal(rec[:], cnt[:])
        dst_stg = utile(pM, [128, C], F32, "dst_stg", bufs=2)
        nc.vector.tensor_tensor(dst_stg[:], acc[:], x1o[m][:], OP.add)
        nc.vector.tensor_scalar(dst_stg[:], dst_stg[:], rec[:], None, OP.mult)
        nc.sync.dma_start(dstnd[ts(m, 128), :], dst_stg[:])

    # ========== MLP (f32r): W1/W2 streamed once; SBUF out accumulation ======
    def row_src_ap(i):
        if i < 4:
            return x1d[256 * i:256 * i + 256:2, :]
        return dstnd[ts(i - 4, 128), :]

    pM.release()
    pZ = tc.alloc_tile_pool(name="pZ", bufs=1)
    pC4 = tc.alloc_tile_pool(name="pC4", bufs=1)
    g2b = brow("g2", pC4)
    be2b = brow("be2", pC4)
    h2 = []
    for i in range(8):
        rsrc = utile(pt, [128, C], F32, "xin")
        nc.sync.dma_start(rsrc[:], row_src_ap(i))
        h = utile(pC4, [128, C], F32, "ht", bufs=2)
        layer_norm(rsrc, h, g2b, be2b, pC4)
        h2.append(h)
        if dbg:
            nc.sync.dma_start(dbg["dbg_mlpin"][ts(i, 128), :], rsrc[:])
    h2T = [utile(pC4, [128, N], F32R, f"h2T{k}") for k in range(8)]
    transpose_blocks(h2, h2T, N, C)

    bm1T = bcol("bm1", pC4)
    bm2T = bcol("bm2", psm)
    # ---- MLP c-major: Y^T groups of 8 hidden tiles, Z^T accum in SBUF ----
    zacc = [utile(pZ, [128, N], F32R, f"zacc{c}") for c in range(8)]
    for g in range(4):
        ytiles = []
        w1h = None
        for mi in range(8):
            mt = g * 8 + mi
            if mi % 4 == 0:
                w1h = []
                for k in range(KT):
                    w = utile(pC4, [128, 512], F32R, "w1h", bufs=8)
                    nc.sync.dma_start(
                        w[:], wd["W1"][ts(k, 128), ts(2 * g + mi // 4, 512)]
                        .bitcast(F32R))
                    w1h.append(w)
            yp = ps_bigA() if mi % 2 == 0 else ps_bigB()
            for k in range(KT):
                for n2 in range(2):
                    nc.tensor.matmul(yp[:, ts(n2, 512)],
                                     w1h[k][:, ts(mi % 4, 128)],
                                     h2T[k][:, ts(n2, 512)],
                                     start=(k == 0), stop=(k == KT - 1))
            yt = utile(pC4, [128, N], F32R, "yt", bufs=8)
            nc.scalar.activation(yt[:], yp[:], AF.Gelu_apprx_tanh,
                                 bias=bm1T[:, mt:mt + 1])
            ytiles.append(yt)
        for ch in range(2):
            w2h = []
            for mi in range(8):
                mt = g * 8 + mi
                w = utile(pC4, [128, 512], F32R, "w2h", bufs=8)
                nc.sync.dma_start(w[:],
                                  wd["W2"][ts(mt, 128), ts(ch, 512)].bitcast(F32R))
                w2h.append(w)
            for cc in range(4):
                c = ch * 4 + cc
                for n2 in range(2):
                    zp = ps_mm() if n2 == 0 else                         ps_av().rearrange("p a b -> p (a b)")
                    for mi in range(8):
                        nc.tensor.matmul(zp[:, :512],
                                         w2h[mi][:, ts(cc, 128)],
                                         ytiles[mi][:, ts(n2, 512)],
                                         start=(mi == 0), stop=(mi == 7))
                    with nc.allow_low_precision(reason="mlp psums f32r"):
                        if g == 0:
                            nc.vector.tensor_scalar(zacc[c][:, ts(n2, 512)],
                                                    zp[:, :512],
                                                    bm2T[:, c:c + 1], None,
                                                    OP.add)
                        else:
                            nc.vector.tensor_tensor(zacc[c][:, ts(n2, 512)],
                                                    zacc[c][:, ts(n2, 512)],
                                                    zp[:, :512], OP.add)

    # ---- residual + transpose -> row-major mod (bf16, pZ-resident) ----
    mod_sb = [utile(pZ, [128, N], BF16, f"modr{i}") for i in range(8)]
    for i in range(8):
        resf = utile(pt, [128, C], F32, "xin")
        nc.sync.dma_start(resf[:], row_src_ap(i))
        zt = ps_bigA() if i % 2 == 0 else ps_bigB()
        for c in range(8):
            nc.tensor.transpose(zt[:, ts(c, 128)].bitcast(F32R),
                                zacc[c][:, ts(i, 128)], ident_r[:, :])
        with nc.allow_low_precision(reason="mod rows stored bf16"):
            nc.vector.tensor_tensor(mod_sb[i][:], zt[:], resf[:], OP.add)
    if dbg:
        for i in range(8):
            mff = utile(pC4, [128, C], F32, "mof", bufs=2)
            nc.vector.tensor_copy(mff[:], mod_sb[i][:])
            nc.sync.dma_start(dbg["dbg_mlpout"][ts(i, 128), :], mff[:])
    pC4.release()

    # ================= Stage D: pooling + Wp -> combined^T =================
    pD = tc.alloc_tile_pool(name="pD", bufs=1)
    # ApT[p, f] = 0.5 iff source row p pools into output f:
    #   even block: base = rank[p]-16, match iff (2f - base) in {-1, 0}
    #   dst  block: base = d,          match iff (2(f-248) - base) in {-1, 0}
    iota2e = utile(pD, [128, 504], F32, "iota2e")
    nc.vector.tensor_scalar_mul(iota2e[:], IOTA504B[:], 2.0)
    apT = [utile(pD, [128, 504], BF16, f"apT{m}") for m in range(8)]
    for m in range(8):
        base = utile(psm, [128, 1], F32, "ap_r")
        if m < 4:
            nc.vector.tensor_scalar_add(base[:], rank_t[:, m:m + 1], -float(R))
        else:
            nc.vector.tensor_scalar_add(base[:], piota[:],
                                        float(128 * (m - 4) + NE - R))
        d1 = utile(pD, [128, 504], F32, "ap_d1")
        nc.vector.tensor_scalar(d1[:], iota2e[:], base[:], None, OP.subtract)
        a1 = utile(pD, [128, 504], F32, "ap_a1")
        nc.vector.tensor_scalar(a1[:], d1[:], -1.5, None, OP.is_ge)
        b1 = utile(pD, [128, 504], F32, "ap_b1")
        nc.vector.tensor_scalar(b1[:], d1[:], 0.5, None, OP.is_le)
        nc.vector.scalar_tensor_tensor(apT[m][:], a1[:], 0.5, b1[:],
                                       OP.mult, OP.mult)
    # pooledT[c][128c, 504] = sum_i mod_sb[i][:, c-tile]^T @ apT[i]
    pooledT = [utile(pD, [128, NP], F32R, f"pooledT{k}") for k in range(8)]
    for c in range(8):
        zp = ps_mm() if c % 2 == 0 else         ps_av().rearrange("p a b -> p (a b)")
        for i in range(8):
            nc.tensor.matmul(zp[:, :NP], mod_sb[i][:, ts(c, 128)], apT[i][:],
                             start=(i == 0), stop=(i == 7))
        nc.vector.tensor_copy(pooledT[c][:], zp[:, :NP])
    if dbg:
        for m in range(4):
            pst = utile(pt, [128, C], F32, "xin")
            for bj in range(8):
                p = ps_av()
                pf = p.rearrange("p a b -> p (a b)")
                nc.tensor.transpose(pf[:PP, :128].bitcast(F32R),
                                    pooledT[bj][:, m * PP:(m + 1) * PP],
                                    ident_r[:, :])
                nc.vector.tensor_copy(pst[:PP, ts(bj, 128)], pf[:PP, :128])
            nc.sync.dma_start(dbg["dbg_pooled"][ts(m, PP), :], pst[:PP, :])

    pE = tc.alloc_tile_pool(name="pE", bufs=1)
    bp3T = bcol("bp", pD, scale=3.0)
    cmbTr = [utile(pD, [128, NP], F32R, f"cmbTr{m}") for m in range(8)]
    for mg in range(2):
        wcs = []
        for k in range(KT):
            wcr = utile(pD, [128, 512], F32R, f"wpc{k}", bufs=1)
            nc.sync.dma_start(wcr[:], wd["Wp"][ts(k, 128), ts(mg, 512)].bitcast(F32R))
            wcs.append(wcr)
        for mi in range(4):
            m = mg * 4 + mi
            acc = ps_mm()
            for k in range(KT):
                nc.tensor.matmul(acc[:, :NP], wcs[k][:, ts(mi, 128)],
                                 pooledT[k][:], start=(k == 0), stop=(k == KT - 1))
            nc.scalar.activation(cmbTr[m][:], acc[:, :NP], AF.Identity,
                                 bias=bp3T[:, m:m + 1], scale=3.0)

    # ================= Stage E: MQA =================
    bqT = bcol("bq", pE)

    _wq512 = {}

    def make_mqT(m):
        mg = m // 4
        if mg not in _wq512:
            ws = []
            for k in range(KT):
                w = utile(pE, [128, 512], F32R, "wq512", bufs=8)
                nc.sync.dma_start(
                    w[:], wd["Wq"][ts(k, 128), ts(mg, 512)].bitcast(F32R))
                ws.append(w)
            _wq512.clear()
            _wq512[mg] = ws
        ws = _wq512[mg]
        acc = ps_mm()
        for k in range(KT):
            nc.tensor.matmul(acc[:, :NP], ws[k][:, ts(m % 4, 128)], cmbTr[k][:],
                             start=(k == 0), stop=(k == KT - 1))
        t = utile(pE, [128, NP], F32R, "mqT", bufs=2)
        with nc.allow_low_precision(reason="bias add, f32r storage"):
            nc.vector.tensor_scalar(t[:], acc[:, :NP], bqT[:, m:m + 1], None,
                                    OP.add)
        return t

    wkvr = utile(pE, [128, KT, 2 * DH], F32R, "wkvr")
    for k in range(KT):
        nc.sync.dma_start(wkvr[:, k, :DH], wd["Wk"][ts(k, 128), :].bitcast(F32R))
        nc.sync.dma_start(wkvr[:, k, DH:], wd["Wv"][ts(k, 128), :].bitcast(F32R))
    bkT = utile(pE, [64, 1], F32, "bkT")
    nc.sync.dma_start(bkT[:], wd["bk"][:, None])
    mkT = utile(pE, [128, NP], F32R, "mkT")
    macc = ps_mm()
    for k in range(KT):
        nc.tensor.matmul(macc[:64, :NP], wkvr[:, k, :DH], cmbTr[k][:],
                         start=(k == 0), stop=(k == KT - 1))
    mkf = utile(pE, [64, NP], F32, "mkf")
    nc.scalar.activation(mkf[:], macc[:64, :NP], AF.Identity, bias=bkT[:])
    nc.vector.tensor_copy(mkT[:64, :], mkf[:])
    nc.sync.dma_start(mkT[64:, :], mkT[:64, :])

    bvb = bcast_row(load_row(wd["bv"], DH, "bv_r", pE), DH, "bv_b", pE)
    mv_pad = [utile(pE, [128, DH + 1], F32R, f"mvp{m}") for m in range(4)]
    for m in range(4):
        acc = ps_av()
        for k in range(KT):
            nc.tensor.matmul(acc[:PP, 0, :DH], cmbTr[k][:, m * PP:(m + 1) * PP],
                             wkvr[:, k, DH:], start=(k == 0), stop=(k == KT - 1))
        nc.vector.tensor_copy(mv_pad[m][:, DH:], ones_part[:, :1])
        nc.vector.tensor_tensor(mv_pad[m][:PP, :DH], acc[:PP, 0, :DH], bvb[:PP, :],
                                OP.add)

    mqT_cur = None
    for h in range(H):
        po = (h % 2) * 64
        if h % 2 == 0:
            mqT_cur = make_mqT(h // 2)
        mqT_h = mqT_cur[po:po + 64, :]
        spA = ps_bigA()
        spB = ps_bigB()
        ep = []
        for mm in range(4):
            spbig = spA if mm < 2 else spB
            sp = spbig[:, 512 * (mm % 2):512 * (mm % 2) + NP]
            nc.tensor.matmul(sp[:PP, :NP], mkT[po:po + 64, mm * PP:(mm + 1) * PP],
                             mqT_h[:], start=True, stop=True)
            et = utile(pE, [128, NP], F32R, "e2", bufs=6)
            nc.scalar.activation(et[:PP, :], sp[:PP, :NP], AF.Exp,
                                 scale=float(DH ** -0.5))
            ep.append(et)
        av2 = ps_av().rearrange("p a b -> p (a b)")
        for mm in range(4):
            nc.tensor.matmul(av2[:DH + 1, :NP], mv_pad[mm][:PP, :],
                             ep[mm][:PP, :], start=(mm == 0), stop=(mm == 3))
        rrow = utile(pE, [1, NP], F32R, "rrow2", bufs=2)
        with nc.allow_low_precision(reason="softmax denom; 11-bit ok downstream"):
            nc.vector.reciprocal(rrow[:], av2[DH:DH + 1, :NP])
        rb = ps_mm()
        nc.tensor.matmul(rb[:DH, :NP], ones_col_r[:, :DH], rrow[:],
                         start=True, stop=True)
        rbs = utile(pE, [64, NP], F32, "rbs2", bufs=2)
        nc.vector.tensor_copy(rbs[:], rb[:DH, :NP])
        stg = utile(pE, [64, NP], F32R, "mqstg", bufs=2)
        nc.vector.tensor_tensor(stg[:], av2[:DH, :NP], rbs[:], OP.mult)
        nc.sync.dma_start(mqaTd[h * 64:h * 64 + 64, :], stg[:])
    pE.release()
    pD.release()
    pZ.release()

    # ================= Stage F: Wmo + FFN =================
    pF = tc.alloc_tile_pool(name="pF", bufs=1)
    pF1 = tc.alloc_tile_pool(name="pF1", bufs=1)
    mqaT = [utile(pF1, [128, NP], F32R, f"mqaT{k}") for k in range(8)]
    for k in range(8):
        nc.sync.dma_start(mqaT[k][:, :NP], mqaTd[ts(k, 128), :])
    bmoT = bcol("bmo", pF1)
    omoT = [utile(pF, [128, NP], F32R, f"omoT{m}") for m in range(8)]
    for mg in range(2):
        wcs = []
        for k in range(KT):
            wcr = utile(pF1, [128, 512], F32R, f"wmc{k}", bufs=1)
            nc.sync.dma_start(wcr[:],
                              wd["Wmo"][ts(k, 128), ts(mg, 512)].bitcast(F32R))
            wcs.append(wcr)
        for mi in range(4):
            m = mg * 4 + mi
            acc = ps_mm()
            for k in range(KT):
                nc.tensor.matmul(acc[:, :NP], wcs[k][:, ts(mi, 128)],
                                 mqaT[k][:], start=(k == 0), stop=(k == KT - 1))
            nc.scalar.activation(omoT[m][:], acc[:, :NP], AF.Identity,
                                 bias=bmoT[:, m:m + 1])
    pF1.release()

    # ---- FFN c-major: Y halves of 16 hidden tiles, Z accum in SBUF ----
    bf1T = bcol("bf1", pF)
    bf2b = brow("bf2", pF)
    zf = [utile(pF, [128, NP], F32R, f"zf{c}") for c in range(8)]
    for hf in range(2):
        yfs = []
        wf1h = None
        for kki in range(16):
            kk = hf * 16 + kki
            if kki % 4 == 0:
                wf1h = []
                for k in range(KT):
                    w = utile(pF, [128, 512], F32R, "wf1h", bufs=9)
                    nc.sync.dma_start(
                        w[:], wd["Wf1"][ts(k, 128), ts(kk // 4, 512)]
                        .bitcast(F32R))
                    wf1h.append(w)
            yp = ps_mm() if kki % 2 == 0 else             ps_av().rearrange("p a b -> p (a b)")
            for k in range(KT):
                nc.tensor.matmul(yp[:, :NP], wf1h[k][:, ts(kki % 4, 128)],
                                 omoT[k][:], start=(k == 0), stop=(k == KT - 1))
            yf = utile(pF, [128, NP], F32R, "yf", bufs=17)
            nc.scalar.activation(yf[:], yp[:, :NP], AF.Silu,
                                 bias=bf1T[:, kk:kk + 1])
            yfs.append(yf)
        for ch in range(2):
            wf2h = []
            for kki in range(16):
                kk = hf * 16 + kki
                w = utile(pF, [128, 512], F32R, "wf2h", bufs=17)
                nc.sync.dma_start(w[:],
                                  wd["Wf2"][ts(kk, 128), ts(ch, 512)].bitcast(F32R))
                wf2h.append(w)
            for cc in range(4):
                c = ch * 4 + cc
                zp = ps_bigA() if cc % 2 == 0 else ps_bigB()
                for kki in range(16):
                    nc.tensor.matmul(zp[:, :NP], wf2h[kki][:, ts(cc, 128)],
                                     yfs[kki][:], start=(kki == 0),
                                     stop=(kki == 15))
                with nc.allow_low_precision(reason="ffn out partials f32r"):
                    if hf == 0:
                        nc.vector.tensor_copy(zf[c][:], zp[:, :NP])
                    else:
                        nc.vector.tensor_tensor(zf[c][:], zf[c][:], zp[:, :NP],
                                                OP.add)
    # ---- transpose back to row-major + bias -> out ----
    for tl in range(4):
        of = utile(pF, [128, C], F32, "of", bufs=2)
        for c in range(8):
            p = ps_av()
            pf = p.rearrange("p a b -> p (a b)")
            nc.tensor.transpose(pf[:PP, :128].bitcast(F32R),
                                zf[c][:, tl * PP:(tl + 1) * PP], ident_r[:, :])
            nc.vector.tensor_tensor(of[:PP, ts(c, 128)], pf[:PP, :128],
                                    bf2b[:PP, ts(c, 128)], OP.add)
        nc.sync.dma_start(out_d[tl * PP:(tl + 1) * PP, :], of[:PP, :])
    pF.release()
    for pool in (pt, pw, psm, pc, pp):
        pool.release()


_BUILT = None


def kernel(**inputs):
    global _BUILT
    if _BUILT is None:
        _BUILT = build(debug=DEBUG)
    nc = _BUILT
    x = np.ascontiguousarray(inputs["x"], dtype=np.float32)
    base = {k: np.ascontiguousarray(v, dtype=np.float32) for k, v in inputs.items()
            if k != "x"}
    in_maps = []
    for i in range(8):
        m = dict(base)
        m["x"] = x[i]
        in_maps.append(m)
    res = run_bass_kernel_spmd(nc, in_maps, core_ids=list(range(8)))
    out = np.stack([res.results[i]["out"] for i in range(8)], axis=0)
    return out.astype(np.float32)



# revision 66
# speedup vs baseline: 1.0465x; 1.0078x over previous
"""AdaptiveTokenMerger (ToMe block + merger) TRN2 Bass kernel.

Data-parallel over batch: 8 samples -> 8 NeuronCores, one sample per core.
Per-core pipeline (sample x [1024, 1024]):
  A (f32, ranking-critical): LN1 -> qkv -> MHA (transposed-softmax with the
    denominator folded in as an appended ones-column of v) -> Wo -> x_attn
  B: metric scores -> node_max/argmax -> ranks via pairwise comparisons ->
    dst scatter-add expressed as a one-hot matmul
  C (f32r): MLP over rows [x1_even(512); dst_new(512)], fused W1/W2 per
    token-quarter, output accumulated in PSUM across all 32 W1 column tiles
  D: pooling as a rank-dependent one-hot matmul -> Wp -> combined = 3q
  E (f32r): multi-query attention  F (f32r): FFN -> out [504, 1024]

Precision: everything upstream of the rank/argmax decisions is true fp32
(4 cyc/row on PE); post-merge matmuls use float32r (TF32-ish, 1 cyc/row).

PSUM budget (8 banks): BIGA/BIGB [128,1024] (2+2), MM [128,512] x2 (2),
AV [128,4,128] x2 (2).
"""
import numpy as np

import concourse.bass as bass
import concourse.tile as tile
from concourse import bacc, mybir
from concourse.bass import ts
from concourse.bass_utils import run_bass_kernel_spmd
from concourse.masks import make_identity

F32 = mybir.dt.float32
F32R = mybir.dt.float32r
FP16 = mybir.dt.float16
BF16 = mybir.dt.bfloat16
U32 = mybir.dt.uint32

N, C, H = 1024, 1024, 16
R = 16
DH = C // H          # 64
NE = N // 2          # 512
NP = (N - R) // 2    # 504
PP = 126             # pooled tokens per partition tile
KT = C // 128        # 8
AF = mybir.ActivationFunctionType
OP = mybir.AluOpType

DEBUG = False


def build(debug=False):
    nc = bacc.Bacc("TRN2", target_bir_lowering=False, debug=False, num_devices=8)
    x_d = nc.dram_tensor("x", [N, C], F32, kind="ExternalInput").ap()
    wd = {}
    for name, shape in [
        ("g1", [C]), ("be1", [C]), ("Wqkv", [C, 3 * C]), ("bqkv", [3 * C]),
        ("Wo", [C, C]), ("bo", [C]), ("g2", [C]), ("be2", [C]),
        ("W1", [C, 4 * C]), ("bm1", [4 * C]), ("W2", [4 * C, C]), ("bm2", [C]),
        ("Wp", [C, C]), ("bp", [C]), ("Wq", [C, C]), ("bq", [C]),
        ("Wk", [C, DH]), ("bk", [DH]), ("Wv", [C, DH]), ("bv", [DH]),
        ("Wmo", [C, C]), ("bmo", [C]), ("Wf1", [C, 4 * C]), ("bf1", [4 * C]),
        ("Wf2", [4 * C, C]), ("bf2", [C]),
    ]:
        wd[name] = nc.dram_tensor(name, shape, F32, kind="ExternalInput").ap()
    out_d = nc.dram_tensor("out", [NP, C], F32, kind="ExternalOutput").ap()
    dbg = {}
    if debug:
        for name, shape in [
            ("dbg_xattn", [N, C]), ("dbg_nm", [NE]), ("dbg_rank", [NE]),
            ("dbg_nodeidx", [NE]), ("dbg_mlpin", [N, C]), ("dbg_mlpout", [N, C]),
            ("dbg_pooled", [NP, C]),
        ]:
            dbg[name] = nc.dram_tensor(name, shape, F32, kind="ExternalOutput").ap()
    with tile.TileContext(nc) as tc:
        _build_tile(nc, tc, x_d, wd, out_d, dbg)
    nc.compile()
    return nc


def _build_tile(nc, tc, x_d, wd, out_d, dbg):
    # DRAM spill buffers
    qkTd = nc.dram_tensor("qkTd", [2 * C, N], F32).ap()
    aoTd = nc.dram_tensor("aoTd", [C, N], F32).ap()
    x1d = nc.dram_tensor("x1d", [N, C], F32).ap()
    dstnd = nc.dram_tensor("dstnd", [NE, C], F32).ap()
    mqaTd = nc.dram_tensor("mqaTd", [C, NP], F32R).ap()

    pc = tc.alloc_tile_pool(name="const", bufs=1)
    psm = tc.alloc_tile_pool(name="small", bufs=1)
    pw = tc.alloc_tile_pool(name="wstream", bufs=2)
    pt = tc.alloc_tile_pool(name="tmp", bufs=2)
    pp = tc.alloc_tile_pool(name="psum", bufs=1, space="PSUM")

    _ct = {}

    def utile(pool, shape, dtype, tag, bufs=None):
        _ct[tag] = _ct.get(tag, 0) + 1
        kw = {"bufs": bufs} if bufs is not None else {}
        return pool.tile(shape, dtype, tag=tag, name=f"{tag}_{_ct[tag]}", **kw)

    def ps_bigA():
        return utile(pp, [128, 1024], F32, "BIGA")

    def ps_bigB():
        return utile(pp, [128, 1024], F32, "BIGB")

    def ps_mm():
        return utile(pp, [128, 512], F32, "MM", bufs=2)

    def ps_av():
        return utile(pp, [128, 4, 128], F32, "AV", bufs=2)

    # ---------- constants ----------
    ident = pc.tile([128, 128], F32)
    make_identity(nc, ident[:])
    ident_r = pc.tile([128, 128], F32R)
    nc.vector.tensor_copy(ident_r[:], ident[:])
    ones_col = pc.tile([1, 128], F32)
    nc.gpsimd.memset(ones_col[:], 1.0)
    ones_col_r = pc.tile([1, 128], F32R)
    nc.vector.tensor_copy(ones_col_r[:], ones_col[:])
    ones_part = pc.tile([128, 8], F32)
    nc.gpsimd.memset(ones_part[:], 1.0)
    piota = pc.tile([128, 1], F32)
    nc.gpsimd.iota(piota[:], [[0, 1]], channel_multiplier=1,
                   allow_small_or_imprecise_dtypes=True)
    iota512_row = pc.tile([1, 512], F32)
    nc.gpsimd.iota(iota512_row[:], [[1, 512]], channel_multiplier=0,
                   allow_small_or_imprecise_dtypes=True)
    iota504_row = pc.tile([1, 504], F32)
    nc.gpsimd.iota(iota504_row[:], [[1, 504]], channel_multiplier=0,
                   allow_small_or_imprecise_dtypes=True)

    def bcast_row(row_ap, n, tag, pool, scale=1.0):
        t = utile(pool, [128, n], F32, tag)
        for c0 in range(0, n, 512):
            cw = min(512, n - c0)
            p = ps_mm()
            nc.tensor.matmul(p[:, :cw], ones_col[:], row_ap[:, c0:c0 + cw],
                             start=True, stop=True)
            if scale == 1.0:
                nc.vector.tensor_copy(t[:, c0:c0 + cw], p[:, :cw])
            else:
                nc.vector.tensor_scalar_mul(t[:, c0:c0 + cw], p[:, :cw], scale)
        return t

    def load_row(dram_ap, n, tag, pool):
        t = utile(pw, [1, n], F32, "rowstg", bufs=1)
        nc.sync.dma_start(t[:], dram_ap[None, :])
        return t

    def brow(name, pool, scale=1.0):
        n = wd[name].shape[0]
        return bcast_row(load_row(wd[name], n, name + "_r", pool), n,
                         name + "_b", pool, scale)

    def bcol(name, pool, scale=1.0):
        n = wd[name].shape[0]
        t = utile(pool, [128, n // 128], F32, name + "_c")
        nc.sync.dma_start(t[:], wd[name].rearrange("(t p) -> p t", p=128))
        if scale != 1.0:
            nc.vector.tensor_scalar_mul(t[:], t[:], scale)
        return t

    IOTA512B = bcast_row(iota512_row[:], 512, "iota512b", pc)
    IOTA504B = bcast_row(iota504_row[:], 504, "iota504b", pc)

    def transpose_blocks(src_tiles, dst, n_rows, n_cols):
        """dst[c, r] = src[r, c]; dst is tile-list or sink(bj, bi, pf, cw, rw)."""
        for bi in range((n_rows + 127) // 128):
            rw = min(128, n_rows - bi * 128)
            for bj in range((n_cols + 127) // 128):
                cw = min(128, n_cols - bj * 128)
                p = ps_av()
                pf = p.rearrange("p a b -> p (a b)")
                nc.tensor.transpose(pf[:cw, :rw],
                                    src_tiles[bi][:rw, bj * 128:bj * 128 + cw],
                                    ident[:rw, :rw])
                if callable(dst):
                    dst(bj, bi, pf, cw, rw)
                else:
                    nc.vector.tensor_copy(dst[bj][:cw, bi * 128:bi * 128 + rw],
                                          pf[:cw, :rw])

    def refined_rsqrt_recip(vv, tag):
        """returns 1/sqrt(vv) with one Newton step on sqrt (ACT sqrt is loose)."""
        s0 = utile(psm, [128, 1], F32, tag + "_s0")
        nc.scalar.sqrt(s0[:], vv[:])
        r0 = utile(psm, [128, 1], F32, tag + "_r0")
        nc.vector.reciprocal(r0[:], s0[:])
        t = utile(psm, [128, 1], F32, tag + "_t")
        nc.vector.tensor_tensor(t[:], vv[:], r0[:], OP.mult)
        nc.vector.tensor_tensor(t[:], t[:], s0[:], OP.add)
        nc.vector.tensor_scalar_mul(t[:], t[:], 0.5)
        rr = utile(psm, [128, 1], F32, tag + "_rr")
        nc.vector.reciprocal(rr[:], t[:])
        return rr

    def layer_norm(src, dst, gb, bb, lnpool):
        m = utile(psm, [128, 1], F32, "ln_m")
        nc.vector.reduce_sum(m[:], src[:, :C], axis=mybir.AxisListType.X)
        nc.vector.tensor_scalar_mul(m[:], m[:], 1.0 / C)
        xc = utile(lnpool, [128, C], F32, "ln_xc", bufs=1)
        nc.vector.tensor_scalar(xc[:], src[:, :C], m[:], None, OP.subtract)
        ss = utile(psm, [128, 1], F32, "ln_ss")
        nc.scalar.activation(dst[:, :C], xc[:], AF.Square, accum_out=ss[:])
        v = utile(psm, [128, 1], F32, "ln_v")
        nc.vector.tensor_scalar(v[:], ss[:], 1.0 / C, 1e-5, OP.mult, OP.add)
        rstd = refined_rsqrt_recip(v, "ln")
        nc.vector.tensor_scalar(dst[:, :C], xc[:], rstd[:], None, OP.mult)
        nc.vector.tensor_tensor(dst[:, :C], dst[:, :C], gb[:], OP.mult)
        nc.vector.tensor_tensor(dst[:, :C], dst[:, :C], bb[:], OP.add)

    # ================= Stage A: LN1 -> hT =================
    pbA = tc.alloc_tile_pool(name="biasA", bufs=1)
    pHT = tc.alloc_tile_pool(name="pHT", bufs=1)

    g1b = brow("g1", pbA)
    be1b = brow("be1", pbA)
    hT = [utile(pHT, [128, N], F32, f"hT{k}") for k in range(8)]
    ht = []
    for i in range(8):
        xt = utile(pt, [128, C], F32, "xin")
        nc.sync.dma_start(xt[:], x_d[ts(i, 128), :])
        h = utile(pHT, [128, C], F32, "ht", bufs=2)
        layer_norm(xt, h, g1b, be1b, pbA)
        ht.append(h)
    transpose_blocks(ht, hT, N, C)

    def split16(dst_hi, dst_lo, src_ap):
        nc.scalar.activation(dst_hi, src_ap, AF.Identity)
        with nc.allow_low_precision(reason="fp16 split residual"):
            nc.vector.tensor_tensor(dst_lo, src_ap, dst_hi, OP.subtract)

    def split16s(dst_hi, dst_lo, src_ap):
        # scale by 64 so the lo half stays in fp16 normal range (w ~ 0.02)
        nc.scalar.activation(dst_hi, src_ap, AF.Identity, scale=64.0)
        with nc.allow_low_precision(reason="fp16 split residual"):
            nc.vector.scalar_tensor_tensor(dst_lo, src_ap, 64.0, dst_hi,
                                           OP.mult, OP.subtract)

    # ===== qk^T -> qkTd (DRAM) ; v_pad (SBUF) =====
    pQK = tc.alloc_tile_pool(name="pQK", bufs=1)
    bqkT = bcol("bqkv", pbA)
    hThi = [utile(pQK, [128, N], FP16, f"hThi{k}") for k in range(8)]
    hTlo = [utile(pQK, [128, N], FP16, f"hTlo{k}") for k in range(8)]
    for k in range(8):
        split16(hThi[k][:], hTlo[k][:], hT[k][:])
    for mpg in range(2):
        wqh, wql, wkh, wkl = [], [], [], []
        for k in range(KT):
            wqf = utile(pw, [128, 512], F32, "wqk512", bufs=3)
            nc.sync.dma_start(wqf[:], wd["Wqkv"][ts(k, 128), ts(mpg, 512)])
            hi = utile(pQK, [128, 512], FP16, "wqh", bufs=8)
            lo = utile(pQK, [128, 512], FP16, "wql", bufs=8)
            split16(hi[:], lo[:], wqf[:])
            wqh.append(hi)
            wql.append(lo)
            wkf = utile(pw, [128, 512], F32, "wqk512", bufs=3)
            nc.sync.dma_start(wkf[:],
                              wd["Wqkv"][ts(k, 128), C + mpg * 512:C + mpg * 512 + 512])
            hi = utile(pQK, [128, 512], FP16, "wkh", bufs=8)
            lo = utile(pQK, [128, 512], FP16, "wkl", bufs=8)
            split16(hi[:], lo[:], wkf[:])
            wkh.append(hi)
            wkl.append(lo)
        for mi in range(4):
            mp = mpg * 4 + mi
            accq = ps_bigA()
            acck = ps_bigB()
            for k in range(KT):
                for n2 in range(2):
                    for wgt, mov in ((wqh[k], hThi[k]), (wqh[k], hTlo[k]),
                                     (wql[k], hThi[k])):
                        nc.tensor.matmul(accq[:, ts(n2, 512)],
                                         wgt[:, ts(mi, 128)],
                                         mov[:, ts(n2, 512)],
                                         start=(k == 0 and wgt is wqh[k]
                                                and mov is hThi[k]),
                                         stop=(k == KT - 1 and wgt is wql[k]))
                    for wgt, mov in ((wkh[k], hThi[k]), (wkh[k], hTlo[k]),
                                     (wkl[k], hThi[k])):
                        nc.tensor.matmul(acck[:, ts(n2, 512)],
                                         wgt[:, ts(mi, 128)],
                                         mov[:, ts(n2, 512)],
                                         start=(k == 0 and wgt is wkh[k]
                                                and mov is hThi[k]),
                                         stop=(k == KT - 1 and wgt is wkl[k]))
            stgq = utile(pQK, [128, N], F32, "qkstg", bufs=2)
            nc.scalar.activation(stgq[:], accq[:], AF.Identity,
                                 bias=bqkT[:, mp:mp + 1])
            nc.sync.dma_start(qkTd[ts(mp, 128), :], stgq[:])
            stgk = utile(pQK, [128, N], F32, "qkstg", bufs=2)
            nc.scalar.activation(stgk[:], acck[:], AF.Identity,
                                 bias=bqkT[:, 8 + mp:9 + mp])
            nc.sync.dma_start(qkTd[C + mp * 128:C + (mp + 1) * 128, :], stgk[:])

    pQK.release()
    pVP = tc.alloc_tile_pool(name="pVP", bufs=1)
    bvqkvb = bcast_row(load_row(wd["bqkv"][2 * C:], C, "bvq_r", pbA), C,
                       "bvq_b", pbA)
    v_pad = [utile(pVP, [128, H, DH + 1], F32, f"vp{j}") for j in range(8)]
    for j in range(8):
        nc.vector.memset(v_pad[j][:, :, DH:DH + 1], 1.0)
    for jg in range(4):
        accs = [ps_bigA(), ps_bigB()]
        for k in range(KT):
            wv = utile(pVP, [128, C], F32, "wv", bufs=2)
            nc.sync.dma_start(wv[:], wd["Wqkv"][ts(k, 128), 2 * C:])
            for jj in range(2):
                j = jg * 2 + jj
                for n2 in range(2):
                    nc.tensor.matmul(accs[jj][:, ts(n2, 512)],
                                     hT[k][:, ts(j, 128)], wv[:, ts(n2, 512)],
                                     start=(k == 0), stop=(k == KT - 1))
        for jj in range(2):
            j = jg * 2 + jj
            for h in range(H):
                nc.vector.tensor_tensor(v_pad[j][:, h, :DH],
                                        accs[jj][:, ts(h, DH)],
                                        bvqkvb[:, ts(h, DH)], OP.add)

    # ===== attention: stream kT/qT per head; out -> aoTd (already c-major) ==
    # out[dh|sum, i] = v_pad[j].T @ expT[j, i], accumulated over j-tiles.
    pAttn = tc.alloc_tile_pool(name="pAttn", bufs=1)
    for h in range(H):
        kth = utile(pAttn, [64, N], F32, "kth", bufs=2)
        nc.sync.dma_start(kth[:], qkTd[C + h * 64:C + h * 64 + 64, :])
        qth = utile(pAttn, [64, N], F32, "qth", bufs=2)
        nc.sync.dma_start(qth[:], qkTd[h * 64:h * 64 + 64, :])
        khi = utile(pAttn, [64, N], FP16, "khi", bufs=2)
        klo = utile(pAttn, [64, N], FP16, "klo", bufs=2)
        split16(khi[:], klo[:], kth[:])
        qhi = utile(pAttn, [64, N], FP16, "qhi", bufs=2)
        qlo = utile(pAttn, [64, N], FP16, "qlo", bufs=2)
        split16(qhi[:], qlo[:], qth[:])
        av = [ps_av().rearrange("p a b -> p (a b)") for _ in range(2)]
        for j in range(8):
            for n2 in range(2):
                sp = ps_mm()
                for sta, mov in ((khi, qhi), (khi, qlo), (klo, qhi)):
                    nc.tensor.matmul(sp[:], sta[:, ts(j, 128)],
                                     mov[:, ts(n2, 512)],
                                     start=(sta is khi and mov is qhi),
                                     stop=(sta is klo))
                et = utile(pAttn, [128, 512], F32, "exp", bufs=3)
                nc.scalar.activation(et[:], sp[:], AF.Exp, scale=float(DH ** -0.5))
                nc.tensor.matmul(av[n2][:DH + 1, :512], v_pad[j][:, h, :], et[:],
                                 start=(j == 0), stop=(j == 7))
        for n2 in range(2):
            rrow = utile(pAttn, [1, 512], F32, "rrow", bufs=2)
            nc.vector.reciprocal(rrow[:], av[n2][DH:DH + 1, :512])
            rb = ps_mm()
            nc.tensor.matmul(rb[:DH, :512], ones_col[:, :DH], rrow[:],
                             start=True, stop=True)
            rbs = utile(pAttn, [64, 512], F32, "rbs", bufs=2)
            nc.vector.tensor_copy(rbs[:], rb[:DH, :512])
            stg = utile(pAttn, [64, 512], F32, "aot_stg", bufs=2)
            nc.vector.tensor_tensor(stg[:], av[n2][:DH, :512], rbs[:],
                                    OP.mult)
            nc.sync.dma_start(aoTd[h * 64:h * 64 + 64, ts(n2, 512)], stg[:])
    pAttn.release()
    pVP.release()
    pHT.release()
    pbA.release()

    # ================= Wo -> x_attn, x1 (-> DRAM), metric =================
    pB2 = tc.alloc_tile_pool(name="pB2", bufs=1)
    bob = brow("bo", pB2)
    xa = [utile(pB2, [128, C], F32, f"xa{m}") for m in range(8)]
    woR = [utile(pB2, [128, C], F32, f"woR{k}") for k in range(8)]
    for k in range(KT):
        nc.sync.dma_start(woR[k][:], wd["Wo"][ts(k, 128), :])
    rn = psm.tile([128, 8], F32)
    for m in range(8):
        acc = ps_bigA()
        for k in range(KT):
            ao = utile(pw, [128, 128], F32, "wqkb", bufs=4)
            nc.sync.dma_start(ao[:], aoTd[ts(k, 128), ts(m, 128)])
            for n2 in range(2):
                nc.tensor.matmul(acc[:, ts(n2, 512)], ao[:],
                                 woR[k][:, ts(n2, 512)],
                                 start=(k == 0), stop=(k == KT - 1))
        nc.vector.tensor_tensor(xa[m][:], acc[:], bob[:], OP.add)
        xt = utile(pt, [128, C], F32, "xin")
        nc.sync.dma_start(xt[:], x_d[ts(m, 128), :])
        x1stg = utile(pw, [128, C], F32, "x1stg", bufs=1)
        nc.vector.tensor_tensor(x1stg[:], xa[m][:], xt[:], OP.add)
        nc.sync.dma_start(x1d[ts(m, 128), :], x1stg[:])
        ss = utile(psm, [128, 1], F32, "nrm_ss")
        sq = utile(pt, [128, C], F32, "ln_xc")
        nc.scalar.activation(sq[:], xa[m][:], AF.Square, accum_out=ss[:])
        rr = refined_rsqrt_recip(ss, "nrm")
        nc.vector.tensor_copy(rn[:, m:m + 1], rr[:])
        nc.vector.tensor_scalar(xa[m][:], xa[m][:], rn[:, m:m + 1], None, OP.mult)
        if dbg:
            nc.sync.dma_start(dbg["dbg_xattn"][ts(m, 128), :], xa[m][:])

    # ===== de-interleave metric -> maT/mbT; scores; node stats; ranks =====
    pB3 = tc.alloc_tile_pool(name="pB3", bufs=1)
    xae = [utile(pB3, [128, C], F32, f"xae{m}") for m in range(4)]
    xao = [utile(pB3, [128, C], F32, f"xao{m}") for m in range(4)]
    for m in range(4):
        nc.sync.dma_start(xae[m][:64, :], xa[2 * m][0:128:2, :])
        nc.sync.dma_start(xae[m][64:, :], xa[2 * m + 1][0:128:2, :])
        nc.sync.dma_start(xao[m][:64, :], xa[2 * m][1:128:2, :])
        nc.sync.dma_start(xao[m][64:, :], xa[2 * m + 1][1:128:2, :])
    pB4 = tc.alloc_tile_pool(name="pB4", bufs=1)
    maT = [utile(pB4, [128, NE], F32, f"maT{k}") for k in range(8)]
    mbT = [utile(pB4, [128, NE], F32, f"mbT{k}") for k in range(8)]
    transpose_blocks(xae, maT, NE, C)
    transpose_blocks(xao, mbT, NE, C)

    nm_t = psm.tile([128, 4], F32)
    ni_t = psm.tile([128, 4], F32)
    for m in range(4):
        acc = ps_bigA() if m % 2 == 0 else ps_bigB()
        for k in range(KT):
            nc.tensor.matmul(acc[:, :512], maT[k][:, ts(m, 128)], mbT[k][:],
                             start=(k == 0), stop=(k == KT - 1))
        mx8 = utile(psm, [128, 8], F32, "mx8")
        ix8 = utile(psm, [128, 8], U32, "ix8")
        nc.vector.max_with_indices(mx8[:], ix8[:], acc[:, :512])
        nc.vector.tensor_copy(nm_t[:, m:m + 1], mx8[:, 0:1])
        nc.vector.tensor_copy(ni_t[:, m:m + 1], ix8[:, 0:1])

    nm_row = utile(pB4, [1, 512], F32, "nm_row")
    for m in range(4):
        p = ps_av()
        pf = p.rearrange("p a b -> p (a b)")
        nc.tensor.transpose(pf[:1, :128], nm_t[:, m:m + 1], ident[:])
        nc.vector.tensor_copy(nm_row[:, ts(m, 128)], pf[:1, :128])
    NMB = bcast_row(nm_row[:], 512, "nmb", pB4)

    rank_t = psm.tile([128, 4], F32)
    for m in range(4):
        gt = utile(pB4, [128, 512], F32, "rk_gt", bufs=1)
        nc.vector.tensor_scalar(gt[:], NMB[:], nm_t[:, m:m + 1], None, OP.is_gt)
        eq = utile(pB4, [128, 512], F32, "rk_eq", bufs=1)
        nc.vector.tensor_scalar(eq[:], NMB[:], nm_t[:, m:m + 1], None, OP.is_equal)
        flt = utile(pB4, [128, 512], F32, "rk_flt", bufs=1)
        pio = utile(psm, [128, 1], F32, "rk_pio")
        nc.vector.tensor_scalar_add(pio[:], piota[:], float(128 * m))
        nc.vector.tensor_scalar(flt[:], IOTA512B[:], pio[:], None, OP.is_lt)
        nc.vector.tensor_tensor(eq[:], eq[:], flt[:], OP.mult)
        nc.vector.tensor_tensor(gt[:], gt[:], eq[:], OP.add)
        nc.vector.reduce_sum(rank_t[:, m:m + 1], gt[:], axis=mybir.AxisListType.X)
    if dbg:
        for (tt, nme) in [(nm_t, "dbg_nm"), (rank_t, "dbg_rank"),
                          (ni_t, "dbg_nodeidx")]:
            nc.sync.dma_start(dbg[nme].rearrange("(m p) -> p m", p=128), tt[:])
    pB4.release()
    pB3.release()
    pB2.release()

    # ================= dst merge (x1 from DRAM; dstn -> DRAM) =============
    pM = tc.alloc_tile_pool(name="pM", bufs=1)
    x1e = [utile(pM, [128, C + 8], F32R, f"x1e{m}") for m in range(4)]
    x1o = [utile(pM, [128, C], F32, f"x1o{m}") for m in range(4)]
    for m in range(4):
        nc.vector.tensor_copy(x1e[m][:, C:C + 8], ones_part[:])
        nc.sync.dma_start(x1e[m][:, :C],
                          x1d[256 * m:256 * m + 256:2, :].bitcast(F32R))
        nc.sync.dma_start(x1o[m][:], x1d[256 * m + 1:256 * m + 256:2, :])
    st = [utile(pM, [128, 512], F32R, f"st{m}") for m in range(4)]
    for m in range(4):
        msk = utile(psm, [128, 1], F32, "st_m")
        nc.vector.tensor_scalar(msk[:], rank_t[:, m:m + 1], float(R) - 0.5, None,
                                OP.is_lt)
        nc.vector.tensor_scalar(st[m][:], IOTA512B[:], ni_t[:, m:m + 1], None,
                                OP.is_equal)
        nc.vector.tensor_scalar(st[m][:], st[m][:], msk[:], None, OP.mult)
    for m in range(4):
        acc = ps_bigA()
        cacc = ps_av()
        for k in range(4):
            for n2 in range(2):
                nc.tensor.matmul(acc[:, ts(n2, 512)], st[k][:, ts(m, 128)],
                                 x1e[k][:, n2 * 512:n2 * 512 + 512],
                                 start=(k == 0), stop=(k == 3))
            nc.tensor.matmul(cacc[:, 0, :8], st[k][:, ts(m, 128)],
                             x1e[k][:, C:C + 8], start=(k == 0), stop=(k == 3))
        cnt = utile(psm, [128, 1], F32, "cnt")
        nc.vector.tensor_scalar_add(cnt[:], cacc[:, 0, 0:1], 1.0)
        rec = utile(psm, [128, 1], F32, "cntr")
        nc.vector.reciprocal(rec[:], cnt[:])
        dst_stg = utile(pM, [128, C], F32, "dst_stg", bufs=2)
        nc.vector.tensor_tensor(dst_stg[:], acc[:], x1o[m][:], OP.add)
        nc.vector.tensor_scalar(dst_stg[:], dst_stg[:], rec[:], None, OP.mult)
        nc.sync.dma_start(dstnd[ts(m, 128), :], dst_stg[:])

    # ========== MLP (f32r): W1/W2 streamed once; SBUF out accumulation ======
    def row_src_ap(i):
        if i < 4:
            return x1d[256 * i:256 * i + 256:2, :]
        return dstnd[ts(i - 4, 128), :]

    pM.release()
    pZ = tc.alloc_tile_pool(name="pZ", bufs=1)
    pC4 = tc.alloc_tile_pool(name="pC4", bufs=1)
    g2b = brow("g2", pC4)
    be2b = brow("be2", pC4)
    h2 = []
    for i in range(8):
        rsrc = utile(pt, [128, C], F32, "xin")
        nc.sync.dma_start(rsrc[:], row_src_ap(i))
        h = utile(pC4, [128, C], F32, "ht", bufs=2)
        layer_norm(rsrc, h, g2b, be2b, pC4)
        h2.append(h)
        if dbg:
            nc.sync.dma_start(dbg["dbg_mlpin"][ts(i, 128), :], rsrc[:])
    h2T = [utile(pC4, [128, N], F32R, f"h2T{k}") for k in range(8)]
    transpose_blocks(h2, h2T, N, C)

    bm1T = bcol("bm1", pC4)
    bm2T = bcol("bm2", psm)
    # ---- MLP c-major: Y^T groups of 8 hidden tiles, Z^T accum in SBUF ----
    zacc = [utile(pZ, [128, N], F32R, f"zacc{c}") for c in range(8)]
    for g in range(4):
        ytiles = []
        w1h = None
        for mi in range(8):
            mt = g * 8 + mi
            if mi % 4 == 0:
                w1h = []
                for k in range(KT):
                    w = utile(pC4, [128, 512], F32R, "w1h", bufs=8)
                    nc.sync.dma_start(
                        w[:], wd["W1"][ts(k, 128), ts(2 * g + mi // 4, 512)]
                        .bitcast(F32R))
                    w1h.append(w)
            yp = ps_bigA() if mi % 2 == 0 else ps_bigB()
            for k in range(KT):
                for n2 in range(2):
                    nc.tensor.matmul(yp[:, ts(n2, 512)],
                                     w1h[k][:, ts(mi % 4, 128)],
                                     h2T[k][:, ts(n2, 512)],
                                     start=(k == 0), stop=(k == KT - 1))
            yt = utile(pC4, [128, N], F32R, "yt", bufs=8)
            nc.scalar.activation(yt[:], yp[:], AF.Gelu_apprx_tanh,
                                 bias=bm1T[:, mt:mt + 1])
            ytiles.append(yt)
        for ch in range(2):
            w2h = []
            for mi in range(8):
                mt = g * 8 + mi
                w = utile(pC4, [128, 512], F32R, "w2h", bufs=8)
                nc.sync.dma_start(w[:],
                                  wd["W2"][ts(mt, 128), ts(ch, 512)].bitcast(F32R))
                w2h.append(w)
            for cc in range(4):
                c = ch * 4 + cc
                for n2 in range(2):
                    zp = ps_mm() if n2 == 0 else                         ps_av().rearrange("p a b -> p (a b)")
                    for mi in range(8):
                        nc.tensor.matmul(zp[:, :512],
                                         w2h[mi][:, ts(cc, 128)],
                                         ytiles[mi][:, ts(n2, 512)],
                                         start=(mi == 0), stop=(mi == 7))
                    with nc.allow_low_precision(reason="mlp psums f32r"):
                        if g == 0:
                            nc.vector.tensor_scalar(zacc[c][:, ts(n2, 512)],
                                                    zp[:, :512],
                                                    bm2T[:, c:c + 1], None,
                                                    OP.add)
                        else:
                            nc.vector.tensor_tensor(zacc[c][:, ts(n2, 512)],
                                                    zacc[c][:, ts(n2, 512)],
                                                    zp[:, :512], OP.add)

    # ---- residual + transpose -> row-major mod (bf16, pZ-resident) ----
    mod_sb = [utile(pZ, [128, N], BF16, f"modr{i}") for i in range(8)]
    for i in range(8):
        resf = utile(pt, [128, C], F32, "xin")
        nc.sync.dma_start(resf[:], row_src_ap(i))
        zt = ps_bigA() if i % 2 == 0 else ps_bigB()
        for c in range(8):
            nc.tensor.transpose(zt[:, ts(c, 128)].bitcast(F32R),
                                zacc[c][:, ts(i, 128)], ident_r[:, :])
        with nc.allow_low_precision(reason="mod rows stored bf16"):
            nc.vector.tensor_tensor(mod_sb[i][:], zt[:], resf[:], OP.add)
    if dbg:
        for i in range(8):
            mff = utile(pC4, [128, C], F32, "mof", bufs=2)
            nc.vector.tensor_copy(mff[:], mod_sb[i][:])
            nc.sync.dma_start(dbg["dbg_mlpout"][ts(i, 128), :], mff[:])
    pC4.release()

    # ================= Stage D: pooling + Wp -> combined^T =================
    pD = tc.alloc_tile_pool(name="pD", bufs=1)
    # ApT[p, f] = 0.5 iff source row p pools into output f:
    #   even block: base = rank[p]-16, match iff (2f - base) in {-1, 0}
    #   dst  block: base = d,          match iff (2(f-248) - base) in {-1, 0}
    iota2e = utile(pD, [128, 504], F32, "iota2e")
    nc.vector.tensor_scalar_mul(iota2e[:], IOTA504B[:], 2.0)
    apT = [utile(pD, [128, 504], BF16, f"apT{m}") for m in range(8)]
    for m in range(8):
        base = utile(psm, [128, 1], F32, "ap_r")
        if m < 4:
            nc.vector.tensor_scalar_add(base[:], rank_t[:, m:m + 1], -float(R))
        else:
            nc.vector.tensor_scalar_add(base[:], piota[:],
                                        float(128 * (m - 4) + NE - R))
        d1 = utile(pD, [128, 504], F32, "ap_d1")
        nc.vector.tensor_scalar(d1[:], iota2e[:], base[:], None, OP.subtract)
        a1 = utile(pD, [128, 504], F32, "ap_a1")
        nc.vector.tensor_scalar(a1[:], d1[:], -1.5, None, OP.is_ge)
        b1 = utile(pD, [128, 504], F32, "ap_b1")
        nc.vector.tensor_scalar(b1[:], d1[:], 0.5, None, OP.is_le)
        nc.vector.scalar_tensor_tensor(apT[m][:], a1[:], 0.5, b1[:],
                                       OP.mult, OP.mult)
    # pooledT[c][128c, 504] = sum_i mod_sb[i][:, c-tile]^T @ apT[i]
    pooledT = [utile(pD, [128, NP], F32R, f"pooledT{k}") for k in range(8)]
    for c in range(8):
        zp = ps_mm() if c % 2 == 0 else         ps_av().rearrange("p a b -> p (a b)")
        for i in range(8):
            nc.tensor.matmul(zp[:, :NP], mod_sb[i][:, ts(c, 128)], apT[i][:],
                             start=(i == 0), stop=(i == 7))
        nc.vector.tensor_copy(pooledT[c][:], zp[:, :NP])
    if dbg:
        for m in range(4):
            pst = utile(pt, [128, C], F32, "xin")
            for bj in range(8):
                p = ps_av()
                pf = p.rearrange("p a b -> p (a b)")
                nc.tensor.transpose(pf[:PP, :128].bitcast(F32R),
                                    pooledT[bj][:, m * PP:(m + 1) * PP],
                                    ident_r[:, :])
                nc.vector.tensor_copy(pst[:PP, ts(bj, 128)], pf[:PP, :128])
            nc.sync.dma_start(dbg["dbg_pooled"][ts(m, PP), :], pst[:PP, :])

    pE = tc.alloc_tile_pool(name="pE", bufs=1)
    bp3T = bcol("bp", pD, scale=3.0)
    cmbTr = [utile(pD, [128, NP], F32R, f"cmbTr{m}") for m in range(8)]
    for mg in range(2):
        wcs = []
        for k in range(KT):
            wcr = utile(pD, [128, 512], F32R, f"wpc{k}", bufs=1)
            nc.sync.dma_start(wcr[:], wd["Wp"][ts(k, 128), ts(mg, 512)].bitcast(F32R))
            wcs.append(wcr)
        for mi in range(4):
            m = mg * 4 + mi
            acc = ps_mm()
            for k in range(KT):
                nc.tensor.matmul(acc[:, :NP], wcs[k][:, ts(mi, 128)],
                                 pooledT[k][:], start=(k == 0), stop=(k == KT - 1))
            nc.scalar.activation(cmbTr[m][:], acc[:, :NP], AF.Identity,
                                 bias=bp3T[:, m:m + 1], scale=3.0)

    # ================= Stage E: MQA =================
    bqT = bcol("bq", pE)

    _wq512 = {}

    def make_mqT(m):
        mg = m // 4
        if mg not in _wq512:
            ws = []
            for k in range(KT):
                w = utile(pE, [128, 512], F32R, "wq512", bufs=8)
                nc.sync.dma_start(
                    w[:], wd["Wq"][ts(k, 128), ts(mg, 512)].bitcast(F32R))
                ws.append(w)
            _wq512.clear()
            _wq512[mg] = ws
        ws = _wq512[mg]
        acc = ps_mm()
        for k in range(KT):
            nc.tensor.matmul(acc[:, :NP], ws[k][:, ts(m % 4, 128)], cmbTr[k][:],
                             start=(k == 0), stop=(k == KT - 1))
        t = utile(pE, [128, NP], F32R, "mqT", bufs=2)
        with nc.allow_low_precision(reason="bias add, f32r storage"):
            nc.vector.tensor_scalar(t[:], acc[:, :NP], bqT[:, m:m + 1], None,
                                    OP.add)
        return t

    wkvr = utile(pE, [128, KT, 2 * DH], F32R, "wkvr")
    for k in range(KT):
        nc.sync.dma_start(wkvr[:, k, :DH], wd["Wk"][ts(k, 128), :].bitcast(F32R))
        nc.sync.dma_start(wkvr[:, k, DH:], wd["Wv"][ts(k, 128), :].bitcast(F32R))
    bkT = utile(pE, [64, 1], F32, "bkT")
    nc.sync.dma_start(bkT[:], wd["bk"][:, None])
    mkT = utile(pE, [128, NP], F32R, "mkT")
    macc = ps_mm()
    for k in range(KT):
        nc.tensor.matmul(macc[:64, :NP], wkvr[:, k, :DH], cmbTr[k][:],
                         start=(k == 0), stop=(k == KT - 1))
    mkf = utile(pE, [64, NP], F32, "mkf")
    nc.scalar.activation(mkf[:], macc[:64, :NP], AF.Identity, bias=bkT[:])
    nc.vector.tensor_copy(mkT[:64, :], mkf[:])
    nc.sync.dma_start(mkT[64:, :], mkT[:64, :])

    bvb = bcast_row(load_row(wd["bv"], DH, "bv_r", pE), DH, "bv_b", pE)
    mv_pad = [utile(pE, [128, DH + 1], F32R, f"mvp{m}") for m in range(4)]
    for m in range(4):
        acc = ps_av()
        for k in range(KT):
            nc.tensor.matmul(acc[:PP, 0, :DH], cmbTr[k][:, m * PP:(m + 1) * PP],
                             wkvr[:, k, DH:], start=(k == 0), stop=(k == KT - 1))
        nc.vector.tensor_copy(mv_pad[m][:, DH:], ones_part[:, :1])
        nc.vector.tensor_tensor(mv_pad[m][:PP, :DH], acc[:PP, 0, :DH], bvb[:PP, :],
                                OP.add)

    mqT_cur = None
    for h in range(H):
        po = (h % 2) * 64
        if h % 2 == 0:
            mqT_cur = make_mqT(h // 2)
        mqT_h = mqT_cur[po:po + 64, :]
        spA = ps_bigA()
        spB = ps_bigB()
        ep = []
        for mm in range(4):
            spbig = spA if mm < 2 else spB
            sp = spbig[:, 512 * (mm % 2):512 * (mm % 2) + NP]
            nc.tensor.matmul(sp[:PP, :NP], mkT[po:po + 64, mm * PP:(mm + 1) * PP],
                             mqT_h[:], start=True, stop=True)
            et = utile(pE, [128, NP], F32R, "e2", bufs=6)
            nc.scalar.activation(et[:PP, :], sp[:PP, :NP], AF.Exp,
                                 scale=float(DH ** -0.5))
            ep.append(et)
        av2 = ps_av().rearrange("p a b -> p (a b)")
        for mm in range(4):
            nc.tensor.matmul(av2[:DH + 1, :NP], mv_pad[mm][:PP, :],
                             ep[mm][:PP, :], start=(mm == 0), stop=(mm == 3))
        rrow = utile(pE, [1, NP], F32R, "rrow2", bufs=2)
        with nc.allow_low_precision(reason="softmax denom; 11-bit ok downstream"):
            nc.vector.reciprocal(rrow[:], av2[DH:DH + 1, :NP])
        rb = ps_mm()
        nc.tensor.matmul(rb[:DH, :NP], ones_col_r[:, :DH], rrow[:],
                         start=True, stop=True)
        rbs = utile(pE, [64, NP], F32, "rbs2", bufs=2)
        nc.vector.tensor_copy(rbs[:], rb[:DH, :NP])
        stg = utile(pE, [64, NP], F32R, "mqstg", bufs=2)
        nc.vector.tensor_tensor(stg[:], av2[:DH, :NP], rbs[:], OP.mult)
        nc.sync.dma_start(mqaTd[h * 64:h * 64 + 64, :], stg[:])
    pE.release()
    pD.release()
    pZ.release()

    # ================= Stage F: Wmo + FFN =================
    pF = tc.alloc_tile_pool(name="pF", bufs=1)
    pF1 = tc.alloc_tile_pool(name="pF1", bufs=1)
    mqaT = [utile(pF1, [128, NP], F32R, f"mqaT{k}") for k in range(8)]
    for k in range(8):
        nc.sync.dma_start(mqaT[k][:, :NP], mqaTd[ts(k, 128), :])
    bmoT = bcol("bmo", pF1)
    omoT = [utile(pF, [128, NP], F32R, f"omoT{m}") for m in range(8)]
    for mg in range(2):
        wcs = []
        for k in range(KT):
            wcr = utile(pF1, [128, 512], F32R, f"wmc{k}", bufs=1)
            nc.sync.dma_start(wcr[:],
                              wd["Wmo"][ts(k, 128), ts(mg, 512)].bitcast(F32R))
            wcs.append(wcr)
        for mi in range(4):
            m = mg * 4 + mi
            acc = ps_mm()
            for k in range(KT):
                nc.tensor.matmul(acc[:, :NP], wcs[k][:, ts(mi, 128)],
                                 mqaT[k][:], start=(k == 0), stop=(k == KT - 1))
            nc.scalar.activation(omoT[m][:], acc[:, :NP], AF.Identity,
                                 bias=bmoT[:, m:m + 1])
    pF1.release()

    # ---- FFN c-major: Y halves of 16 hidden tiles, Z accum in SBUF ----
    bf1T = bcol("bf1", pF)
    bf2b = brow("bf2", pF)
    zf = [utile(pF, [128, NP], F32R, f"zf{c}") for c in range(8)]
    for hf in range(2):
        yfs = []
        wf1h = None
        for kki in range(16):
            kk = hf * 16 + kki
            if kki % 4 == 0:
                wf1h = []
                for k in range(KT):
                    w = utile(pF, [128, 512], F32R, "wf1h", bufs=9)
                    nc.sync.dma_start(
                        w[:], wd["Wf1"][ts(k, 128), ts(kk // 4, 512)]
                        .bitcast(F32R))
                    wf1h.append(w)
            yp = ps_mm() if kki % 2 == 0 else             ps_av().rearrange("p a b -> p (a b)")
            for k in range(KT):
                nc.tensor.matmul(yp[:, :NP], wf1h[k][:, ts(kki % 4, 128)],
                                 omoT[k][:], start=(k == 0), stop=(k == KT - 1))
            yf = utile(pF, [128, NP], F32R, "yf", bufs=17)
            nc.scalar.activation(yf[:], yp[:, :NP], AF.Silu,
                                 bias=bf1T[:, kk:kk + 1])
            yfs.append(yf)
        for ch in range(2):
            wf2h = []
            for kki in range(16):
                kk = hf * 16 + kki
                w = utile(pF, [128, 512], F32R, "wf2h", bufs=17)
                nc.sync.dma_start(w[:],
                                  wd["Wf2"][ts(kk, 128), ts(ch, 512)].bitcast(F32R))
                wf2h.append(w)
            for cc in range(4):
                c = ch * 4 + cc
                zp = ps_bigA() if cc % 2 == 0 else ps_bigB()
                for kki in range(16):
                    nc.tensor.matmul(zp[:, :NP], wf2h[kki][:, ts(cc, 128)],
                                     yfs[kki][:], start=(kki == 0),
                                     stop=(kki == 15))
                with nc.allow_low_precision(reason="ffn out partials f32r"):
                    if hf == 0:
                        nc.vector.tensor_copy(zf[c][:], zp[:, :NP])
                    else:
                        nc.vector.tensor_tensor(zf[c][:], zf[c][:], zp[:, :NP],
                                                OP.add)
    # ---- transpose back to row-major + bias -> out ----
    for tl in range(4):
        of = utile(pF, [128, C], F32, "of", bufs=2)
        for c in range(8):
            p = ps_av()
            pf = p.rearrange("p a b -> p (a b)")
            nc.tensor.transpose(pf[:PP, :128].bitcast(F32R),
                                zf[c][:, tl * PP:(tl + 1) * PP], ident_r[:, :])
            nc.vector.tensor_tensor(of[:PP, ts(c, 128)], pf[:PP, :128],
                                    bf2b[:PP, ts(c, 128)], OP.add)
        nc.sync.dma_start(out_d[tl * PP:(tl + 1) * PP, :], of[:PP, :])
    pF.release()
    for pool in (pt, pw, psm, pc, pp):
        pool.release()


_BUILT = None


def kernel(**inputs):
    global _BUILT
    if _BUILT is None:
        _BUILT = build(debug=DEBUG)
    nc = _BUILT
    x = np.ascontiguousarray(inputs["x"], dtype=np.float32)
    base = {k: np.ascontiguousarray(v, dtype=np.float32) for k, v in inputs.items()
            if k != "x"}
    in_maps = []
    for i in range(8):
        m = dict(base)
        m["x"] = x[i]
        in_maps.append(m)
    res = run_bass_kernel_spmd(nc, in_maps, core_ids=list(range(8)))
    out = np.stack([res.results[i]["out"] for i in range(8)], axis=0)
    return out.astype(np.float32)



# revision 67
# speedup vs baseline: 1.0524x; 1.0057x over previous
"""AdaptiveTokenMerger (ToMe block + merger) TRN2 Bass kernel.

Data-parallel over batch: 8 samples -> 8 NeuronCores, one sample per core.
Per-core pipeline (sample x [1024, 1024]):
  A (f32, ranking-critical): LN1 -> qkv -> MHA (transposed-softmax with the
    denominator folded in as an appended ones-column of v) -> Wo -> x_attn
  B: metric scores -> node_max/argmax -> ranks via pairwise comparisons ->
    dst scatter-add expressed as a one-hot matmul
  C (f32r): MLP over rows [x1_even(512); dst_new(512)], fused W1/W2 per
    token-quarter, output accumulated in PSUM across all 32 W1 column tiles
  D: pooling as a rank-dependent one-hot matmul -> Wp -> combined = 3q
  E (f32r): multi-query attention  F (f32r): FFN -> out [504, 1024]

Precision: everything upstream of the rank/argmax decisions is true fp32
(4 cyc/row on PE); post-merge matmuls use float32r (TF32-ish, 1 cyc/row).

PSUM budget (8 banks): BIGA/BIGB [128,1024] (2+2), MM [128,512] x2 (2),
AV [128,4,128] x2 (2).
"""
import numpy as np

import concourse.bass as bass
import concourse.tile as tile
from concourse import bacc, mybir
from concourse.bass import ts
from concourse.bass_utils import run_bass_kernel_spmd
from concourse.masks import make_identity

F32 = mybir.dt.float32
F32R = mybir.dt.float32r
FP16 = mybir.dt.float16
BF16 = mybir.dt.bfloat16
U32 = mybir.dt.uint32

N, C, H = 1024, 1024, 16
R = 16
DH = C // H          # 64
NE = N // 2          # 512
NP = (N - R) // 2    # 504
PP = 126             # pooled tokens per partition tile
KT = C // 128        # 8
AF = mybir.ActivationFunctionType
OP = mybir.AluOpType

DEBUG = False


def build(debug=False):
    nc = bacc.Bacc("TRN2", target_bir_lowering=False, debug=False, num_devices=8)
    x_d = nc.dram_tensor("x", [N, C], F32, kind="ExternalInput").ap()
    wd = {}
    for name, shape in [
        ("g1", [C]), ("be1", [C]), ("Wqkv", [C, 3 * C]), ("bqkv", [3 * C]),
        ("Wo", [C, C]), ("bo", [C]), ("g2", [C]), ("be2", [C]),
        ("W1", [C, 4 * C]), ("bm1", [4 * C]), ("W2", [4 * C, C]), ("bm2", [C]),
        ("Wp", [C, C]), ("bp", [C]), ("Wq", [C, C]), ("bq", [C]),
        ("Wk", [C, DH]), ("bk", [DH]), ("Wv", [C, DH]), ("bv", [DH]),
        ("Wmo", [C, C]), ("bmo", [C]), ("Wf1", [C, 4 * C]), ("bf1", [4 * C]),
        ("Wf2", [4 * C, C]), ("bf2", [C]),
    ]:
        wd[name] = nc.dram_tensor(name, shape, F32, kind="ExternalInput").ap()
    out_d = nc.dram_tensor("out", [NP, C], F32, kind="ExternalOutput").ap()
    dbg = {}
    if debug:
        for name, shape in [
            ("dbg_xattn", [N, C]), ("dbg_nm", [NE]), ("dbg_rank", [NE]),
            ("dbg_nodeidx", [NE]), ("dbg_mlpin", [N, C]), ("dbg_mlpout", [N, C]),
            ("dbg_pooled", [NP, C]),
        ]:
            dbg[name] = nc.dram_tensor(name, shape, F32, kind="ExternalOutput").ap()
    with tile.TileContext(nc) as tc:
        _build_tile(nc, tc, x_d, wd, out_d, dbg)
    nc.compile()
    return nc


def _build_tile(nc, tc, x_d, wd, out_d, dbg):
    # DRAM spill buffers
    qkTd = nc.dram_tensor("qkTd", [2 * C, N], F32).ap()
    aoTd = nc.dram_tensor("aoTd", [C, N], F32).ap()
    x1d = nc.dram_tensor("x1d", [N, C], F32).ap()
    dstnd = nc.dram_tensor("dstnd", [NE, C], F32).ap()
    mqaTd = nc.dram_tensor("mqaTd", [C, NP], F32R).ap()

    pc = tc.alloc_tile_pool(name="const", bufs=1)
    psm = tc.alloc_tile_pool(name="small", bufs=1)
    pw = tc.alloc_tile_pool(name="wstream", bufs=2)
    pt = tc.alloc_tile_pool(name="tmp", bufs=2)
    pp = tc.alloc_tile_pool(name="psum", bufs=1, space="PSUM")

    _ct = {}

    def utile(pool, shape, dtype, tag, bufs=None):
        _ct[tag] = _ct.get(tag, 0) + 1
        kw = {"bufs": bufs} if bufs is not None else {}
        return pool.tile(shape, dtype, tag=tag, name=f"{tag}_{_ct[tag]}", **kw)

    def ps_bigA():
        return utile(pp, [128, 1024], F32, "BIGA")

    def ps_bigB():
        return utile(pp, [128, 1024], F32, "BIGB")

    def ps_mm():
        return utile(pp, [128, 512], F32, "MM", bufs=2)

    def ps_av():
        return utile(pp, [128, 4, 128], F32, "AV", bufs=2)

    # ---------- constants ----------
    ident = pc.tile([128, 128], F32)
    make_identity(nc, ident[:])
    ident_r = pc.tile([128, 128], F32R)
    nc.vector.tensor_copy(ident_r[:], ident[:])
    ones_col = pc.tile([1, 128], F32)
    nc.gpsimd.memset(ones_col[:], 1.0)
    ones_col_r = pc.tile([1, 128], F32R)
    nc.vector.tensor_copy(ones_col_r[:], ones_col[:])
    ones_part = pc.tile([128, 8], F32)
    nc.gpsimd.memset(ones_part[:], 1.0)
    piota = pc.tile([128, 1], F32)
    nc.gpsimd.iota(piota[:], [[0, 1]], channel_multiplier=1,
                   allow_small_or_imprecise_dtypes=True)
    iota512_row = pc.tile([1, 512], F32)
    nc.gpsimd.iota(iota512_row[:], [[1, 512]], channel_multiplier=0,
                   allow_small_or_imprecise_dtypes=True)
    iota504_row = pc.tile([1, 504], F32)
    nc.gpsimd.iota(iota504_row[:], [[1, 504]], channel_multiplier=0,
                   allow_small_or_imprecise_dtypes=True)

    def bcast_row(row_ap, n, tag, pool, scale=1.0):
        t = utile(pool, [128, n], F32, tag)
        for c0 in range(0, n, 512):
            cw = min(512, n - c0)
            p = ps_mm()
            nc.tensor.matmul(p[:, :cw], ones_col[:], row_ap[:, c0:c0 + cw],
                             start=True, stop=True)
            if scale == 1.0:
                nc.vector.tensor_copy(t[:, c0:c0 + cw], p[:, :cw])
            else:
                nc.vector.tensor_scalar_mul(t[:, c0:c0 + cw], p[:, :cw], scale)
        return t

    def load_row(dram_ap, n, tag, pool):
        t = utile(pw, [1, n], F32, "rowstg", bufs=1)
        nc.sync.dma_start(t[:], dram_ap[None, :])
        return t

    def brow(name, pool, scale=1.0):
        n = wd[name].shape[0]
        return bcast_row(load_row(wd[name], n, name + "_r", pool), n,
                         name + "_b", pool, scale)

    def bcol(name, pool, scale=1.0):
        n = wd[name].shape[0]
        t = utile(pool, [128, n // 128], F32, name + "_c")
        nc.sync.dma_start(t[:], wd[name].rearrange("(t p) -> p t", p=128))
        if scale != 1.0:
            nc.vector.tensor_scalar_mul(t[:], t[:], scale)
        return t

    IOTA512B = bcast_row(iota512_row[:], 512, "iota512b", pc)
    IOTA504B = bcast_row(iota504_row[:], 504, "iota504b", pc)

    def transpose_blocks(src_tiles, dst, n_rows, n_cols):
        """dst[c, r] = src[r, c]; dst is tile-list or sink(bj, bi, pf, cw, rw)."""
        for bi in range((n_rows + 127) // 128):
            rw = min(128, n_rows - bi * 128)
            for bj in range((n_cols + 127) // 128):
                cw = min(128, n_cols - bj * 128)
                p = ps_av()
                pf = p.rearrange("p a b -> p (a b)")
                nc.tensor.transpose(pf[:cw, :rw],
                                    src_tiles[bi][:rw, bj * 128:bj * 128 + cw],
                                    ident[:rw, :rw])
                if callable(dst):
                    dst(bj, bi, pf, cw, rw)
                else:
                    nc.vector.tensor_copy(dst[bj][:cw, bi * 128:bi * 128 + rw],
                                          pf[:cw, :rw])

    def refined_rsqrt_recip(vv, tag):
        """returns 1/sqrt(vv) with one Newton step on sqrt (ACT sqrt is loose)."""
        s0 = utile(psm, [128, 1], F32, tag + "_s0")
        nc.scalar.sqrt(s0[:], vv[:])
        r0 = utile(psm, [128, 1], F32, tag + "_r0")
        nc.vector.reciprocal(r0[:], s0[:])
        t = utile(psm, [128, 1], F32, tag + "_t")
        nc.vector.tensor_tensor(t[:], vv[:], r0[:], OP.mult)
        nc.vector.tensor_tensor(t[:], t[:], s0[:], OP.add)
        nc.vector.tensor_scalar_mul(t[:], t[:], 0.5)
        rr = utile(psm, [128, 1], F32, tag + "_rr")
        nc.vector.reciprocal(rr[:], t[:])
        return rr

    def layer_norm(src, dst, gb, bb, lnpool):
        m = utile(psm, [128, 1], F32, "ln_m")
        nc.vector.reduce_sum(m[:], src[:, :C], axis=mybir.AxisListType.X)
        nc.vector.tensor_scalar_mul(m[:], m[:], 1.0 / C)
        xc = utile(lnpool, [128, C], F32, "ln_xc", bufs=1)
        nc.vector.tensor_scalar(xc[:], src[:, :C], m[:], None, OP.subtract)
        ss = utile(psm, [128, 1], F32, "ln_ss")
        nc.scalar.activation(dst[:, :C], xc[:], AF.Square, accum_out=ss[:])
        v = utile(psm, [128, 1], F32, "ln_v")
        nc.vector.tensor_scalar(v[:], ss[:], 1.0 / C, 1e-5, OP.mult, OP.add)
        rstd = refined_rsqrt_recip(v, "ln")
        nc.vector.tensor_scalar(dst[:, :C], xc[:], rstd[:], None, OP.mult)
        nc.vector.tensor_tensor(dst[:, :C], dst[:, :C], gb[:], OP.mult)
        nc.vector.tensor_tensor(dst[:, :C], dst[:, :C], bb[:], OP.add)

    # ================= Stage A: LN1 -> hT =================
    pbA = tc.alloc_tile_pool(name="biasA", bufs=1)
    pHT = tc.alloc_tile_pool(name="pHT", bufs=1)

    g1b = brow("g1", pbA)
    be1b = brow("be1", pbA)
    hT = [utile(pHT, [128, N], F32, f"hT{k}") for k in range(8)]
    ht = []
    for i in range(8):
        xt = utile(pt, [128, C], F32, "xin")
        nc.sync.dma_start(xt[:], x_d[ts(i, 128), :])
        h = utile(pHT, [128, C], F32, "ht", bufs=2)
        layer_norm(xt, h, g1b, be1b, pbA)
        ht.append(h)
    transpose_blocks(ht, hT, N, C)

    def split16(dst_hi, dst_lo, src_ap):
        nc.scalar.activation(dst_hi, src_ap, AF.Identity)
        with nc.allow_low_precision(reason="fp16 split residual"):
            nc.vector.tensor_tensor(dst_lo, src_ap, dst_hi, OP.subtract)

    def split16s(dst_hi, dst_lo, src_ap):
        # scale by 64 so the lo half stays in fp16 normal range (w ~ 0.02)
        nc.scalar.activation(dst_hi, src_ap, AF.Identity, scale=64.0)
        with nc.allow_low_precision(reason="fp16 split residual"):
            nc.vector.scalar_tensor_tensor(dst_lo, src_ap, 64.0, dst_hi,
                                           OP.mult, OP.subtract)

    # ===== qk^T -> qkTd (DRAM) ; v_pad (SBUF) =====
    pQK = tc.alloc_tile_pool(name="pQK", bufs=1)
    bqkT = bcol("bqkv", pbA)
    hThi = [utile(pQK, [128, N], FP16, f"hThi{k}") for k in range(8)]
    hTlo = [utile(pQK, [128, N], FP16, f"hTlo{k}") for k in range(8)]
    for k in range(8):
        split16(hThi[k][:], hTlo[k][:], hT[k][:])
    for mpg in range(2):
        wqh, wql, wkh, wkl = [], [], [], []
        for k in range(KT):
            wqf = utile(pw, [128, 512], F32, "wqk512", bufs=3)
            nc.sync.dma_start(wqf[:], wd["Wqkv"][ts(k, 128), ts(mpg, 512)])
            hi = utile(pQK, [128, 512], FP16, "wqh", bufs=8)
            lo = utile(pQK, [128, 512], FP16, "wql", bufs=8)
            split16(hi[:], lo[:], wqf[:])
            wqh.append(hi)
            wql.append(lo)
            wkf = utile(pw, [128, 512], F32, "wqk512", bufs=3)
            nc.sync.dma_start(wkf[:],
                              wd["Wqkv"][ts(k, 128), C + mpg * 512:C + mpg * 512 + 512])
            hi = utile(pQK, [128, 512], FP16, "wkh", bufs=8)
            lo = utile(pQK, [128, 512], FP16, "wkl", bufs=8)
            split16(hi[:], lo[:], wkf[:])
            wkh.append(hi)
            wkl.append(lo)
        for mi in range(4):
            mp = mpg * 4 + mi
            accq = ps_bigA()
            acck = ps_bigB()
            for k in range(KT):
                for n2 in range(2):
                    for wgt, mov in ((wqh[k], hThi[k]), (wqh[k], hTlo[k]),
                                     (wql[k], hThi[k])):
                        nc.tensor.matmul(accq[:, ts(n2, 512)],
                                         wgt[:, ts(mi, 128)],
                                         mov[:, ts(n2, 512)],
                                         start=(k == 0 and wgt is wqh[k]
                                                and mov is hThi[k]),
                                         stop=(k == KT - 1 and wgt is wql[k]))
                    for wgt, mov in ((wkh[k], hThi[k]), (wkh[k], hTlo[k]),
                                     (wkl[k], hThi[k])):
                        nc.tensor.matmul(acck[:, ts(n2, 512)],
                                         wgt[:, ts(mi, 128)],
                                         mov[:, ts(n2, 512)],
                                         start=(k == 0 and wgt is wkh[k]
                                                and mov is hThi[k]),
                                         stop=(k == KT - 1 and wgt is wkl[k]))
            stgq = utile(pQK, [128, N], F32, "qkstg", bufs=2)
            nc.scalar.activation(stgq[:], accq[:], AF.Identity,
                                 bias=bqkT[:, mp:mp + 1])
            nc.sync.dma_start(qkTd[ts(mp, 128), :], stgq[:])
            stgk = utile(pQK, [128, N], F32, "qkstg", bufs=2)
            nc.scalar.activation(stgk[:], acck[:], AF.Identity,
                                 bias=bqkT[:, 8 + mp:9 + mp])
            nc.sync.dma_start(qkTd[C + mp * 128:C + (mp + 1) * 128, :], stgk[:])

    pQK.release()
    pVP = tc.alloc_tile_pool(name="pVP", bufs=1)
    bvqkvb = bcast_row(load_row(wd["bqkv"][2 * C:], C, "bvq_r", pbA), C,
                       "bvq_b", pbA)
    v_pad = [utile(pVP, [128, H, DH + 1], F32, f"vp{j}") for j in range(8)]
    for j in range(8):
        nc.vector.memset(v_pad[j][:, :, DH:DH + 1], 1.0)
    for jg in range(4):
        accs = [ps_bigA(), ps_bigB()]
        for k in range(KT):
            wv = utile(pVP, [128, C], F32, "wv", bufs=2)
            nc.sync.dma_start(wv[:], wd["Wqkv"][ts(k, 128), 2 * C:])
            wvh = utile(pVP, [128, C], FP16, "wvh", bufs=2)
            wvl = utile(pVP, [128, C], FP16, "wvl", bufs=2)
            split16(wvh[:], wvl[:], wv[:])
            hh = utile(pVP, [128, C], FP16, "hvh", bufs=2)
            hl = utile(pVP, [128, C], FP16, "hvl", bufs=2)
            split16(hh[:], hl[:], hT[k][:])
            for jj in range(2):
                j = jg * 2 + jj
                for n2 in range(2):
                    for sta, mov in ((hh, wvh), (hh, wvl), (hl, wvh)):
                        nc.tensor.matmul(accs[jj][:, ts(n2, 512)],
                                         sta[:, ts(j, 128)],
                                         mov[:, ts(n2, 512)],
                                         start=(k == 0 and sta is hh
                                                and mov is wvh),
                                         stop=(k == KT - 1 and sta is hl))
        for jj in range(2):
            j = jg * 2 + jj
            for h in range(H):
                nc.vector.tensor_tensor(v_pad[j][:, h, :DH],
                                        accs[jj][:, ts(h, DH)],
                                        bvqkvb[:, ts(h, DH)], OP.add)

    # ===== attention: stream kT/qT per head; out -> aoTd (already c-major) ==
    # out[dh|sum, i] = v_pad[j].T @ expT[j, i], accumulated over j-tiles.
    pAttn = tc.alloc_tile_pool(name="pAttn", bufs=1)
    for h in range(H):
        kth = utile(pAttn, [64, N], F32, "kth", bufs=2)
        nc.sync.dma_start(kth[:], qkTd[C + h * 64:C + h * 64 + 64, :])
        qth = utile(pAttn, [64, N], F32, "qth", bufs=2)
        nc.sync.dma_start(qth[:], qkTd[h * 64:h * 64 + 64, :])
        khi = utile(pAttn, [64, N], FP16, "khi", bufs=2)
        klo = utile(pAttn, [64, N], FP16, "klo", bufs=2)
        split16(khi[:], klo[:], kth[:])
        qhi = utile(pAttn, [64, N], FP16, "qhi", bufs=2)
        qlo = utile(pAttn, [64, N], FP16, "qlo", bufs=2)
        split16(qhi[:], qlo[:], qth[:])
        av = [ps_av().rearrange("p a b -> p (a b)") for _ in range(2)]
        for j in range(8):
            for n2 in range(2):
                sp = ps_mm()
                for sta, mov in ((khi, qhi), (khi, qlo), (klo, qhi)):
                    nc.tensor.matmul(sp[:], sta[:, ts(j, 128)],
                                     mov[:, ts(n2, 512)],
                                     start=(sta is khi and mov is qhi),
                                     stop=(sta is klo))
                et = utile(pAttn, [128, 512], F32, "exp", bufs=3)
                nc.scalar.activation(et[:], sp[:], AF.Exp, scale=float(DH ** -0.5))
                nc.tensor.matmul(av[n2][:DH + 1, :512], v_pad[j][:, h, :], et[:],
                                 start=(j == 0), stop=(j == 7))
        for n2 in range(2):
            rrow = utile(pAttn, [1, 512], F32, "rrow", bufs=2)
            nc.vector.reciprocal(rrow[:], av[n2][DH:DH + 1, :512])
            rb = ps_mm()
            nc.tensor.matmul(rb[:DH, :512], ones_col[:, :DH], rrow[:],
                             start=True, stop=True)
            rbs = utile(pAttn, [64, 512], F32, "rbs", bufs=2)
            nc.vector.tensor_copy(rbs[:], rb[:DH, :512])
            stg = utile(pAttn, [64, 512], F32, "aot_stg", bufs=2)
            nc.vector.tensor_tensor(stg[:], av[n2][:DH, :512], rbs[:],
                                    OP.mult)
            nc.sync.dma_start(aoTd[h * 64:h * 64 + 64, ts(n2, 512)], stg[:])
    pAttn.release()
    pVP.release()
    pHT.release()
    pbA.release()

    # ================= Wo -> x_attn, x1 (-> DRAM), metric =================
    pB2 = tc.alloc_tile_pool(name="pB2", bufs=1)
    bob = brow("bo", pB2)
    xa = [utile(pB2, [128, C], F32, f"xa{m}") for m in range(8)]
    woR = [utile(pB2, [128, C], F32, f"woR{k}") for k in range(8)]
    for k in range(KT):
        nc.sync.dma_start(woR[k][:], wd["Wo"][ts(k, 128), :])
    rn = psm.tile([128, 8], F32)
    for m in range(8):
        acc = ps_bigA()
        for k in range(KT):
            ao = utile(pw, [128, 128], F32, "wqkb", bufs=4)
            nc.sync.dma_start(ao[:], aoTd[ts(k, 128), ts(m, 128)])
            for n2 in range(2):
                nc.tensor.matmul(acc[:, ts(n2, 512)], ao[:],
                                 woR[k][:, ts(n2, 512)],
                                 start=(k == 0), stop=(k == KT - 1))
        nc.vector.tensor_tensor(xa[m][:], acc[:], bob[:], OP.add)
        xt = utile(pt, [128, C], F32, "xin")
        nc.sync.dma_start(xt[:], x_d[ts(m, 128), :])
        x1stg = utile(pw, [128, C], F32, "x1stg", bufs=1)
        nc.vector.tensor_tensor(x1stg[:], xa[m][:], xt[:], OP.add)
        nc.sync.dma_start(x1d[ts(m, 128), :], x1stg[:])
        ss = utile(psm, [128, 1], F32, "nrm_ss")
        sq = utile(pt, [128, C], F32, "ln_xc")
        nc.scalar.activation(sq[:], xa[m][:], AF.Square, accum_out=ss[:])
        rr = refined_rsqrt_recip(ss, "nrm")
        nc.vector.tensor_copy(rn[:, m:m + 1], rr[:])
        nc.vector.tensor_scalar(xa[m][:], xa[m][:], rn[:, m:m + 1], None, OP.mult)
        if dbg:
            nc.sync.dma_start(dbg["dbg_xattn"][ts(m, 128), :], xa[m][:])

    # ===== de-interleave metric -> maT/mbT; scores; node stats; ranks =====
    pB3 = tc.alloc_tile_pool(name="pB3", bufs=1)
    xae = [utile(pB3, [128, C], F32, f"xae{m}") for m in range(4)]
    xao = [utile(pB3, [128, C], F32, f"xao{m}") for m in range(4)]
    for m in range(4):
        nc.sync.dma_start(xae[m][:64, :], xa[2 * m][0:128:2, :])
        nc.sync.dma_start(xae[m][64:, :], xa[2 * m + 1][0:128:2, :])
        nc.sync.dma_start(xao[m][:64, :], xa[2 * m][1:128:2, :])
        nc.sync.dma_start(xao[m][64:, :], xa[2 * m + 1][1:128:2, :])
    pB4 = tc.alloc_tile_pool(name="pB4", bufs=1)
    maT = [utile(pB4, [128, NE], F32, f"maT{k}") for k in range(8)]
    mbT = [utile(pB4, [128, NE], F32, f"mbT{k}") for k in range(8)]
    transpose_blocks(xae, maT, NE, C)
    transpose_blocks(xao, mbT, NE, C)

    nm_t = psm.tile([128, 4], F32)
    ni_t = psm.tile([128, 4], F32)
    for m in range(4):
        acc = ps_bigA() if m % 2 == 0 else ps_bigB()
        for k in range(KT):
            nc.tensor.matmul(acc[:, :512], maT[k][:, ts(m, 128)], mbT[k][:],
                             start=(k == 0), stop=(k == KT - 1))
        mx8 = utile(psm, [128, 8], F32, "mx8")
        ix8 = utile(psm, [128, 8], U32, "ix8")
        nc.vector.max_with_indices(mx8[:], ix8[:], acc[:, :512])
        nc.vector.tensor_copy(nm_t[:, m:m + 1], mx8[:, 0:1])
        nc.vector.tensor_copy(ni_t[:, m:m + 1], ix8[:, 0:1])

    nm_row = utile(pB4, [1, 512], F32, "nm_row")
    for m in range(4):
        p = ps_av()
        pf = p.rearrange("p a b -> p (a b)")
        nc.tensor.transpose(pf[:1, :128], nm_t[:, m:m + 1], ident[:])
        nc.vector.tensor_copy(nm_row[:, ts(m, 128)], pf[:1, :128])
    NMB = bcast_row(nm_row[:], 512, "nmb", pB4)

    rank_t = psm.tile([128, 4], F32)
    for m in range(4):
        gt = utile(pB4, [128, 512], F32, "rk_gt", bufs=1)
        nc.vector.tensor_scalar(gt[:], NMB[:], nm_t[:, m:m + 1], None, OP.is_gt)
        eq = utile(pB4, [128, 512], F32, "rk_eq", bufs=1)
        nc.vector.tensor_scalar(eq[:], NMB[:], nm_t[:, m:m + 1], None, OP.is_equal)
        flt = utile(pB4, [128, 512], F32, "rk_flt", bufs=1)
        pio = utile(psm, [128, 1], F32, "rk_pio")
        nc.vector.tensor_scalar_add(pio[:], piota[:], float(128 * m))
        nc.vector.tensor_scalar(flt[:], IOTA512B[:], pio[:], None, OP.is_lt)
        nc.vector.tensor_tensor(eq[:], eq[:], flt[:], OP.mult)
        nc.vector.tensor_tensor(gt[:], gt[:], eq[:], OP.add)
        nc.vector.reduce_sum(rank_t[:, m:m + 1], gt[:], axis=mybir.AxisListType.X)
    if dbg:
        for (tt, nme) in [(nm_t, "dbg_nm"), (rank_t, "dbg_rank"),
                          (ni_t, "dbg_nodeidx")]:
            nc.sync.dma_start(dbg[nme].rearrange("(m p) -> p m", p=128), tt[:])
    pB4.release()
    pB3.release()
    pB2.release()

    # ================= dst merge (x1 from DRAM; dstn -> DRAM) =============
    pM = tc.alloc_tile_pool(name="pM", bufs=1)
    x1e = [utile(pM, [128, C + 8], F32R, f"x1e{m}") for m in range(4)]
    x1o = [utile(pM, [128, C], F32, f"x1o{m}") for m in range(4)]
    for m in range(4):
        nc.vector.tensor_copy(x1e[m][:, C:C + 8], ones_part[:])
        nc.sync.dma_start(x1e[m][:, :C],
                          x1d[256 * m:256 * m + 256:2, :].bitcast(F32R))
        nc.sync.dma_start(x1o[m][:], x1d[256 * m + 1:256 * m + 256:2, :])
    st = [utile(pM, [128, 512], F32R, f"st{m}") for m in range(4)]
    for m in range(4):
        msk = utile(psm, [128, 1], F32, "st_m")
        nc.vector.tensor_scalar(msk[:], rank_t[:, m:m + 1], float(R) - 0.5, None,
                                OP.is_lt)
        nc.vector.tensor_scalar(st[m][:], IOTA512B[:], ni_t[:, m:m + 1], None,
                                OP.is_equal)
        nc.vector.tensor_scalar(st[m][:], st[m][:], msk[:], None, OP.mult)
    for m in range(4):
        acc = ps_bigA()
        cacc = ps_av()
        for k in range(4):
            for n2 in range(2):
                nc.tensor.matmul(acc[:, ts(n2, 512)], st[k][:, ts(m, 128)],
                                 x1e[k][:, n2 * 512:n2 * 512 + 512],
                                 start=(k == 0), stop=(k == 3))
            nc.tensor.matmul(cacc[:, 0, :8], st[k][:, ts(m, 128)],
                             x1e[k][:, C:C + 8], start=(k == 0), stop=(k == 3))
        cnt = utile(psm, [128, 1], F32, "cnt")
        nc.vector.tensor_scalar_add(cnt[:], cacc[:, 0, 0:1], 1.0)
        rec = utile(psm, [128, 1], F32, "cntr")
        nc.vector.reciprocal(rec[:], cnt[:])
        dst_stg = utile(pM, [128, C], F32, "dst_stg", bufs=2)
        nc.vector.tensor_tensor(dst_stg[:], acc[:], x1o[m][:], OP.add)
        nc.vector.tensor_scalar(dst_stg[:], dst_stg[:], rec[:], None, OP.mult)
        nc.sync.dma_start(dstnd[ts(m, 128), :], dst_stg[:])

    # ========== MLP (f32r): W1/W2 streamed once; SBUF out accumulation ======
    def row_src_ap(i):
        if i < 4:
            return x1d[256 * i:256 * i + 256:2, :]
        return dstnd[ts(i - 4, 128), :]

    pM.release()
    pZ = tc.alloc_tile_pool(name="pZ", bufs=1)
    pC4 = tc.alloc_tile_pool(name="pC4", bufs=1)
    g2b = brow("g2", pC4)
    be2b = brow("be2", pC4)
    h2 = []
    for i in range(8):
        rsrc = utile(pt, [128, C], F32, "xin")
        nc.sync.dma_start(rsrc[:], row_src_ap(i))
        h = utile(pC4, [128, C], F32, "ht", bufs=2)
        layer_norm(rsrc, h, g2b, be2b, pC4)
        h2.append(h)
        if dbg:
            nc.sync.dma_start(dbg["dbg_mlpin"][ts(i, 128), :], rsrc[:])
    h2T = [utile(pC4, [128, N], F32R, f"h2T{k}") for k in range(8)]
    transpose_blocks(h2, h2T, N, C)

    bm1T = bcol("bm1", pC4)
    bm2T = bcol("bm2", psm)
    # ---- MLP c-major: Y^T groups of 8 hidden tiles, Z^T accum in SBUF ----
    zacc = [utile(pZ, [128, N], F32R, f"zacc{c}") for c in range(8)]
    for g in range(4):
        ytiles = []
        w1h = None
        for mi in range(8):
            mt = g * 8 + mi
            if mi % 4 == 0:
                w1h = []
                for k in range(KT):
                    w = utile(pC4, [128, 512], F32R, "w1h", bufs=8)
                    nc.sync.dma_start(
                        w[:], wd["W1"][ts(k, 128), ts(2 * g + mi // 4, 512)]
                        .bitcast(F32R))
                    w1h.append(w)
            yp = ps_bigA() if mi % 2 == 0 else ps_bigB()
            for k in range(KT):
                for n2 in range(2):
                    nc.tensor.matmul(yp[:, ts(n2, 512)],
                                     w1h[k][:, ts(mi % 4, 128)],
                                     h2T[k][:, ts(n2, 512)],
                                     start=(k == 0), stop=(k == KT - 1))
            yt = utile(pC4, [128, N], F32R, "yt", bufs=8)
            nc.scalar.activation(yt[:], yp[:], AF.Gelu_apprx_tanh,
                                 bias=bm1T[:, mt:mt + 1])
            ytiles.append(yt)
        for ch in range(2):
            w2h = []
            for mi in range(8):
                mt = g * 8 + mi
                w = utile(pC4, [128, 512], F32R, "w2h", bufs=8)
                nc.sync.dma_start(w[:],
                                  wd["W2"][ts(mt, 128), ts(ch, 512)].bitcast(F32R))
                w2h.append(w)
            for cc in range(4):
                c = ch * 4 + cc
                for n2 in range(2):
                    zp = ps_mm() if n2 == 0 else                         ps_av().rearrange("p a b -> p (a b)")
                    for mi in range(8):
                        nc.tensor.matmul(zp[:, :512],
                                         w2h[mi][:, ts(cc, 128)],
                                         ytiles[mi][:, ts(n2, 512)],
                                         start=(mi == 0), stop=(mi == 7))
                    with nc.allow_low_precision(reason="mlp psums f32r"):
                        if g == 0:
                            nc.vector.tensor_scalar(zacc[c][:, ts(n2, 512)],
                                                    zp[:, :512],
                                                    bm2T[:, c:c + 1], None,
                                                    OP.add)
                        else:
                            nc.vector.tensor_tensor(zacc[c][:, ts(n2, 512)],
                                                    zacc[c][:, ts(n2, 512)],
                                                    zp[:, :512], OP.add)

    # ---- residual + transpose -> row-major mod (bf16, pZ-resident) ----
    mod_sb = [utile(pZ, [128, N], BF16, f"modr{i}") for i in range(8)]
    for i in range(8):
        resf = utile(pt, [128, C], F32, "xin")
        nc.sync.dma_start(resf[:], row_src_ap(i))
        zt = ps_bigA() if i % 2 == 0 else ps_bigB()
        for c in range(8):
            nc.tensor.transpose(zt[:, ts(c, 128)].bitcast(F32R),
                                zacc[c][:, ts(i, 128)], ident_r[:, :])
        with nc.allow_low_precision(reason="mod rows stored bf16"):
            nc.vector.tensor_tensor(mod_sb[i][:], zt[:], resf[:], OP.add)
    if dbg:
        for i in range(8):
            mff = utile(pC4, [128, C], F32, "mof", bufs=2)
            nc.vector.tensor_copy(mff[:], mod_sb[i][:])
            nc.sync.dma_start(dbg["dbg_mlpout"][ts(i, 128), :], mff[:])
    pC4.release()

    # ================= Stage D: pooling + Wp -> combined^T =================
    pD = tc.alloc_tile_pool(name="pD", bufs=1)
    # ApT[p, f] = 0.5 iff source row p pools into output f:
    #   even block: base = rank[p]-16, match iff (2f - base) in {-1, 0}
    #   dst  block: base = d,          match iff (2(f-248) - base) in {-1, 0}
    iota2e = utile(pD, [128, 504], F32, "iota2e")
    nc.vector.tensor_scalar_mul(iota2e[:], IOTA504B[:], 2.0)
    apT = [utile(pD, [128, 504], BF16, f"apT{m}") for m in range(8)]
    for m in range(8):
        base = utile(psm, [128, 1], F32, "ap_r")
        if m < 4:
            nc.vector.tensor_scalar_add(base[:], rank_t[:, m:m + 1], -float(R))
        else:
            nc.vector.tensor_scalar_add(base[:], piota[:],
                                        float(128 * (m - 4) + NE - R))
        d1 = utile(pD, [128, 504], F32, "ap_d1")
        nc.vector.tensor_scalar(d1[:], iota2e[:], base[:], None, OP.subtract)
        a1 = utile(pD, [128, 504], F32, "ap_a1")
        nc.vector.tensor_scalar(a1[:], d1[:], -1.5, None, OP.is_ge)
        b1 = utile(pD, [128, 504], F32, "ap_b1")
        nc.vector.tensor_scalar(b1[:], d1[:], 0.5, None, OP.is_le)
        nc.vector.scalar_tensor_tensor(apT[m][:], a1[:], 0.5, b1[:],
                                       OP.mult, OP.mult)
    # pooledT[c][128c, 504] = sum_i mod_sb[i][:, c-tile]^T @ apT[i]
    pooledT = [utile(pD, [128, NP], F32R, f"pooledT{k}") for k in range(8)]
    for c in range(8):
        zp = ps_mm() if c % 2 == 0 else         ps_av().rearrange("p a b -> p (a b)")
        for i in range(8):
            nc.tensor.matmul(zp[:, :NP], mod_sb[i][:, ts(c, 128)], apT[i][:],
                             start=(i == 0), stop=(i == 7))
        nc.vector.tensor_copy(pooledT[c][:], zp[:, :NP])
    if dbg:
        for m in range(4):
            pst = utile(pt, [128, C], F32, "xin")
            for bj in range(8):
                p = ps_av()
                pf = p.rearrange("p a b -> p (a b)")
                nc.tensor.transpose(pf[:PP, :128].bitcast(F32R),
                                    pooledT[bj][:, m * PP:(m + 1) * PP],
                                    ident_r[:, :])
                nc.vector.tensor_copy(pst[:PP, ts(bj, 128)], pf[:PP, :128])
            nc.sync.dma_start(dbg["dbg_pooled"][ts(m, PP), :], pst[:PP, :])

    pE = tc.alloc_tile_pool(name="pE", bufs=1)
    bp3T = bcol("bp", pD, scale=3.0)
    cmbTr = [utile(pD, [128, NP], F32R, f"cmbTr{m}") for m in range(8)]
    for mg in range(2):
        wcs = []
        for k in range(KT):
            wcr = utile(pD, [128, 512], F32R, f"wpc{k}", bufs=1)
            nc.sync.dma_start(wcr[:], wd["Wp"][ts(k, 128), ts(mg, 512)].bitcast(F32R))
            wcs.append(wcr)
        for mi in range(4):
            m = mg * 4 + mi
            acc = ps_mm()
            for k in range(KT):
                nc.tensor.matmul(acc[:, :NP], wcs[k][:, ts(mi, 128)],
                                 pooledT[k][:], start=(k == 0), stop=(k == KT - 1))
            nc.scalar.activation(cmbTr[m][:], acc[:, :NP], AF.Identity,
                                 bias=bp3T[:, m:m + 1], scale=3.0)

    # ================= Stage E: MQA =================
    bqT = bcol("bq", pE)

    _wq512 = {}

    def make_mqT(m):
        mg = m // 4
        if mg not in _wq512:
            ws = []
            for k in range(KT):
                w = utile(pE, [128, 512], F32R, "wq512", bufs=8)
                nc.sync.dma_start(
                    w[:], wd["Wq"][ts(k, 128), ts(mg, 512)].bitcast(F32R))
                ws.append(w)
            _wq512.clear()
            _wq512[mg] = ws
        ws = _wq512[mg]
        acc = ps_mm()
        for k in range(KT):
            nc.tensor.matmul(acc[:, :NP], ws[k][:, ts(m % 4, 128)], cmbTr[k][:],
                             start=(k == 0), stop=(k == KT - 1))
        t = utile(pE, [128, NP], F32R, "mqT", bufs=2)
        with nc.allow_low_precision(reason="bias add, f32r storage"):
            nc.vector.tensor_scalar(t[:], acc[:, :NP], bqT[:, m:m + 1], None,
                                    OP.add)
        return t

    wkvr = utile(pE, [128, KT, 2 * DH], F32R, "wkvr")
    for k in range(KT):
        nc.sync.dma_start(wkvr[:, k, :DH], wd["Wk"][ts(k, 128), :].bitcast(F32R))
        nc.sync.dma_start(wkvr[:, k, DH:], wd["Wv"][ts(k, 128), :].bitcast(F32R))
    bkT = utile(pE, [64, 1], F32, "bkT")
    nc.sync.dma_start(bkT[:], wd["bk"][:, None])
    mkT = utile(pE, [128, NP], F32R, "mkT")
    macc = ps_mm()
    for k in range(KT):
        nc.tensor.matmul(macc[:64, :NP], wkvr[:, k, :DH], cmbTr[k][:],
                         start=(k == 0), stop=(k == KT - 1))
    mkf = utile(pE, [64, NP], F32, "mkf")
    nc.scalar.activation(mkf[:], macc[:64, :NP], AF.Identity, bias=bkT[:])
    nc.vector.tensor_copy(mkT[:64, :], mkf[:])
    nc.sync.dma_start(mkT[64:, :], mkT[:64, :])

    bvb = bcast_row(load_row(wd["bv"], DH, "bv_r", pE), DH, "bv_b", pE)
    mv_pad = [utile(pE, [128, DH + 1], F32R, f"mvp{m}") for m in range(4)]
    for m in range(4):
        acc = ps_av()
        for k in range(KT):
            nc.tensor.matmul(acc[:PP, 0, :DH], cmbTr[k][:, m * PP:(m + 1) * PP],
                             wkvr[:, k, DH:], start=(k == 0), stop=(k == KT - 1))
        nc.vector.tensor_copy(mv_pad[m][:, DH:], ones_part[:, :1])
        nc.vector.tensor_tensor(mv_pad[m][:PP, :DH], acc[:PP, 0, :DH], bvb[:PP, :],
                                OP.add)

    mqT_cur = None
    for h in range(H):
        po = (h % 2) * 64
        if h % 2 == 0:
            mqT_cur = make_mqT(h // 2)
        mqT_h = mqT_cur[po:po + 64, :]
        spA = ps_bigA()
        spB = ps_bigB()
        ep = []
        for mm in range(4):
            spbig = spA if mm < 2 else spB
            sp = spbig[:, 512 * (mm % 2):512 * (mm % 2) + NP]
            nc.tensor.matmul(sp[:PP, :NP], mkT[po:po + 64, mm * PP:(mm + 1) * PP],
                             mqT_h[:], start=True, stop=True)
            et = utile(pE, [128, NP], F32R, "e2", bufs=6)
            nc.scalar.activation(et[:PP, :], sp[:PP, :NP], AF.Exp,
                                 scale=float(DH ** -0.5))
            ep.append(et)
        av2 = ps_av().rearrange("p a b -> p (a b)")
        for mm in range(4):
            nc.tensor.matmul(av2[:DH + 1, :NP], mv_pad[mm][:PP, :],
                             ep[mm][:PP, :], start=(mm == 0), stop=(mm == 3))
        rrow = utile(pE, [1, NP], F32R, "rrow2", bufs=2)
        with nc.allow_low_precision(reason="softmax denom; 11-bit ok downstream"):
            nc.vector.reciprocal(rrow[:], av2[DH:DH + 1, :NP])
        rb = ps_mm()
        nc.tensor.matmul(rb[:DH, :NP], ones_col_r[:, :DH], rrow[:],
                         start=True, stop=True)
        rbs = utile(pE, [64, NP], F32, "rbs2", bufs=2)
        nc.vector.tensor_copy(rbs[:], rb[:DH, :NP])
        stg = utile(pE, [64, NP], F32R, "mqstg", bufs=2)
        nc.vector.tensor_tensor(stg[:], av2[:DH, :NP], rbs[:], OP.mult)
        nc.sync.dma_start(mqaTd[h * 64:h * 64 + 64, :], stg[:])
    pE.release()
    pD.release()
    pZ.release()

    # ================= Stage F: Wmo + FFN =================
    pF = tc.alloc_tile_pool(name="pF", bufs=1)
    pF1 = tc.alloc_tile_pool(name="pF1", bufs=1)
    mqaT = [utile(pF1, [128, NP], F32R, f"mqaT{k}") for k in range(8)]
    for k in range(8):
        nc.sync.dma_start(mqaT[k][:, :NP], mqaTd[ts(k, 128), :])
    bmoT = bcol("bmo", pF1)
    omoT = [utile(pF, [128, NP], F32R, f"omoT{m}") for m in range(8)]
    for mg in range(2):
        wcs = []
        for k in range(KT):
            wcr = utile(pF1, [128, 512], F32R, f"wmc{k}", bufs=1)
            nc.sync.dma_start(wcr[:],
                              wd["Wmo"][ts(k, 128), ts(mg, 512)].bitcast(F32R))
            wcs.append(wcr)
        for mi in range(4):
            m = mg * 4 + mi
            acc = ps_mm()
            for k in range(KT):
                nc.tensor.matmul(acc[:, :NP], wcs[k][:, ts(mi, 128)],
                                 mqaT[k][:], start=(k == 0), stop=(k == KT - 1))
            nc.scalar.activation(omoT[m][:], acc[:, :NP], AF.Identity,
                                 bias=bmoT[:, m:m + 1])
    pF1.release()

    # ---- FFN c-major: Y halves of 16 hidden tiles, Z accum in SBUF ----
    bf1T = bcol("bf1", pF)
    bf2b = brow("bf2", pF)
    zf = [utile(pF, [128, NP], F32R, f"zf{c}") for c in range(8)]
    for hf in range(2):
        yfs = []
        wf1h = None
        for kki in range(16):
            kk = hf * 16 + kki
            if kki % 4 == 0:
                wf1h = []
                for k in range(KT):
                    w = utile(pF, [128, 512], F32R, "wf1h", bufs=9)
                    nc.sync.dma_start(
                        w[:], wd["Wf1"][ts(k, 128), ts(kk // 4, 512)]
                        .bitcast(F32R))
                    wf1h.append(w)
            yp = ps_mm() if kki % 2 == 0 else             ps_av().rearrange("p a b -> p (a b)")
            for k in range(KT):
                nc.tensor.matmul(yp[:, :NP], wf1h[k][:, ts(kki % 4, 128)],
                                 omoT[k][:], start=(k == 0), stop=(k == KT - 1))
            yf = utile(pF, [128, NP], F32R, "yf", bufs=17)
            nc.scalar.activation(yf[:], yp[:, :NP], AF.Silu,
                                 bias=bf1T[:, kk:kk + 1])
            yfs.append(yf)
        for ch in range(2):
            wf2h = []
            for kki in range(16):
                kk = hf * 16 + kki
                w = utile(pF, [128, 512], F32R, "wf2h", bufs=17)
                nc.sync.dma_start(w[:],
                                  wd["Wf2"][ts(kk, 128), ts(ch, 512)].bitcast(F32R))
                wf2h.append(w)
            for cc in range(4):
                c = ch * 4 + cc
                zp = ps_bigA() if cc % 2 == 0 else ps_bigB()
                for kki in range(16):
                    nc.tensor.matmul(zp[:, :NP], wf2h[kki][:, ts(cc, 128)],
                                     yfs[kki][:], start=(kki == 0),
                                     stop=(kki == 15))
                with nc.allow_low_precision(reason="ffn out partials f32r"):
                    if hf == 0:
                        nc.vector.tensor_copy(zf[c][:], zp[:, :NP])
                    else:
                        nc.vector.tensor_tensor(zf[c][:], zf[c][:], zp[:, :NP],
                                                OP.add)
    # ---- transpose back to row-major + bias -> out ----
    for tl in range(4):
        of = utile(pF, [128, C], F32, "of", bufs=2)
        for c in range(8):
            p = ps_av()
            pf = p.rearrange("p a b -> p (a b)")
            nc.tensor.transpose(pf[:PP, :128].bitcast(F32R),
                                zf[c][:, tl * PP:(tl + 1) * PP], ident_r[:, :])
            nc.vector.tensor_tensor(of[:PP, ts(c, 128)], pf[:PP, :128],
                                    bf2b[:PP, ts(c, 128)], OP.add)
        nc.sync.dma_start(out_d[tl * PP:(tl + 1) * PP, :], of[:PP, :])
    pF.release()
    for pool in (pt, pw, psm, pc, pp):
        pool.release()


_BUILT = None


def kernel(**inputs):
    global _BUILT
    if _BUILT is None:
        _BUILT = build(debug=DEBUG)
    nc = _BUILT
    x = np.ascontiguousarray(inputs["x"], dtype=np.float32)
    base = {k: np.ascontiguousarray(v, dtype=np.float32) for k, v in inputs.items()
            if k != "x"}
    in_maps = []
    for i in range(8):
        m = dict(base)
        m["x"] = x[i]
        in_maps.append(m)
    res = run_bass_kernel_spmd(nc, in_maps, core_ids=list(range(8)))
    out = np.stack([res.results[i]["out"] for i in range(8)], axis=0)
    return out.astype(np.float32)



# revision 69
# speedup vs baseline: 1.0539x; 1.0014x over previous
"""AdaptiveTokenMerger (ToMe block + merger) TRN2 Bass kernel.

Data-parallel over batch: 8 samples -> 8 NeuronCores, one sample per core.
Per-core pipeline (sample x [1024, 1024]):
  A (f32, ranking-critical): LN1 -> qkv -> MHA (transposed-softmax with the
    denominator folded in as an appended ones-column of v) -> Wo -> x_attn
  B: metric scores -> node_max/argmax -> ranks via pairwise comparisons ->
    dst scatter-add expressed as a one-hot matmul
  C (f32r): MLP over rows [x1_even(512); dst_new(512)], fused W1/W2 per
    token-quarter, output accumulated in PSUM across all 32 W1 column tiles
  D: pooling as a rank-dependent one-hot matmul -> Wp -> combined = 3q
  E (f32r): multi-query attention  F (f32r): FFN -> out [504, 1024]

Precision: everything upstream of the rank/argmax decisions is true fp32
(4 cyc/row on PE); post-merge matmuls use float32r (TF32-ish, 1 cyc/row).

PSUM budget (8 banks): BIGA/BIGB [128,1024] (2+2), MM [128,512] x2 (2),
AV [128,4,128] x2 (2).
"""
import numpy as np

import concourse.bass as bass
import concourse.tile as tile
from concourse import bacc, mybir
from concourse.bass import ts
from concourse.bass_utils import run_bass_kernel_spmd
from concourse.masks import make_identity

F32 = mybir.dt.float32
F32R = mybir.dt.float32r
FP16 = mybir.dt.float16
BF16 = mybir.dt.bfloat16
U32 = mybir.dt.uint32

N, C, H = 1024, 1024, 16
R = 16
DH = C // H          # 64
NE = N // 2          # 512
NP = (N - R) // 2    # 504
PP = 126             # pooled tokens per partition tile
KT = C // 128        # 8
AF = mybir.ActivationFunctionType
OP = mybir.AluOpType

DEBUG = False


def build(debug=False):
    nc = bacc.Bacc("TRN2", target_bir_lowering=False, debug=False, num_devices=8)
    x_d = nc.dram_tensor("x", [N, C], F32, kind="ExternalInput").ap()
    wd = {}
    for name, shape in [
        ("g1", [C]), ("be1", [C]), ("Wqkv", [C, 3 * C]), ("bqkv", [3 * C]),
        ("Wo", [C, C]), ("bo", [C]), ("g2", [C]), ("be2", [C]),
        ("W1", [C, 4 * C]), ("bm1", [4 * C]), ("W2", [4 * C, C]), ("bm2", [C]),
        ("Wp", [C, C]), ("bp", [C]), ("Wq", [C, C]), ("bq", [C]),
        ("Wk", [C, DH]), ("bk", [DH]), ("Wv", [C, DH]), ("bv", [DH]),
        ("Wmo", [C, C]), ("bmo", [C]), ("Wf1", [C, 4 * C]), ("bf1", [4 * C]),
        ("Wf2", [4 * C, C]), ("bf2", [C]),
    ]:
        wd[name] = nc.dram_tensor(name, shape, F32, kind="ExternalInput").ap()
    out_d = nc.dram_tensor("out", [NP, C], F32, kind="ExternalOutput").ap()
    dbg = {}
    if debug:
        for name, shape in [
            ("dbg_xattn", [N, C]), ("dbg_nm", [NE]), ("dbg_rank", [NE]),
            ("dbg_nodeidx", [NE]), ("dbg_mlpin", [N, C]), ("dbg_mlpout", [N, C]),
            ("dbg_pooled", [NP, C]),
        ]:
            dbg[name] = nc.dram_tensor(name, shape, F32, kind="ExternalOutput").ap()
    with tile.TileContext(nc) as tc:
        _build_tile(nc, tc, x_d, wd, out_d, dbg)
    nc.compile()
    return nc


def _build_tile(nc, tc, x_d, wd, out_d, dbg):
    # DRAM spill buffers
    qkTd = nc.dram_tensor("qkTd", [2 * C, N], F32).ap()
    aoTd = nc.dram_tensor("aoTd", [C, N], F32).ap()
    x1d = nc.dram_tensor("x1d", [N, C], F32).ap()
    dstnd = nc.dram_tensor("dstnd", [NE, C], F32).ap()
    mqaTd = nc.dram_tensor("mqaTd", [C, NP], F32R).ap()

    pc = tc.alloc_tile_pool(name="const", bufs=1)
    psm = tc.alloc_tile_pool(name="small", bufs=1)
    pw = tc.alloc_tile_pool(name="wstream", bufs=2)
    pt = tc.alloc_tile_pool(name="tmp", bufs=2)
    pp = tc.alloc_tile_pool(name="psum", bufs=1, space="PSUM")

    _ct = {}

    def utile(pool, shape, dtype, tag, bufs=None):
        _ct[tag] = _ct.get(tag, 0) + 1
        kw = {"bufs": bufs} if bufs is not None else {}
        return pool.tile(shape, dtype, tag=tag, name=f"{tag}_{_ct[tag]}", **kw)

    def ps_bigA():
        return utile(pp, [128, 1024], F32, "BIGA")

    def ps_bigB():
        return utile(pp, [128, 1024], F32, "BIGB")

    def ps_mm():
        return utile(pp, [128, 512], F32, "MM", bufs=2)

    def ps_av():
        return utile(pp, [128, 4, 128], F32, "AV", bufs=2)

    # ---------- constants ----------
    ident = pc.tile([128, 128], F32)
    make_identity(nc, ident[:])
    ident_r = pc.tile([128, 128], F32R)
    nc.vector.tensor_copy(ident_r[:], ident[:])
    ones_col = pc.tile([1, 128], F32)
    nc.gpsimd.memset(ones_col[:], 1.0)
    ones_col_r = pc.tile([1, 128], F32R)
    nc.vector.tensor_copy(ones_col_r[:], ones_col[:])
    ones_part = pc.tile([128, 8], F32)
    nc.gpsimd.memset(ones_part[:], 1.0)
    piota = pc.tile([128, 1], F32)
    nc.gpsimd.iota(piota[:], [[0, 1]], channel_multiplier=1,
                   allow_small_or_imprecise_dtypes=True)
    iota512_row = pc.tile([1, 512], F32)
    nc.gpsimd.iota(iota512_row[:], [[1, 512]], channel_multiplier=0,
                   allow_small_or_imprecise_dtypes=True)
    iota504_row = pc.tile([1, 504], F32)
    nc.gpsimd.iota(iota504_row[:], [[1, 504]], channel_multiplier=0,
                   allow_small_or_imprecise_dtypes=True)

    def bcast_row(row_ap, n, tag, pool, scale=1.0):
        t = utile(pool, [128, n], F32, tag)
        for c0 in range(0, n, 512):
            cw = min(512, n - c0)
            p = ps_mm()
            nc.tensor.matmul(p[:, :cw], ones_col[:], row_ap[:, c0:c0 + cw],
                             start=True, stop=True)
            if scale == 1.0:
                nc.vector.tensor_copy(t[:, c0:c0 + cw], p[:, :cw])
            else:
                nc.vector.tensor_scalar_mul(t[:, c0:c0 + cw], p[:, :cw], scale)
        return t

    def load_row(dram_ap, n, tag, pool):
        t = utile(pw, [1, n], F32, "rowstg", bufs=1)
        nc.sync.dma_start(t[:], dram_ap[None, :])
        return t

    def brow(name, pool, scale=1.0):
        n = wd[name].shape[0]
        return bcast_row(load_row(wd[name], n, name + "_r", pool), n,
                         name + "_b", pool, scale)

    def bcol(name, pool, scale=1.0):
        n = wd[name].shape[0]
        t = utile(pool, [128, n // 128], F32, name + "_c")
        nc.sync.dma_start(t[:], wd[name].rearrange("(t p) -> p t", p=128))
        if scale != 1.0:
            nc.vector.tensor_scalar_mul(t[:], t[:], scale)
        return t

    IOTA512B = bcast_row(iota512_row[:], 512, "iota512b", pc)
    IOTA504B = bcast_row(iota504_row[:], 504, "iota504b", pc)

    def transpose_blocks(src_tiles, dst, n_rows, n_cols):
        """dst[c, r] = src[r, c]; dst is tile-list or sink(bj, bi, pf, cw, rw)."""
        for bi in range((n_rows + 127) // 128):
            rw = min(128, n_rows - bi * 128)
            for bj in range((n_cols + 127) // 128):
                cw = min(128, n_cols - bj * 128)
                p = ps_av()
                pf = p.rearrange("p a b -> p (a b)")
                nc.tensor.transpose(pf[:cw, :rw],
                                    src_tiles[bi][:rw, bj * 128:bj * 128 + cw],
                                    ident[:rw, :rw])
                if callable(dst):
                    dst(bj, bi, pf, cw, rw)
                else:
                    nc.vector.tensor_copy(dst[bj][:cw, bi * 128:bi * 128 + rw],
                                          pf[:cw, :rw])

    def refined_rsqrt_recip(vv, tag):
        """returns 1/sqrt(vv) with one Newton step on sqrt (ACT sqrt is loose)."""
        s0 = utile(psm, [128, 1], F32, tag + "_s0")
        nc.scalar.sqrt(s0[:], vv[:])
        r0 = utile(psm, [128, 1], F32, tag + "_r0")
        nc.vector.reciprocal(r0[:], s0[:])
        t = utile(psm, [128, 1], F32, tag + "_t")
        nc.vector.tensor_tensor(t[:], vv[:], r0[:], OP.mult)
        nc.vector.tensor_tensor(t[:], t[:], s0[:], OP.add)
        nc.vector.tensor_scalar_mul(t[:], t[:], 0.5)
        rr = utile(psm, [128, 1], F32, tag + "_rr")
        nc.vector.reciprocal(rr[:], t[:])
        return rr

    def layer_norm(src, dst, gb, bb, lnpool):
        m = utile(psm, [128, 1], F32, "ln_m")
        nc.vector.reduce_sum(m[:], src[:, :C], axis=mybir.AxisListType.X)
        nc.vector.tensor_scalar_mul(m[:], m[:], 1.0 / C)
        xc = utile(lnpool, [128, C], F32, "ln_xc", bufs=1)
        nc.vector.tensor_scalar(xc[:], src[:, :C], m[:], None, OP.subtract)
        ss = utile(psm, [128, 1], F32, "ln_ss")
        nc.scalar.activation(dst[:, :C], xc[:], AF.Square, accum_out=ss[:])
        v = utile(psm, [128, 1], F32, "ln_v")
        nc.vector.tensor_scalar(v[:], ss[:], 1.0 / C, 1e-5, OP.mult, OP.add)
        rstd = refined_rsqrt_recip(v, "ln")
        nc.vector.tensor_scalar(dst[:, :C], xc[:], rstd[:], None, OP.mult)
        nc.vector.tensor_tensor(dst[:, :C], dst[:, :C], gb[:], OP.mult)
        nc.vector.tensor_tensor(dst[:, :C], dst[:, :C], bb[:], OP.add)

    # ================= Stage A: LN1 -> hT =================
    pbA = tc.alloc_tile_pool(name="biasA", bufs=1)
    pHT = tc.alloc_tile_pool(name="pHT", bufs=1)

    g1b = brow("g1", pbA)
    be1b = brow("be1", pbA)
    hT = [utile(pHT, [128, N], F32, f"hT{k}") for k in range(8)]
    ht = []
    for i in range(8):
        xt = utile(pt, [128, C], F32, "xin")
        nc.sync.dma_start(xt[:], x_d[ts(i, 128), :])
        h = utile(pHT, [128, C], F32, "ht", bufs=2)
        layer_norm(xt, h, g1b, be1b, pbA)
        ht.append(h)
    transpose_blocks(ht, hT, N, C)

    def split16(dst_hi, dst_lo, src_ap):
        nc.scalar.activation(dst_hi, src_ap, AF.Identity)
        with nc.allow_low_precision(reason="fp16 split residual"):
            nc.vector.tensor_tensor(dst_lo, src_ap, dst_hi, OP.subtract)

    def split16s(dst_hi, dst_lo, src_ap):
        # scale by 64 so the lo half stays in fp16 normal range (w ~ 0.02)
        nc.scalar.activation(dst_hi, src_ap, AF.Identity, scale=64.0)
        with nc.allow_low_precision(reason="fp16 split residual"):
            nc.vector.scalar_tensor_tensor(dst_lo, src_ap, 64.0, dst_hi,
                                           OP.mult, OP.subtract)

    # ===== qk^T -> qkTd (DRAM) ; v_pad (SBUF) =====
    pQK = tc.alloc_tile_pool(name="pQK", bufs=1)
    bqkT = bcol("bqkv", pbA)
    hThi = [utile(pQK, [128, N], FP16, f"hThi{k}") for k in range(8)]
    hTlo = [utile(pQK, [128, N], FP16, f"hTlo{k}") for k in range(8)]
    for k in range(8):
        split16(hThi[k][:], hTlo[k][:], hT[k][:])
    for mpg in range(2):
        wqh, wql, wkh, wkl = [], [], [], []
        for k in range(KT):
            wqf = utile(pw, [128, 512], F32, "wqk512", bufs=2)
            nc.sync.dma_start(wqf[:], wd["Wqkv"][ts(k, 128), ts(mpg, 512)])
            hi = utile(pQK, [128, 512], FP16, "wqh", bufs=8)
            lo = utile(pQK, [128, 512], FP16, "wql", bufs=8)
            split16(hi[:], lo[:], wqf[:])
            wqh.append(hi)
            wql.append(lo)
            wkf = utile(pw, [128, 512], F32, "wqk512", bufs=2)
            nc.sync.dma_start(wkf[:],
                              wd["Wqkv"][ts(k, 128), C + mpg * 512:C + mpg * 512 + 512])
            hi = utile(pQK, [128, 512], FP16, "wkh", bufs=8)
            lo = utile(pQK, [128, 512], FP16, "wkl", bufs=8)
            split16(hi[:], lo[:], wkf[:])
            wkh.append(hi)
            wkl.append(lo)
        for mi in range(4):
            mp = mpg * 4 + mi
            accq = ps_bigA()
            acck = ps_bigB()
            for k in range(KT):
                for n2 in range(2):
                    for wgt, mov in ((wqh[k], hThi[k]), (wqh[k], hTlo[k]),
                                     (wql[k], hThi[k])):
                        nc.tensor.matmul(accq[:, ts(n2, 512)],
                                         wgt[:, ts(mi, 128)],
                                         mov[:, ts(n2, 512)],
                                         start=(k == 0 and wgt is wqh[k]
                                                and mov is hThi[k]),
                                         stop=(k == KT - 1 and wgt is wql[k]))
                    for wgt, mov in ((wkh[k], hThi[k]), (wkh[k], hTlo[k]),
                                     (wkl[k], hThi[k])):
                        nc.tensor.matmul(acck[:, ts(n2, 512)],
                                         wgt[:, ts(mi, 128)],
                                         mov[:, ts(n2, 512)],
                                         start=(k == 0 and wgt is wkh[k]
                                                and mov is hThi[k]),
                                         stop=(k == KT - 1 and wgt is wkl[k]))
            stgq = utile(pQK, [128, N], F32, "qkstg", bufs=2)
            nc.scalar.activation(stgq[:], accq[:], AF.Identity,
                                 bias=bqkT[:, mp:mp + 1])
            nc.sync.dma_start(qkTd[ts(mp, 128), :], stgq[:])
            stgk = utile(pQK, [128, N], F32, "qkstg", bufs=2)
            nc.scalar.activation(stgk[:], acck[:], AF.Identity,
                                 bias=bqkT[:, 8 + mp:9 + mp])
            nc.sync.dma_start(qkTd[C + mp * 128:C + (mp + 1) * 128, :], stgk[:])

    pQK.release()
    pVP = tc.alloc_tile_pool(name="pVP", bufs=1)
    bvqkvb = bcast_row(load_row(wd["bqkv"][2 * C:], C, "bvq_r", pbA), C,
                       "bvq_b", pbA)
    v_pad = [utile(pVP, [128, H, DH + 1], F32, f"vp{j}") for j in range(8)]
    for j in range(8):
        nc.vector.memset(v_pad[j][:, :, DH:DH + 1], 1.0)
    for jg in range(4):
        accs = [ps_bigA(), ps_bigB()]
        for k in range(KT):
            wv = utile(pVP, [128, C], F32, "wv", bufs=2)
            nc.sync.dma_start(wv[:], wd["Wqkv"][ts(k, 128), 2 * C:])
            for jj in range(2):
                j = jg * 2 + jj
                for n2 in range(2):
                    nc.tensor.matmul(accs[jj][:, ts(n2, 512)],
                                     hT[k][:, ts(j, 128)], wv[:, ts(n2, 512)],
                                     start=(k == 0), stop=(k == KT - 1))
        for jj in range(2):
            j = jg * 2 + jj
            for h in range(H):
                nc.vector.tensor_tensor(v_pad[j][:, h, :DH],
                                        accs[jj][:, ts(h, DH)],
                                        bvqkvb[:, ts(h, DH)], OP.add)

    # ===== attention: stream kT/qT per head; out -> aoTd (already c-major) ==
    # out[dh|sum, i] = v_pad[j].T @ expT[j, i], accumulated over j-tiles.
    pAttn = tc.alloc_tile_pool(name="pAttn", bufs=1)
    for h in range(H):
        kth = utile(pAttn, [64, N], F32, "kth", bufs=2)
        nc.sync.dma_start(kth[:], qkTd[C + h * 64:C + h * 64 + 64, :])
        qth = utile(pAttn, [64, N], F32, "qth", bufs=2)
        nc.sync.dma_start(qth[:], qkTd[h * 64:h * 64 + 64, :])
        khi = utile(pAttn, [64, N], FP16, "khi", bufs=2)
        klo = utile(pAttn, [64, N], FP16, "klo", bufs=2)
        split16(khi[:], klo[:], kth[:])
        qhi = utile(pAttn, [64, N], FP16, "qhi", bufs=2)
        qlo = utile(pAttn, [64, N], FP16, "qlo", bufs=2)
        split16(qhi[:], qlo[:], qth[:])
        av = [ps_av().rearrange("p a b -> p (a b)") for _ in range(2)]
        for j in range(8):
            for n2 in range(2):
                sp = ps_mm()
                for sta, mov in ((khi, qhi), (khi, qlo), (klo, qhi)):
                    nc.tensor.matmul(sp[:], sta[:, ts(j, 128)],
                                     mov[:, ts(n2, 512)],
                                     start=(sta is khi and mov is qhi),
                                     stop=(sta is klo))
                et = utile(pAttn, [128, 512], F32, "exp", bufs=3)
                nc.scalar.activation(et[:], sp[:], AF.Exp, scale=float(DH ** -0.5))
                nc.tensor.matmul(av[n2][:DH + 1, :512], v_pad[j][:, h, :], et[:],
                                 start=(j == 0), stop=(j == 7))
        for n2 in range(2):
            rrow = utile(pAttn, [1, 512], F32, "rrow", bufs=2)
            nc.vector.reciprocal(rrow[:], av[n2][DH:DH + 1, :512])
            rb = ps_mm()
            nc.tensor.matmul(rb[:DH, :512], ones_col[:, :DH], rrow[:],
                             start=True, stop=True)
            rbs = utile(pAttn, [64, 512], F32, "rbs", bufs=2)
            nc.vector.tensor_copy(rbs[:], rb[:DH, :512])
            stg = utile(pAttn, [64, 512], F32, "aot_stg", bufs=2)
            nc.vector.tensor_tensor(stg[:], av[n2][:DH, :512], rbs[:],
                                    OP.mult)
            nc.sync.dma_start(aoTd[h * 64:h * 64 + 64, ts(n2, 512)], stg[:])
    pAttn.release()
    pVP.release()
    pHT.release()
    pbA.release()

    # ================= Wo -> x_attn, x1 (-> DRAM), metric =================
    pB2 = tc.alloc_tile_pool(name="pB2", bufs=1)
    bob = brow("bo", pB2)
    xa = [utile(pB2, [128, C], F32, f"xa{m}") for m in range(8)]
    woh = [utile(pB2, [128, C], FP16, f"woh{k}") for k in range(8)]
    wol = [utile(pB2, [128, C], FP16, f"wol{k}") for k in range(8)]
    for k in range(KT):
        woR = utile(pw, [128, C], F32, "x1stg", bufs=1)
        nc.sync.dma_start(woR[:], wd["Wo"][ts(k, 128), :])
        split16(woh[k][:], wol[k][:], woR[:])
    rn = psm.tile([128, 8], F32)
    for m in range(8):
        acc = ps_bigA()
        for k in range(KT):
            ao = utile(pw, [128, 128], F32, "wqkb", bufs=4)
            nc.sync.dma_start(ao[:], aoTd[ts(k, 128), ts(m, 128)])
            aoh = utile(pw, [128, 128], FP16, "aoh", bufs=2)
            aol = utile(pw, [128, 128], FP16, "aol", bufs=2)
            split16(aoh[:], aol[:], ao[:])
            for n2 in range(2):
                for sta, mov in ((aoh, woh[k]), (aoh, wol[k]), (aol, woh[k])):
                    nc.tensor.matmul(acc[:, ts(n2, 512)], sta[:],
                                     mov[:, ts(n2, 512)],
                                     start=(k == 0 and sta is aoh
                                            and mov is woh[k]),
                                     stop=(k == KT - 1 and sta is aol))
        nc.vector.tensor_tensor(xa[m][:], acc[:], bob[:], OP.add)
        xt = utile(pt, [128, C], F32, "xin")
        nc.sync.dma_start(xt[:], x_d[ts(m, 128), :])
        x1stg = utile(pw, [128, C], F32, "x1stg", bufs=1)
        nc.vector.tensor_tensor(x1stg[:], xa[m][:], xt[:], OP.add)
        nc.sync.dma_start(x1d[ts(m, 128), :], x1stg[:])
        ss = utile(psm, [128, 1], F32, "nrm_ss")
        sq = utile(pt, [128, C], F32, "ln_xc")
        nc.scalar.activation(sq[:], xa[m][:], AF.Square, accum_out=ss[:])
        rr = refined_rsqrt_recip(ss, "nrm")
        nc.vector.tensor_copy(rn[:, m:m + 1], rr[:])
        nc.vector.tensor_scalar(xa[m][:], xa[m][:], rn[:, m:m + 1], None, OP.mult)
        if dbg:
            nc.sync.dma_start(dbg["dbg_xattn"][ts(m, 128), :], xa[m][:])

    # ===== de-interleave metric -> maT/mbT; scores; node stats; ranks =====
    pB3 = tc.alloc_tile_pool(name="pB3", bufs=1)
    xae = [utile(pB3, [128, C], F32, f"xae{m}") for m in range(4)]
    xao = [utile(pB3, [128, C], F32, f"xao{m}") for m in range(4)]
    for m in range(4):
        nc.sync.dma_start(xae[m][:64, :], xa[2 * m][0:128:2, :])
        nc.sync.dma_start(xae[m][64:, :], xa[2 * m + 1][0:128:2, :])
        nc.sync.dma_start(xao[m][:64, :], xa[2 * m][1:128:2, :])
        nc.sync.dma_start(xao[m][64:, :], xa[2 * m + 1][1:128:2, :])
    pB4 = tc.alloc_tile_pool(name="pB4", bufs=1)
    maT = [utile(pB4, [128, NE], F32, f"maT{k}") for k in range(8)]
    mbT = [utile(pB4, [128, NE], F32, f"mbT{k}") for k in range(8)]
    transpose_blocks(xae, maT, NE, C)
    transpose_blocks(xao, mbT, NE, C)

    nm_t = psm.tile([128, 4], F32)
    ni_t = psm.tile([128, 4], F32)
    for m in range(4):
        acc = ps_bigA() if m % 2 == 0 else ps_bigB()
        for k in range(KT):
            nc.tensor.matmul(acc[:, :512], maT[k][:, ts(m, 128)], mbT[k][:],
                             start=(k == 0), stop=(k == KT - 1))
        mx8 = utile(psm, [128, 8], F32, "mx8")
        ix8 = utile(psm, [128, 8], U32, "ix8")
        nc.vector.max_with_indices(mx8[:], ix8[:], acc[:, :512])
        nc.vector.tensor_copy(nm_t[:, m:m + 1], mx8[:, 0:1])
        nc.vector.tensor_copy(ni_t[:, m:m + 1], ix8[:, 0:1])

    nm_row = utile(pB4, [1, 512], F32, "nm_row")
    for m in range(4):
        p = ps_av()
        pf = p.rearrange("p a b -> p (a b)")
        nc.tensor.transpose(pf[:1, :128], nm_t[:, m:m + 1], ident[:])
        nc.vector.tensor_copy(nm_row[:, ts(m, 128)], pf[:1, :128])
    NMB = bcast_row(nm_row[:], 512, "nmb", pB4)

    rank_t = psm.tile([128, 4], F32)
    for m in range(4):
        gt = utile(pB4, [128, 512], F32, "rk_gt", bufs=1)
        nc.vector.tensor_scalar(gt[:], NMB[:], nm_t[:, m:m + 1], None, OP.is_gt)
        eq = utile(pB4, [128, 512], F32, "rk_eq", bufs=1)
        nc.vector.tensor_scalar(eq[:], NMB[:], nm_t[:, m:m + 1], None, OP.is_equal)
        flt = utile(pB4, [128, 512], F32, "rk_flt", bufs=1)
        pio = utile(psm, [128, 1], F32, "rk_pio")
        nc.vector.tensor_scalar_add(pio[:], piota[:], float(128 * m))
        nc.vector.tensor_scalar(flt[:], IOTA512B[:], pio[:], None, OP.is_lt)
        nc.vector.tensor_tensor(eq[:], eq[:], flt[:], OP.mult)
        nc.vector.tensor_tensor(gt[:], gt[:], eq[:], OP.add)
        nc.vector.reduce_sum(rank_t[:, m:m + 1], gt[:], axis=mybir.AxisListType.X)
    if dbg:
        for (tt, nme) in [(nm_t, "dbg_nm"), (rank_t, "dbg_rank"),
                          (ni_t, "dbg_nodeidx")]:
            nc.sync.dma_start(dbg[nme].rearrange("(m p) -> p m", p=128), tt[:])
    pB4.release()
    pB3.release()
    pB2.release()

    # ================= dst merge (x1 from DRAM; dstn -> DRAM) =============
    pM = tc.alloc_tile_pool(name="pM", bufs=1)
    x1e = [utile(pM, [128, C + 8], F32R, f"x1e{m}") for m in range(4)]
    x1o = [utile(pM, [128, C], F32, f"x1o{m}") for m in range(4)]
    for m in range(4):
        nc.vector.tensor_copy(x1e[m][:, C:C + 8], ones_part[:])
        nc.sync.dma_start(x1e[m][:, :C],
                          x1d[256 * m:256 * m + 256:2, :].bitcast(F32R))
        nc.sync.dma_start(x1o[m][:], x1d[256 * m + 1:256 * m + 256:2, :])
    st = [utile(pM, [128, 512], F32R, f"st{m}") for m in range(4)]
    for m in range(4):
        msk = utile(psm, [128, 1], F32, "st_m")
        nc.vector.tensor_scalar(msk[:], rank_t[:, m:m + 1], float(R) - 0.5, None,
                                OP.is_lt)
        nc.vector.tensor_scalar(st[m][:], IOTA512B[:], ni_t[:, m:m + 1], None,
                                OP.is_equal)
        nc.vector.tensor_scalar(st[m][:], st[m][:], msk[:], None, OP.mult)
    for m in range(4):
        acc = ps_bigA()
        cacc = ps_av()
        for k in range(4):
            for n2 in range(2):
                nc.tensor.matmul(acc[:, ts(n2, 512)], st[k][:, ts(m, 128)],
                                 x1e[k][:, n2 * 512:n2 * 512 + 512],
                                 start=(k == 0), stop=(k == 3))
            nc.tensor.matmul(cacc[:, 0, :8], st[k][:, ts(m, 128)],
                             x1e[k][:, C:C + 8], start=(k == 0), stop=(k == 3))
        cnt = utile(psm, [128, 1], F32, "cnt")
        nc.vector.tensor_scalar_add(cnt[:], cacc[:, 0, 0:1], 1.0)
        rec = utile(psm, [128, 1], F32, "cntr")
        nc.vector.reciprocal(rec[:], cnt[:])
        dst_stg = utile(pM, [128, C], F32, "dst_stg", bufs=2)
        nc.vector.tensor_tensor(dst_stg[:], acc[:], x1o[m][:], OP.add)
        nc.vector.tensor_scalar(dst_stg[:], dst_stg[:], rec[:], None, OP.mult)
        nc.sync.dma_start(dstnd[ts(m, 128), :], dst_stg[:])

    # ========== MLP (f32r): W1/W2 streamed once; SBUF out accumulation ======
    def row_src_ap(i):
        if i < 4:
            return x1d[256 * i:256 * i + 256:2, :]
        return dstnd[ts(i - 4, 128), :]

    pM.release()
    pZ = tc.alloc_tile_pool(name="pZ", bufs=1)
    pC4 = tc.alloc_tile_pool(name="pC4", bufs=1)
    g2b = brow("g2", pC4)
    be2b = brow("be2", pC4)
    h2 = []
    for i in range(8):
        rsrc = utile(pt, [128, C], F32, "xin")
        nc.sync.dma_start(rsrc[:], row_src_ap(i))
        h = utile(pC4, [128, C], F32, "ht", bufs=2)
        layer_norm(rsrc, h, g2b, be2b, pC4)
        h2.append(h)
        if dbg:
            nc.sync.dma_start(dbg["dbg_mlpin"][ts(i, 128), :], rsrc[:])
    h2T = [utile(pC4, [128, N], F32R, f"h2T{k}") for k in range(8)]
    transpose_blocks(h2, h2T, N, C)

    bm1T = bcol("bm1", pC4)
    bm2T = bcol("bm2", psm)
    # ---- MLP c-major: Y^T groups of 8 hidden tiles, Z^T accum in SBUF ----
    zacc = [utile(pZ, [128, N], F32R, f"zacc{c}") for c in range(8)]
    for g in range(4):
        ytiles = []
        w1h = None
        for mi in range(8):
            mt = g * 8 + mi
            if mi % 4 == 0:
                w1h = []
                for k in range(KT):
                    w = utile(pC4, [128, 512], F32R, "w1h", bufs=8)
                    nc.sync.dma_start(
                        w[:], wd["W1"][ts(k, 128), ts(2 * g + mi // 4, 512)]
                        .bitcast(F32R))
                    w1h.append(w)
            yp = ps_bigA() if mi % 2 == 0 else ps_bigB()
            for k in range(KT):
                for n2 in range(2):
                    nc.tensor.matmul(yp[:, ts(n2, 512)],
                                     w1h[k][:, ts(mi % 4, 128)],
                                     h2T[k][:, ts(n2, 512)],
                                     start=(k == 0), stop=(k == KT - 1))
            yt = utile(pC4, [128, N], F32R, "yt", bufs=8)
            nc.scalar.activation(yt[:], yp[:], AF.Gelu_apprx_tanh,
                                 bias=bm1T[:, mt:mt + 1])
            ytiles.append(yt)
        for ch in range(2):
            w2h = []
            for mi in range(8):
                mt = g * 8 + mi
                w = utile(pC4, [128, 512], F32R, "w2h", bufs=8)
                nc.sync.dma_start(w[:],
                                  wd["W2"][ts(mt, 128), ts(ch, 512)].bitcast(F32R))
                w2h.append(w)
            for cc in range(4):
                c = ch * 4 + cc
                for n2 in range(2):
                    zp = ps_mm() if n2 == 0 else                         ps_av().rearrange("p a b -> p (a b)")
                    for mi in range(8):
                        nc.tensor.matmul(zp[:, :512],
                                         w2h[mi][:, ts(cc, 128)],
                                         ytiles[mi][:, ts(n2, 512)],
                                         start=(mi == 0), stop=(mi == 7))
                    with nc.allow_low_precision(reason="mlp psums f32r"):
                        if g == 0:
                            nc.vector.tensor_scalar(zacc[c][:, ts(n2, 512)],
                                                    zp[:, :512],
                                                    bm2T[:, c:c + 1], None,
                                                    OP.add)
                        else:
                            nc.vector.tensor_tensor(zacc[c][:, ts(n2, 512)],
                                                    zacc[c][:, ts(n2, 512)],
                                                    zp[:, :512], OP.add)

    # ---- residual + transpose -> row-major mod (bf16, pZ-resident) ----
    mod_sb = [utile(pZ, [128, N], BF16, f"modr{i}") for i in range(8)]
    for i in range(8):
        resf = utile(pt, [128, C], F32, "xin")
        nc.sync.dma_start(resf[:], row_src_ap(i))
        zt = ps_bigA() if i % 2 == 0 else ps_bigB()
        for c in range(8):
            nc.tensor.transpose(zt[:, ts(c, 128)].bitcast(F32R),
                                zacc[c][:, ts(i, 128)], ident_r[:, :])
        with nc.allow_low_precision(reason="mod rows stored bf16"):
            nc.vector.tensor_tensor(mod_sb[i][:], zt[:], resf[:], OP.add)
    if dbg:
        for i in range(8):
            mff = utile(pC4, [128, C], F32, "mof", bufs=2)
            nc.vector.tensor_copy(mff[:], mod_sb[i][:])
            nc.sync.dma_start(dbg["dbg_mlpout"][ts(i, 128), :], mff[:])
    pC4.release()

    # ================= Stage D: pooling + Wp -> combined^T =================
    pD = tc.alloc_tile_pool(name="pD", bufs=1)
    # ApT[p, f] = 0.5 iff source row p pools into output f:
    #   even block: base = rank[p]-16, match iff (2f - base) in {-1, 0}
    #   dst  block: base = d,          match iff (2(f-248) - base) in {-1, 0}
    iota2e = utile(pD, [128, 504], F32, "iota2e")
    nc.vector.tensor_scalar_mul(iota2e[:], IOTA504B[:], 2.0)
    apT = [utile(pD, [128, 504], BF16, f"apT{m}") for m in range(8)]
    for m in range(8):
        base = utile(psm, [128, 1], F32, "ap_r")
        if m < 4:
            nc.vector.tensor_scalar_add(base[:], rank_t[:, m:m + 1], -float(R))
        else:
            nc.vector.tensor_scalar_add(base[:], piota[:],
                                        float(128 * (m - 4) + NE - R))
        d1 = utile(pD, [128, 504], F32, "ap_d1")
        nc.vector.tensor_scalar(d1[:], iota2e[:], base[:], None, OP.subtract)
        a1 = utile(pD, [128, 504], F32, "ap_a1")
        nc.vector.tensor_scalar(a1[:], d1[:], -1.5, None, OP.is_ge)
        b1 = utile(pD, [128, 504], F32, "ap_b1")
        nc.vector.tensor_scalar(b1[:], d1[:], 0.5, None, OP.is_le)
        nc.vector.scalar_tensor_tensor(apT[m][:], a1[:], 0.5, b1[:],
                                       OP.mult, OP.mult)
    # pooledT[c][128c, 504] = sum_i mod_sb[i][:, c-tile]^T @ apT[i]
    pooledT = [utile(pD, [128, NP], F32R, f"pooledT{k}") for k in range(8)]
    for c in range(8):
        zp = ps_mm() if c % 2 == 0 else         ps_av().rearrange("p a b -> p (a b)")
        for i in range(8):
            nc.tensor.matmul(zp[:, :NP], mod_sb[i][:, ts(c, 128)], apT[i][:],
                             start=(i == 0), stop=(i == 7))
        nc.vector.tensor_copy(pooledT[c][:], zp[:, :NP])
    if dbg:
        for m in range(4):
            pst = utile(pt, [128, C], F32, "xin")
            for bj in range(8):
                p = ps_av()
                pf = p.rearrange("p a b -> p (a b)")
                nc.tensor.transpose(pf[:PP, :128].bitcast(F32R),
                                    pooledT[bj][:, m * PP:(m + 1) * PP],
                                    ident_r[:, :])
                nc.vector.tensor_copy(pst[:PP, ts(bj, 128)], pf[:PP, :128])
            nc.sync.dma_start(dbg["dbg_pooled"][ts(m, PP), :], pst[:PP, :])

    pE = tc.alloc_tile_pool(name="pE", bufs=1)
    bp3T = bcol("bp", pD, scale=3.0)
    cmbTr = [utile(pD, [128, NP], F32R, f"cmbTr{m}") for m in range(8)]
    for mg in range(2):
        wcs = []
        for k in range(KT):
            wcr = utile(pD, [128, 512], F32R, f"wpc{k}", bufs=1)
            nc.sync.dma_start(wcr[:], wd["Wp"][ts(k, 128), ts(mg, 512)].bitcast(F32R))
            wcs.append(wcr)
        for mi in range(4):
            m = mg * 4 + mi
            acc = ps_mm()
            for k in range(KT):
                nc.tensor.matmul(acc[:, :NP], wcs[k][:, ts(mi, 128)],
                                 pooledT[k][:], start=(k == 0), stop=(k == KT - 1))
            nc.scalar.activation(cmbTr[m][:], acc[:, :NP], AF.Identity,
                                 bias=bp3T[:, m:m + 1], scale=3.0)

    # ================= Stage E: MQA =================
    bqT = bcol("bq", pE)

    _wq512 = {}

    def make_mqT(m):
        mg = m // 4
        if mg not in _wq512:
            ws = []
            for k in range(KT):
                w = utile(pE, [128, 512], F32R, "wq512", bufs=8)
                nc.sync.dma_start(
                    w[:], wd["Wq"][ts(k, 128), ts(mg, 512)].bitcast(F32R))
                ws.append(w)
            _wq512.clear()
            _wq512[mg] = ws
        ws = _wq512[mg]
        acc = ps_mm()
        for k in range(KT):
            nc.tensor.matmul(acc[:, :NP], ws[k][:, ts(m % 4, 128)], cmbTr[k][:],
                             start=(k == 0), stop=(k == KT - 1))
        t = utile(pE, [128, NP], F32R, "mqT", bufs=2)
        with nc.allow_low_precision(reason="bias add, f32r storage"):
            nc.vector.tensor_scalar(t[:], acc[:, :NP], bqT[:, m:m + 1], None,
                                    OP.add)
        return t

    wkvr = utile(pE, [128, KT, 2 * DH], F32R, "wkvr")
    for k in range(KT):
        nc.sync.dma_start(wkvr[:, k, :DH], wd["Wk"][ts(k, 128), :].bitcast(F32R))
        nc.sync.dma_start(wkvr[:, k, DH:], wd["Wv"][ts(k, 128), :].bitcast(F32R))
    bkT = utile(pE, [64, 1], F32, "bkT")
    nc.sync.dma_start(bkT[:], wd["bk"][:, None])
    mkT = utile(pE, [128, NP], F32R, "mkT")
    macc = ps_mm()
    for k in range(KT):
        nc.tensor.matmul(macc[:64, :NP], wkvr[:, k, :DH], cmbTr[k][:],
                         start=(k == 0), stop=(k == KT - 1))
    mkf = utile(pE, [64, NP], F32, "mkf")
    nc.scalar.activation(mkf[:], macc[:64, :NP], AF.Identity, bias=bkT[:])
    nc.vector.tensor_copy(mkT[:64, :], mkf[:])
    nc.sync.dma_start(mkT[64:, :], mkT[:64, :])

    bvb = bcast_row(load_row(wd["bv"], DH, "bv_r", pE), DH, "bv_b", pE)
    mv_pad = [utile(pE, [128, DH + 1], F32R, f"mvp{m}") for m in range(4)]
    for m in range(4):
        acc = ps_av()
        for k in range(KT):
            nc.tensor.matmul(acc[:PP, 0, :DH], cmbTr[k][:, m * PP:(m + 1) * PP],
                             wkvr[:, k, DH:], start=(k == 0), stop=(k == KT - 1))
        nc.vector.tensor_copy(mv_pad[m][:, DH:], ones_part[:, :1])
        nc.vector.tensor_tensor(mv_pad[m][:PP, :DH], acc[:PP, 0, :DH], bvb[:PP, :],
                                OP.add)

    mqT_cur = None
    for h in range(H):
        po = (h % 2) * 64
        if h % 2 == 0:
            mqT_cur = make_mqT(h // 2)
        mqT_h = mqT_cur[po:po + 64, :]
        spA = ps_bigA()
        spB = ps_bigB()
        ep = []
        for mm in range(4):
            spbig = spA if mm < 2 else spB
            sp = spbig[:, 512 * (mm % 2):512 * (mm % 2) + NP]
            nc.tensor.matmul(sp[:PP, :NP], mkT[po:po + 64, mm * PP:(mm + 1) * PP],
                             mqT_h[:], start=True, stop=True)
            et = utile(pE, [128, NP], F32R, "e2", bufs=6)
            nc.scalar.activation(et[:PP, :], sp[:PP, :NP], AF.Exp,
                                 scale=float(DH ** -0.5))
            ep.append(et)
        av2 = ps_av().rearrange("p a b -> p (a b)")
        for mm in range(4):
            nc.tensor.matmul(av2[:DH + 1, :NP], mv_pad[mm][:PP, :],
                             ep[mm][:PP, :], start=(mm == 0), stop=(mm == 3))
        rrow = utile(pE, [1, NP], F32R, "rrow2", bufs=2)
        with nc.allow_low_precision(reason="softmax denom; 11-bit ok downstream"):
            nc.vector.reciprocal(rrow[:], av2[DH:DH + 1, :NP])
        rb = ps_mm()
        nc.tensor.matmul(rb[:DH, :NP], ones_col_r[:, :DH], rrow[:],
                         start=True, stop=True)
        rbs = utile(pE, [64, NP], F32, "rbs2", bufs=2)
        nc.vector.tensor_copy(rbs[:], rb[:DH, :NP])
        stg = utile(pE, [64, NP], F32R, "mqstg", bufs=2)
        nc.vector.tensor_tensor(stg[:], av2[:DH, :NP], rbs[:], OP.mult)
        nc.sync.dma_start(mqaTd[h * 64:h * 64 + 64, :], stg[:])
    pE.release()
    pD.release()
    pZ.release()

    # ================= Stage F: Wmo + FFN =================
    pF = tc.alloc_tile_pool(name="pF", bufs=1)
    pF1 = tc.alloc_tile_pool(name="pF1", bufs=1)
    mqaT = [utile(pF1, [128, NP], F32R, f"mqaT{k}") for k in range(8)]
    for k in range(8):
        nc.sync.dma_start(mqaT[k][:, :NP], mqaTd[ts(k, 128), :])
    bmoT = bcol("bmo", pF1)
    omoT = [utile(pF, [128, NP], F32R, f"omoT{m}") for m in range(8)]
    for mg in range(2):
        wcs = []
        for k in range(KT):
            wcr = utile(pF1, [128, 512], F32R, f"wmc{k}", bufs=1)
            nc.sync.dma_start(wcr[:],
                              wd["Wmo"][ts(k, 128), ts(mg, 512)].bitcast(F32R))
            wcs.append(wcr)
        for mi in range(4):
            m = mg * 4 + mi
            acc = ps_mm()
            for k in range(KT):
                nc.tensor.matmul(acc[:, :NP], wcs[k][:, ts(mi, 128)],
                                 mqaT[k][:], start=(k == 0), stop=(k == KT - 1))
            nc.scalar.activation(omoT[m][:], acc[:, :NP], AF.Identity,
                                 bias=bmoT[:, m:m + 1])
    pF1.release()

    # ---- FFN c-major: Y halves of 16 hidden tiles, Z accum in SBUF ----
    bf1T = bcol("bf1", pF)
    bf2b = brow("bf2", pF)
    zf = [utile(pF, [128, NP], F32R, f"zf{c}") for c in range(8)]
    for hf in range(2):
        yfs = []
        wf1h = None
        for kki in range(16):
            kk = hf * 16 + kki
            if kki % 4 == 0:
                wf1h = []
                for k in range(KT):
                    w = utile(pF, [128, 512], F32R, "wf1h", bufs=9)
                    nc.sync.dma_start(
                        w[:], wd["Wf1"][ts(k, 128), ts(kk // 4, 512)]
                        .bitcast(F32R))
                    wf1h.append(w)
            yp = ps_mm() if kki % 2 == 0 else             ps_av().rearrange("p a b -> p (a b)")
            for k in range(KT):
                nc.tensor.matmul(yp[:, :NP], wf1h[k][:, ts(kki % 4, 128)],
                                 omoT[k][:], start=(k == 0), stop=(k == KT - 1))
            yf = utile(pF, [128, NP], F32R, "yf", bufs=17)
            nc.scalar.activation(yf[:], yp[:, :NP], AF.Silu,
                                 bias=bf1T[:, kk:kk + 1])
            yfs.append(yf)
        for ch in range(2):
            wf2h = []
            for kki in range(16):
                kk = hf * 16 + kki
                w = utile(pF, [128, 512], F32R, "wf2h", bufs=17)
                nc.sync.dma_start(w[:],
                                  wd["Wf2"][ts(kk, 128), ts(ch, 512)].bitcast(F32R))
                wf2h.append(w)
            for cc in range(4):
                c = ch * 4 + cc
                zp = ps_bigA() if cc % 2 == 0 else ps_bigB()
                for kki in range(16):
                    nc.tensor.matmul(zp[:, :NP], wf2h[kki][:, ts(cc, 128)],
                                     yfs[kki][:], start=(kki == 0),
                                     stop=(kki == 15))
                with nc.allow_low_precision(reason="ffn out partials f32r"):
                    if hf == 0:
                        nc.vector.tensor_copy(zf[c][:], zp[:, :NP])
                    else:
                        nc.vector.tensor_tensor(zf[c][:], zf[c][:], zp[:, :NP],
                                                OP.add)
    # ---- transpose back to row-major + bias -> out ----
    for tl in range(4):
        of = utile(pF, [128, C], F32, "of", bufs=2)
        for c in range(8):
            p = ps_av()
            pf = p.rearrange("p a b -> p (a b)")
            nc.tensor.transpose(pf[:PP, :128].bitcast(F32R),
                                zf[c][:, tl * PP:(tl + 1) * PP], ident_r[:, :])
            nc.vector.tensor_tensor(of[:PP, ts(c, 128)], pf[:PP, :128],
                                    bf2b[:PP, ts(c, 128)], OP.add)
        nc.sync.dma_start(out_d[tl * PP:(tl + 1) * PP, :], of[:PP, :])
    pF.release()
    for pool in (pt, pw, psm, pc, pp):
        pool.release()


_BUILT = None


def kernel(**inputs):
    global _BUILT
    if _BUILT is None:
        _BUILT = build(debug=DEBUG)
    nc = _BUILT
    x = np.ascontiguousarray(inputs["x"], dtype=np.float32)
    base = {k: np.ascontiguousarray(v, dtype=np.float32) for k, v in inputs.items()
            if k != "x"}
    in_maps = []
    for i in range(8):
        m = dict(base)
        m["x"] = x[i]
        in_maps.append(m)
    res = run_bass_kernel_spmd(nc, in_maps, core_ids=list(range(8)))
    out = np.stack([res.results[i]["out"] for i in range(8)], axis=0)
    return out.astype(np.float32)

